# revision 1
# baseline (speedup 1.0000x reference)
"""Trainium2 Bass kernel for nn_AttentionBlock (causal attn, softmax over the
QUERY axis (dim=1), post-softmax 1/sqrt(K) scale, residual add).

Sharding: data-parallel over batch B=8, one batch element per NeuronCore.

Orientation trick: the reference softmax normalizes over the *query* index i
for each key column j.  We compute logits transposed, lT[j, i], so that the
normalization axis i is the SBUF free axis: causal mask = one additive-mask
DVE op on the diagonal 512-chunk (into a dedicated PSUM bank — PSUM banks are
single-port, in-place RMW silently corrupts on HW), column max = per-chunk
DVE reduces, exp + denominator-part = one ScalarE activation per chunk.  The
final read contraction takes the E_T strips directly as the matmul stationary
operand:
    read[i, :] = sum_j E_T[j, i] * V'[j, :],  V' = (v + bv) / (denom sqrt(K))

Raw Block style with manual semaphores: the walrus build in this container
supports at most ONE embedded sync-wait per instruction, so all cross-engine
deps are standalone wait_ge instructions with statically computed thresholds.
Same-engine producer->consumer pairs also need explicit waits (engines
pipeline with no drain between instructions).

Matmul dtype: float32r (single-pass fp32, 1 cycle/row at N=512 vs 4 for
exact fp32) for projections + logits; bf16 for the probability-weighted read.
The residual (+x) is added on the host.

Pipeline (per core): x loads are chunked by i so projection groups start
~5us in; the PE runs v, k, q projection groups, then interleaves logits
chunk-groups L(jt) with read groups R(jt-1); DVE does mask/max/recip/scale;
ACT does projection-evacuation-with-bias and exp+rowsum.  Output tiles are
evacuated two iterations late so the DVE chain of jt is never gated on R(jt).
"""

import math
import os
import sys

import numpy as np
import ml_dtypes

for _p in ("/opt/trn_rl_repo", "/root/.axon_site/_ro/trn_rl_repo"):
    if os.path.isdir(_p) and _p not in sys.path:
        sys.path.append(_p)

import concourse.bass as bass
from concourse import mybir
from concourse.bass_utils import run_bass_kernel_spmd

B = 8
D = 512
KS = 512
ND = D // 128  # 4 contraction tiles

F32 = mybir.dt.float32
F32R = mybir.dt.float32r
BF16 = mybir.dt.bfloat16
AOP = mybir.AluOpType
AFT = mybir.ActivationFunctionType

INV_SQRT_K = 1.0 / math.sqrt(KS)
FLT_MIN = float(np.finfo(np.float32).min)

TRACE = False
LAST_RESULTS = None
MMDT = F32R


def _c0(jt):
    return (128 * jt) // 512


def build_nc(T=2048, mmdt=None, debug_dump=False):
    if mmdt is None:
        mmdt = MMDT
    NT = T // 128
    NCH = T // 512
    KQ = ND * NCH  # projection output groups for each of q/k

    nc = bass.Bass("TRN2", target_bir_lowering=False, debug=False, num_devices=B)

    # fp32r reads raw IEEE fp32 bits (keeping ~12 mantissa bits), so the
    # host feeds fp32 bytes straight into f32r tensors — no rounding pass
    xT_d = nc.dram_tensor("xT", [D, T], mmdt, kind="ExternalInput")
    wq_d = nc.dram_tensor("wqT", [D, KS], mmdt, kind="ExternalInput")
    wk_d = nc.dram_tensor("wkT", [D, KS], mmdt, kind="ExternalInput")
    wv_d = nc.dram_tensor("wvT", [D, KS], mmdt, kind="ExternalInput")
    bq_d = nc.dram_tensor("bq", [KS], F32, kind="ExternalInput")
    bk_d = nc.dram_tensor("bk", [KS], F32, kind="ExternalInput")
    bv_d = nc.dram_tensor("bv", [KS], F32, kind="ExternalInput")
    ma_d = nc.dram_tensor("madd", [128, 4, 512], BF16, kind="ExternalInput")
    out_d = nc.dram_tensor("out", [T, KS], F32, kind="ExternalOutput")
    if debug_dump:
        de_d = nc.dram_tensor("dbg_e", [128, NT, T], BF16, kind="ExternalOutput")
        dv_d = nc.dram_tensor("dbg_vp", [128, NT, KS], BF16, kind="ExternalOutput")
        dq_d = nc.dram_tensor("dbg_q", [128, ND, T], F32, kind="ExternalOutput")
        dk_d = nc.dram_tensor("dbg_k", [128, ND, T], F32, kind="ExternalOutput")

    # ---- SBUF ----
    xTr = nc.alloc_sbuf_tensor("xTr", [128, ND, T], mmdt)
    wvr = nc.alloc_sbuf_tensor("wvr", [128, ND, KS], mmdt)
    wkr = nc.alloc_sbuf_tensor("wkr", [128, ND, KS], mmdt)
    wqr = nc.alloc_sbuf_tensor("wqr", [128, ND, KS], mmdt)
    kT = nc.alloc_sbuf_tensor("kT", [128, ND, T], mmdt)
    qT = nc.alloc_sbuf_tensor("qT", [128, ND, T], mmdt)
    v_sb = nc.alloc_sbuf_tensor("v_sb", [128, NT, KS], BF16)
    vp_sb = nc.alloc_sbuf_tensor("vp_sb", [128, NT, KS], BF16)
    e_sbs = [
        nc.alloc_sbuf_tensor(f"e{jt}", [128, T - 128 * jt], BF16)
        for jt in range(NT)
    ]
    outst = nc.alloc_sbuf_tensor("outst", [128, 2, KS], F32)
    lm = nc.alloc_sbuf_tensor("lm", [128, 2, 512], F32)
    madd = nc.alloc_sbuf_tensor("madd_sb", [128, 4, 512], BF16)
    bqc = nc.alloc_sbuf_tensor("bqc", [128, ND], F32)
    bkc = nc.alloc_sbuf_tensor("bkc", [128, ND], F32)
    bvb = nc.alloc_sbuf_tensor("bvb", [128, KS], F32)
    macc = nc.alloc_sbuf_tensor("macc", [128, 2, NCH], F32)
    negmax = nc.alloc_sbuf_tensor("negmax", [128, 2], F32)
    dparts = nc.alloc_sbuf_tensor("dparts", [128, 2, NCH], F32)
    denom = nc.alloc_sbuf_tensor("denom", [128, 2], F32)
    dscr = nc.alloc_sbuf_tensor("dscr", [128, 4], F32)
    rec = nc.alloc_sbuf_tensor("rec", [128, 2], F32)

    # ---- PSUM: 8 banks of [128, 512] fp32 ----
    ps = [nc.alloc_psum_tensor(f"ps{i}", [128, 512], F32) for i in range(8)]
    # phase 1: v -> ps[0..1], k/q -> ps[2..5]
    # phase 2: logits chunks -> ps[g % 6]; masked diag goes to the SBUF
    # strip lm[:, jt%2] (PSUM banks are single-port: no in-place RMW);
    # read -> ps[6 + jt % 2]

    # ================= static op-index tables =================
    def _la2(j):
        return 2

    # ---- DVE plan (sDV counts every DVE op) ----
    MADD, REDL, NMX, RECIP = {}, {}, {}, {}
    dv = 0
    VCP = {}
    for jt in range(NT):
        dv += 1
        VCP[jt] = dv
    for jt in range(NT):
        nch = NCH - _c0(jt)
        dv += 1
        MADD[jt] = dv
        if nch == 1:
            dv += 1
            NMX[jt] = dv
        else:
            dv += nch
            REDL[jt] = dv
            dv += 1
            NMX[jt] = dv
        dv += 1
        RECIP[jt] = dv

    # ---- ACT plan (sAC): kq copies, then per jt block:
    #      exps, rsum (nch>1), vp, due outcopies ----
    EXP, RSUMA, VPA, OCPA = {}, {}, {}, {}
    oc_due = {}
    for j in range(NT):
        oc_due.setdefault(j + _la2(j), []).append(j)
    ac = 2 * KQ
    for jt in range(NT):
        nch = NCH - _c0(jt)
        for j in oc_due.get(jt, []):
            ac += 1
            OCPA[j] = ac
        for ic in list(range(_c0(jt) + 1, NCH)) + [_c0(jt)]:
            ac += 1
            EXP[(jt, ic)] = ac
        if nch > 1:
            ac += 1
            RSUMA[jt] = ac
        ac += 1
        VPA[jt] = ac
    for j in range(NT):
        if j + _la2(j) >= NT:
            ac += 1
            OCPA[j] = ac
    EXP_END = {jt: EXP[(jt, _c0(jt))] for jt in range(NT)}  # diag emitted last

    # ---- PE plan (sPE counts GROUPS) ----
    # phase 1 emitted per x-chunk batch: v(4ic..4ic+3), k(:,ic), q(:,ic)
    VG, KG, QG = {}, {}, {}
    KQSEQ = {}  # interleaved k/q copy sequence index (ACT order)
    pe = 0
    kqseq = 0
    p1_order = []
    for ic in range(NCH):
        for jt in range(4 * ic, min(4 * (ic + 1), NT)):
            pe += 1
            VG[jt] = pe
            p1_order.append(("v", jt))
        for kt in range(ND):
            pe += 1
            KG[(kt, ic)] = pe
            kqseq += 1
            KQSEQ[("k", kt, ic)] = kqseq
            p1_order.append(("k", kt, ic))
        for kt in range(ND):
            pe += 1
            QG[(kt, ic)] = pe
            kqseq += 1
            KQSEQ[("q", kt, ic)] = kqseq
            p1_order.append(("q", kt, ic))
    assert pe == NT + 2 * KQ
    LG, RG = {}, {}

    _la = _la2
    fused_order = []
    for m in range(NT):
        fused_order.append(("L", m))
        for j in range(NT):
            if j + _la(j) == m:
                fused_order.append(("R", j))
    for j in range(NT):
        if j + _la(j) >= NT:
            fused_order.append(("R", j))
    gctr = 0
    chunk_of_g = {}
    for kind, jt in fused_order:
        if kind == "L":
            for ic in range(_c0(jt), NCH):
                pe += 1
                LG[(jt, ic)] = pe
                chunk_of_g[gctr] = (jt, ic)
                gctr += 1
        else:
            pe += 1
            RG[jt] = pe
    CUMCH = {}
    cc = 0
    for jt in range(NT):
        cc += NCH - _c0(jt)
        CUMCH[jt] = cc

    with (
        nc.semaphore("sLv") as sLv,
        nc.semaphore("sLk") as sLk,
        nc.semaphore("sLq") as sLq,
        nc.semaphore("sLc") as sLc,
        nc.semaphore("sLx0") as sLx0,
        nc.semaphore("sLx1") as sLx1,
        nc.semaphore("sLx2") as sLx2,
        nc.semaphore("sLx3") as sLx3,
        nc.semaphore("sPE") as sPE,
        nc.semaphore("sDV") as sDV,
        nc.semaphore("sAC") as sAC,
        nc.semaphore("sST") as sST,
        nc.Block() as block,
    ):
        sLxs = [sLx0, sLx1, sLx2, sLx3]

        @block.sync
        def _(sp):
            def ldx(ic):
                sp.dma_start(
                    out=xTr[:, :, 512 * ic : 512 * (ic + 1)],
                    in_=xT_d.ap()[:, 512 * ic : 512 * (ic + 1)].rearrange(
                        "(t p) i -> p t i", p=128
                    ),
                ).then_inc(sLxs[ic], 16)

            # load order tracks first use: wv, x0, wk, wq, consts, x1..x3
            sp.dma_start(
                out=wvr[:, :, :],
                in_=wv_d.ap().rearrange("(t p) k -> p t k", p=128),
            ).then_inc(sLv, 16)
            sp.dma_start(
                out=wkr[:, :, :],
                in_=wk_d.ap().rearrange("(t p) k -> p t k", p=128),
            ).then_inc(sLk, 16)
            ldx(0)
            sp.dma_start(
                out=wqr[:, :, :],
                in_=wq_d.ap().rearrange("(t p) k -> p t k", p=128),
            ).then_inc(sLq, 16)
            with nc.allow_non_contiguous_dma(reason="16B/partition bias loads"):
                sp.dma_start(
                    out=bqc[:, :], in_=bq_d.ap().rearrange("(t p) -> p t", p=128)
                ).then_inc(sLc, 16)
                sp.dma_start(
                    out=bkc[:, :], in_=bk_d.ap().rearrange("(t p) -> p t", p=128)
                ).then_inc(sLc, 16)
            bv_ap = bv_d.ap()
            bv_bcast = bass.AP(
                tensor=bv_ap.tensor, offset=bv_ap.offset, ap=[[0, 128]] + list(bv_ap.ap)
            )
            sp.dma_start(out=bvb[:, :], in_=bv_bcast).then_inc(sLc, 16)
            sp.dma_start(out=madd[:, :, :], in_=ma_d.ap()).then_inc(sLc, 16)
            for ic in range(1, NCH):
                ldx(ic)
            # stores
            out_ap = out_d.ap()
            for jt in range(NT):
                sp.wait_ge(sAC, OCPA[jt])
                sp.dma_start(
                    out=out_ap[128 * jt : 128 * (jt + 1), :],
                    in_=outst[:, jt % 2, :],
                ).then_inc(sST, 16)
            if debug_dump:
                sp.wait_ge(sAC, EXP_END[NT - 1])
                for jt in range(NT):
                    sp.dma_start(
                        out=de_d.ap()[:, jt, 128 * jt : T], in_=e_sbs[jt][:, :]
                    ).then_inc(sST, 16)
                sp.wait_ge(sAC, OCPA[NT - 1])
                sp.dma_start(out=dv_d.ap(), in_=vp_sb[:, :, :]).then_inc(sST, 16)
                sp.dma_start(out=dq_d.ap(), in_=qT[:, :, :].bitcast(F32)).then_inc(
                    sST, 16
                )
                sp.dma_start(out=dk_d.ap(), in_=kT[:, :, :].bitcast(F32)).then_inc(
                    sST, 16
                )
                sp.wait_ge(sST, 16 * (NT + NT + 3))
            else:
                sp.wait_ge(sST, 16 * NT)

        @block.vector
        def _(ve):
            ndv = 0  # running op index, asserted against the plan

            def inc(x):
                nonlocal ndv
                ndv += 1
                x.then_inc(sDV, 1)

            # v strips: psum + bv -> bf16
            ve.wait_ge(sLc, 64)
            for jt in range(NT):
                ve.wait_ge(sPE, VG[jt])
                inc(
                    ve.tensor_tensor(
                        out=v_sb[:, jt, :], in0=ps[jt % 2][:, :],
                        in1=bvb[:, :], op=AOP.add,
                    )
                )
                assert ndv == VCP[jt]

            # fused loop
            gbank = {}
            g = 0
            for jt in range(NT):
                for ic in range(_c0(jt), NCH):
                    gbank[(jt, ic)] = ps[g % 6]
                    g += 1
            for jt in range(NT):
                c0 = _c0(jt)
                off = jt - 4 * c0
                nch = NCH - c0
                if jt >= 2:
                    # negmax/macc[jt%2] reuse: exp(jt-2) must have read them
                    ve.wait_ge(sAC, EXP_END[jt - 2])
                # causal mask on the (narrowed) diagonal chunk into the
                # SBUF strip lm[jt%2] — NOT in place (single-port PSUM);
                # narrowed to start at i=128*jt, so mask class 0 applies;
                # lm[jt%2] reuse is covered by the EXP_END[jt-2] wait above
                w0 = 512 * (c0 + 1) - 128 * jt
                ve.wait_ge(sPE, LG[(jt, c0)])
                inc(
                    ve.tensor_tensor(
                        out=lm[:, jt % 2, 0:w0], in0=gbank[(jt, c0)][:, 0:w0],
                        in1=madd[:, 0, 0:w0], op=AOP.add,
                    )
                )
                assert ndv == MADD[jt]
                if nch == 1:
                    ve.wait_ge(sDV, MADD[jt])  # same-engine RAW fence
                    inc(
                        ve.reduce_max(
                            negmax[:, jt % 2 : jt % 2 + 1], lm[:, jt % 2, 0:w0],
                            mybir.AxisListType.X, negate=True,
                        )
                    )
                    assert ndv == NMX[jt]
                else:
                    # per-chunk column maxes; diagonal chunk reduced LAST and
                    # fenced against the mask-add that wrote its bank
                    for ic in list(range(c0 + 1, NCH)) + [c0]:
                        if ic != c0:
                            ve.wait_ge(sPE, LG[(jt, ic)])
                            src_bank = gbank[(jt, ic)][:, :]
                        else:
                            ve.wait_ge(sDV, MADD[jt])
                            src_bank = lm[:, jt % 2 : jt % 2 + 1, 0:w0]
                        inc(
                            ve.reduce_max(
                                macc[:, jt % 2, ic : ic + 1], src_bank,
                                mybir.AxisListType.X,
                            )
                        )
                    assert ndv == REDL[jt]
                    ve.wait_ge(sDV, REDL[jt])  # same-engine RAW fence
                    inc(
                        ve.reduce_max(
                            negmax[:, jt % 2 : jt % 2 + 1],
                            macc[:, jt % 2, c0:NCH],
                            mybir.AxisListType.X, negate=True,
                        )
                    )
                    assert ndv == NMX[jt]
                if nch == 1:
                    ve.wait_ge(sAC, EXP_END[jt])
                    src = dparts[:, jt % 2, c0 : c0 + 1]
                else:
                    # ACT's rsum accumulated the denominator; its index also
                    # covers the rec[jt%2] reuse (vp(jt-2) is ACT, earlier)
                    ve.wait_ge(sAC, RSUMA[jt])
                    src = denom[:, jt % 2 : jt % 2 + 1]
                inc(ve.reciprocal(rec[:, jt % 2 : jt % 2 + 1], src))
                assert ndv == RECIP[jt]

        @block.scalar
        def _(ac_):
            ac_.wait_ge(sLc, 64)
            for ic in range(NCH):
                for wsel, g_tab, bias in ((0, KG, bkc), (1, QG, bqc)):
                    dst = kT if wsel == 0 else qT
                    for kt in range(ND):
                        seq = KQSEQ[("k" if wsel == 0 else "q", kt, ic)]
                        ac_.wait_ge(sPE, g_tab[(kt, ic)])
                        bank = ps[2 + ((seq - 1) % 4)][:, :]
                        ac_.activation(
                            out=dst[:, kt, 512 * ic : 512 * (ic + 1)],
                            in_=bank,
                            func=AFT.Identity,
                            bias=bias[:, kt : kt + 1],
                            scale=1.0,
                        ).then_inc(sAC, 1)
            # per-jt: exp strips, denominator sum, V' scale, due outcopies
            oc_due2 = {}
            for j in range(NT):
                oc_due2.setdefault(j + _la2(j), []).append(j)

            def outcopy(j):
                ac_.wait_ge(sPE, RG[j])
                if j >= 2:
                    # all stores issued so far must be complete (HWDGE queues
                    # finish out of order; partial counts can't pin which)
                    ac_.wait_ge(sST, 16 * j)
                ac_.activation(
                    out=outst[:, j % 2, :], in_=ps[6 + j % 2][:, :], func=AFT.Copy
                ).then_inc(sAC, 1)

            gbank2 = {}
            g = 0
            for jt in range(NT):
                for ic in range(_c0(jt), NCH):
                    gbank2[(jt, ic)] = ps[g % 6]
                    g += 1
            for jt in range(NT):
                c0 = _c0(jt)
                nch = NCH - c0
                w0 = 512 * (c0 + 1) - 128 * jt
                for j in oc_due2.get(jt, []):
                    outcopy(j)
                first = True
                for ic in list(range(c0 + 1, NCH)) + [c0]:
                    if ic == c0:
                        bank = lm[:, jt % 2 : jt % 2 + 1, 0:w0]
                        eslice = e_sbs[jt][:, 0:w0]
                    else:
                        bank = gbank2[(jt, ic)][:, :]
                        eslice = e_sbs[jt][
                            :, 512 * ic - 128 * jt : 512 * (ic + 1) - 128 * jt
                        ]
                    if first:
                        ac_.wait_ge(sDV, NMX[jt])
                        if jt >= 2 and (NCH - _c0(jt - 2)) == 1:
                            # dparts[jt%2] was read by DVE recip(jt-2)
                            ac_.wait_ge(sDV, RECIP[jt - 2])
                        first = False
                    ac_.activation(
                        out=eslice,
                        in_=bank,
                        func=AFT.Exp,
                        bias=negmax[:, jt % 2 : jt % 2 + 1],
                        scale=1.0,
                        accum_out=dparts[:, jt % 2, ic : ic + 1],
                    ).then_inc(sAC, 1)
                if nch > 1:
                    # denominator = sum of the per-chunk exp sums (Copy+accum)
                    ac_.wait_ge(sAC, EXP_END[jt])  # same-engine RAW fence
                    ac_.activation(
                        out=dscr[:, 0:nch],
                        in_=dparts[:, jt % 2, c0:NCH],
                        func=AFT.Copy,
                        accum_out=denom[:, jt % 2 : jt % 2 + 1],
                    ).then_inc(sAC, 1)
                # V' = v * (1/denom); 1/sqrt(K) is folded into Wv on the host
                ac_.wait_ge(sDV, RECIP[jt])
                ac_.activation(
                    out=vp_sb[:, jt, :], in_=v_sb[:, jt, :], func=AFT.Copy,
                    scale=rec[:, jt % 2 : jt % 2 + 1],
                ).then_inc(sAC, 1)
            for j in range(NT):
                if j + _la2(j) >= NT:
                    outcopy(j)

        @block.tensor
        def _(te):
            # phase 1 per x-chunk batch: v(4ic..4ic+3), k(:,ic), q(:,ic)
            waited = set()

            def ldwait(sem):
                if sem not in waited:
                    te.wait_ge(sem, 16)
                    waited.add(sem)

            for item in p1_order:
                if item[0] == "v":
                    jt = item[1]
                    ldwait(sLv)
                    ldwait(sLxs[jt // 4])
                    if jt >= 2:
                        te.wait_ge(sDV, VCP[jt - 2])
                    for dt_ in range(ND):
                        mm = te.matmul(
                            ps[jt % 2][:, :],
                            lhsT=xTr[:, dt_, 128 * jt : 128 * (jt + 1)],
                            rhs=wvr[:, dt_, :],
                            start=(dt_ == 0),
                            stop=(dt_ == ND - 1),
                        )
                        if dt_ == ND - 1:
                            mm.then_inc(sPE, 1)
                else:
                    kind, kt, ic = item
                    wsb = wkr if kind == "k" else wqr
                    ldwait(sLk if kind == "k" else sLq)
                    ldwait(sLxs[ic])
                    seq = KQSEQ[(kind, kt, ic)]
                    if seq > 4:
                        te.wait_ge(sAC, seq - 4)
                    for dt_ in range(ND):
                        mm = te.matmul(
                            ps[2 + ((seq - 1) % 4)][:, :],
                            lhsT=wsb[:, dt_, 128 * kt : 128 * (kt + 1)],
                            rhs=xTr[:, dt_, 512 * ic : 512 * (ic + 1)],
                            start=(dt_ == 0),
                            stop=(dt_ == ND - 1),
                        )
                        if dt_ == ND - 1:
                            mm.then_inc(sPE, 1)
            # fused: logits chunk groups + read groups
            g = 0
            for kind, jt in fused_order:
                c0 = _c0(jt)
                if kind == "L":
                    for ic in range(c0, NCH):
                        need_ac = KQSEQ[("q", ND - 1, ic)]  # q copies thru ic
                        need_dv = None
                        if g >= 6:
                            pj, pic = chunk_of_g[g - 6]
                            if pic == _c0(pj):
                                # diag bank is released by its mask-add
                                need_dv = MADD[pj]
                            else:
                                need_ac = max(need_ac, EXP[(pj, pic)])
                        elif g % 6 >= 2:
                            need_ac = max(need_ac, 2 * KQ)
                        te.wait_ge(sAC, need_ac)
                        if need_dv is not None:
                            te.wait_ge(sDV, need_dv)
                        if g < 2:
                            # banks 0,1 last used by the v-copy stream (DVE)
                            te.wait_ge(sDV, VCP[NT - 2 + g])
                        bank = ps[g % 6]
                        g += 1
                        # diagonal chunk: only columns i >= 128*jt are valid
                        ilo = 128 * jt if ic == c0 else 512 * ic
                        for kt in range(ND):
                            mm = te.matmul(
                                bank[:, 0 : 512 * (ic + 1) - ilo],
                                lhsT=kT[:, kt, 128 * jt : 128 * (jt + 1)],
                                rhs=qT[:, kt, ilo : 512 * (ic + 1)],
                                start=(kt == 0),
                                stop=(kt == ND - 1),
                            )
                            if kt == ND - 1:
                                mm.then_inc(sPE, 1)
                else:
                    # early MMs only need strips/vp of j2 <= jt-1; the final
                    # MM (j2 == jt) additionally needs this jt's E and V'
                    need = VPA[jt - 1] if jt >= 1 else 0
                    if jt >= 2:
                        need = max(need, OCPA[jt - 2])
                    if need:
                        te.wait_ge(sAC, need)
                    for j2 in range(jt + 1):
                        if j2 == jt:
                            te.wait_ge(sAC, VPA[jt])
                        i0 = 128 * j2
                        mm = te.matmul(
                            ps[6 + jt % 2][:, :],
                            lhsT=e_sbs[j2][:, 128 * jt - i0 : 128 * (jt + 1) - i0],
                            rhs=vp_sb[:, j2, :],
                            start=(j2 == 0),
                            stop=(j2 == jt),
                        )
                        if j2 == jt:
                            mm.then_inc(sPE, 1)

    nc.finalize()
    return nc


def _host_inputs(xb, wqT, wkT, wvT, bq, bk, bv, T):
    # additive causal mask for the diagonal chunk, per offset class o:
    # madd[p, o, x] = 0 where x >= 128*o + p else -3e38
    p = np.arange(128, dtype=np.float32)
    xx = np.arange(512, dtype=np.float32)[None, None, :]
    thr = (p[:, None, None] + 128.0 * np.arange(4, dtype=np.float32)[None, :, None])
    madd = np.where(xx >= thr, 0.0, -3.0e38).astype(ml_dtypes.bfloat16)
    return dict(
        xT=np.ascontiguousarray(xb.T),
        wqT=wqT,
        wkT=wkT,
        wvT=wvT,
        bq=bq,
        bk=bk,
        bv=bv,
        madd=np.ascontiguousarray(madd),
    )


def kernel(x, Wk, bk, Wq, bq, Wv, bv):
    global LAST_RESULTS
    T = 2048
    x = np.ascontiguousarray(np.asarray(x, dtype=np.float32))
    Wk = np.asarray(Wk, dtype=np.float32)
    Wq = np.asarray(Wq, dtype=np.float32)
    Wv = np.asarray(Wv, dtype=np.float32)
    bk = np.ascontiguousarray(np.asarray(bk, dtype=np.float32))
    bq = np.ascontiguousarray(np.asarray(bq, dtype=np.float32))
    bv = np.ascontiguousarray(np.asarray(bv, dtype=np.float32))

    wqT = np.ascontiguousarray(Wq.T)
    wkT = np.ascontiguousarray(Wk.T)
    # fold the post-softmax 1/sqrt(K) into the V projection
    wvT = np.ascontiguousarray(Wv.T * np.float32(INV_SQRT_K))
    bv = np.ascontiguousarray(bv * np.float32(INV_SQRT_K))

    nc = build_nc(T, MMDT)
    in_maps = [_host_inputs(x[b], wqT, wkT, wvT, bq, bk, bv, T) for b in range(B)]
    res = None
    last_exc = None
    for attempt in range(3):
        try:
            res = run_bass_kernel_spmd(nc, in_maps, list(range(B)), trace=TRACE)
            break
        except Exception as e:  # transient NRT device errors; retry fresh
            last_exc = e
            import time as _time
            _time.sleep(10)
            nc = build_nc(T, MMDT)
    if res is None:
        raise last_exc
    LAST_RESULTS = res
    read = np.stack([np.asarray(res.results[b]["out"]) for b in range(B)], axis=0)
    # residual add on host (elementwise, ~0.1% of the FLOPs)
    return (x + read).astype(np.float32)



# revision 2
# speedup vs baseline: 1.1922x; 1.1922x over previous
"""Trainium2 Bass kernel for nn_AttentionBlock (causal attn, softmax over the
QUERY axis (dim=1), post-softmax 1/sqrt(K) scale, residual add).

Sharding: data-parallel over batch B=8, one batch element per NeuronCore.

v4 design:
- K/Q projections + logits in fp16; V projection and the probability-weighted
  read in fp8e4 with MatmulPerfMode.DoubleRow (two 128-deep contraction tiles
  per matmul).
- The causal mask is applied BY THE PE: each diagonal logits group gets one
  extra 128-wide matmul (identity stationary x f16 mask pattern of 0/-60000)
  accumulated into the PSUM bank.  exp() of -60000-ish underflows to exactly
  0.  No DVE mask-add, no staging: every chunk is max-reduced and exp'd
  straight from its PSUM bank.
- Logits PSUM banks live in ONE 6-bank tensor (pse) plus a 2-bank read tensor
  (psRD).  The first 8 logits chunks use all 8 banks (prologue), then a
  6-bank rotation.  Adjacent-bank non-diag chunk runs are processed by SINGLE
  wide ops: one DVE reduce yields 2-3 column maxes, one ACT exp covers 2-3
  chunks (one f32 accum partial per op).
- Denominator combine + reciprocal on DVE; V'-scales split DVE (even jt) /
  ACT (odd jt); output evacuation is PAIRED: one op copies both read banks
  [128,2,512] -> bf16 outst, one DMA stores 256 output rows; pairs alternate
  ACT/DVE.  Pool does only the startup memsets.
- E8[j,i] strips live in PAIRED key-chunk layout e8s[m][:, slot, :]
  (slot = jt%2, strip base column 256m) feeding DoubleRow reads directly;
  slot-1's first 128 columns are memset 0 (sub-diagonal).
    read[i, :] = sum_j E[j, i] * V'[j, :],  V' = (v + bv) * rec_j / sqrt(K)

Raw Block style with manual semaphores: ONE embedded sync-wait per
instruction; cross-engine deps are standalone wait_ge with statically
computed thresholds; same-engine RAW pairs get explicit fences.
"""

import math
import os
import sys

import numpy as np
import ml_dtypes

for _p in ("/opt/trn_rl_repo", "/root/.axon_site/_ro/trn_rl_repo"):
    if os.path.isdir(_p) and _p not in sys.path:
        sys.path.append(_p)

import concourse.bass as bass
from concourse import mybir
from concourse.bass_utils import run_bass_kernel_spmd

B = 8
D = 512
KS = 512
ND = D // 128  # 4 contraction tiles

F32 = mybir.dt.float32
F16 = mybir.dt.float16
BF16 = mybir.dt.bfloat16
F8 = mybir.dt.float8e4
AOP = mybir.AluOpType
AFT = mybir.ActivationFunctionType
DR = mybir.MatmulPerfMode.DoubleRow

INV_SQRT_K = 1.0 / math.sqrt(KS)
MASKVAL = -60000.0  # fits f16; exp(-60000 + max) == 0 exactly

TRACE = False
LAST_RESULTS = None


def _c0(jt):
    return (128 * jt) // 512


def build_nc(T=2048, debug_dump=False):
    NT = T // 128   # 16 row chunks
    NCH = T // 512  # 4 column chunks
    NP = NT // 2    # 8 key-chunk pairs
    KQ = ND * NCH   # 16 projection output groups for each of q/k

    nc = bass.Bass("TRN2", target_bir_lowering=False, debug=False, num_devices=B)

    # ---- DRAM ----
    x16_d = nc.dram_tensor("x16", [D, T], F16, kind="ExternalInput")
    x8_d = nc.dram_tensor("x8", [128, 2, 2, T], F8, kind="ExternalInput")
    wq_d = nc.dram_tensor("wq16", [D, KS], F16, kind="ExternalInput")
    wk_d = nc.dram_tensor("wk16", [D, KS], F16, kind="ExternalInput")
    wv_d = nc.dram_tensor("wv8", [128, 2, 2, KS], F8, kind="ExternalInput")
    bq_d = nc.dram_tensor("bq", [KS], F32, kind="ExternalInput")
    bk_d = nc.dram_tensor("bk", [KS], F32, kind="ExternalInput")
    bv_d = nc.dram_tensor("bv", [KS], F32, kind="ExternalInput")
    msk_d = nc.dram_tensor("msk16", [128, 128], F16, kind="ExternalInput")
    idn_d = nc.dram_tensor("idn16", [128, 128], F16, kind="ExternalInput")
    out_d = nc.dram_tensor("out", [T, KS], BF16, kind="ExternalOutput")

    # ---- SBUF ----
    xTr = nc.alloc_sbuf_tensor("xTr", [128, ND, T], F16)
    x8s = nc.alloc_sbuf_tensor("x8s", [128, 2, 2, T], F8)
    wkr = nc.alloc_sbuf_tensor("wkr", [128, ND, KS], F16)
    wqr = nc.alloc_sbuf_tensor("wqr", [128, ND, KS], F16)
    wv8 = nc.alloc_sbuf_tensor("wv8s", [128, 2, 2, KS], F8)
    kT = nc.alloc_sbuf_tensor("kT", [128, ND, T], F16)
    qT = nc.alloc_sbuf_tensor("qT", [128, ND, T], F16)
    v_sb = nc.alloc_sbuf_tensor("v_sb", [128, NT, KS], BF16)
    vp8 = nc.alloc_sbuf_tensor("vp8", [128, NP, 2, KS], F8)
    e8s = [
        nc.alloc_sbuf_tensor(f"e8_{m}", [128, 2, T - 256 * m], F8)
        for m in range(NP)
    ]
    outst = nc.alloc_sbuf_tensor("outst", [128, 2, 2, KS], BF16)
    mask16 = nc.alloc_sbuf_tensor("mask16", [128, 128], F16)
    ident16 = nc.alloc_sbuf_tensor("ident16", [128, 128], F16)
    bqc = nc.alloc_sbuf_tensor("bqc", [128, ND], F32)
    bkc = nc.alloc_sbuf_tensor("bkc", [128, ND], F32)
    bvb = nc.alloc_sbuf_tensor("bvb", [128, KS], F32)
    macc = nc.alloc_sbuf_tensor("macc", [128, 2, NCH], F32)
    negmax = nc.alloc_sbuf_tensor("negmax", [128, 2], F32)
    dparts = nc.alloc_sbuf_tensor("dparts", [128, 2, NCH], F32)
    denom = nc.alloc_sbuf_tensor("denom", [128, 2], F32)
    rec = nc.alloc_sbuf_tensor("rec", [128, 2], F32)

    # ---- PSUM: 6-bank logits rotation + 2 read banks ----
    pse = nc.alloc_psum_tensor("pse", [128, 6, 512], F32)
    psRD = nc.alloc_psum_tensor("psRD", [128, 2, 512], F32)
    # phase 1: v -> pse[0..1], k/q -> pse[2..5]

    # ================= static op-index tables =================
    LA = 4  # read lookahead: R(j) sits at fused position j + LA

    def bank_of_g(g):
        if g < 6:
            return ("E", g)
        if g < 8:
            return ("R", g - 6)
        return ("E", (g - 8) % 6)

    def prev_user_g(g):
        if g < 8:
            return None
        return g - 8 if g < 14 else g - 6

    gbank = {}
    g_of_chunk = {}
    chunk_of_g = {}
    g = 0
    for jt in range(NT):
        for ic in range(_c0(jt), NCH):
            gbank[(jt, ic)] = bank_of_g(g)
            g_of_chunk[(jt, ic)] = g
            chunk_of_g[g] = (jt, ic)
            g += 1

    # non-diag chunk groups per row: maximal runs of adjacent banks
    row_groups = {}
    for jt in range(NT):
        c0 = _c0(jt)
        ics = list(range(c0 + 1, NCH))
        groups = []
        i = 0
        while i < len(ics):
            run = [ics[i]]
            while (
                i + 1 < len(ics)
                and len(run) < 3
                and gbank[(jt, ics[i + 1])][0] == gbank[(jt, run[0])][0]
                and gbank[(jt, ics[i + 1])][1]
                == gbank[(jt, run[0])][1] + len(run)
            ):
                run.append(ics[i + 1])
                i += 1
            groups.append(run)
            i += 1
        row_groups[jt] = groups

    # ---- PE plan (sPE counts GROUPS) ----
    VG, KG, QG, KQSEQ, LG, RG = {}, {}, {}, {}, {}, {}
    pe = 0
    kqseq = 0
    p1_order = []
    for b in range(NCH):
        for r in range(4):
            pe += 1
            KG[(r, b)] = pe
            kqseq += 1
            KQSEQ[("k", r, b)] = kqseq
            p1_order.append(("k", r, b))
            jt = 4 * b + r
            if jt < NT:
                pe += 1
                VG[jt] = pe
                p1_order.append(("v", jt))
        for kt in range(ND):
            pe += 1
            QG[(kt, b)] = pe
            kqseq += 1
            KQSEQ[("q", kt, b)] = kqseq
            p1_order.append(("q", kt, b))
    assert pe == NT + 2 * KQ

    fused_order = []
    for m in range(NT):
        fused_order.append(("L", m))
        for j in range(NT):
            if j + LA == m:
                fused_order.append(("R", j))
    for j in range(NT):
        if j + LA >= NT:
            fused_order.append(("R", j))

    for kind, jt in fused_order:
        if kind == "L":
            for ic in range(_c0(jt), NCH):
                pe += 1
                LG[(jt, ic)] = pe
        else:
            pe += 1
            RG[jt] = pe

    # paired outcopies: pair pm covers read rows (2pm, 2pm+1);
    # even pm -> ACT, odd pm -> DVE; due at fused position 2pm+1+LA
    aocp_due, docp_due = {}, {}
    AOCP_TRAIL, DOCP_TRAIL = [], []
    for pm in range(NP):
        due = 2 * pm + 1 + LA
        tgt = aocp_due if pm % 2 == 0 else docp_due
        trail = AOCP_TRAIL if pm % 2 == 0 else DOCP_TRAIL
        if due < NT:
            tgt.setdefault(due, []).append(pm)
        else:
            trail.append(pm)

    # ---- ACT plan (sAC): 32 kq copies, then per jt: due paired outcopies,
    #      exps (diag first, then non-diag groups), odd-jt V'-scale ----
    EXPG = {}      # (jt, gi) -> act idx; gi 0 = diag, 1.. = groups
    EXPREL = {}    # (jt, ic) -> act idx of the exp covering the chunk
    EXP_END = {}
    NOPS = {}
    AOCP, VP8A = {}, {}
    ac = 2 * KQ
    for jt in range(NT):
        c0 = _c0(jt)
        for pm in aocp_due.get(jt, []):
            ac += 1
            AOCP[pm] = ac
        ac += 1
        EXPG[(jt, 0)] = ac  # diag
        EXPREL[(jt, c0)] = ac
        gi = 1
        for run in row_groups[jt]:
            ac += 1
            EXPG[(jt, gi)] = ac
            for ic in run:
                EXPREL[(jt, ic)] = ac
            gi += 1
        EXP_END[jt] = ac
        NOPS[jt] = gi
        if jt % 2 == 1:
            ac += 1
            VP8A[jt] = ac
    for pm in AOCP_TRAIL:
        ac += 1
        AOCP[pm] = ac

    # ---- DVE plan (sDV): 16 v-copies, then per jt: DRED (diag max from
    #      bank; folded into NMX when nch==1), RED groups, NMX, [DENOM],
    #      RECIP, even-jt V'-scale, due paired outcopies ----
    VCP, DRED, REDG, NMX, DENOM, RECIP, VP8D, DOCP = {}, {}, {}, {}, {}, {}, {}, {}
    dv = 0
    for jt in range(NT):
        dv += 1
        VCP[jt] = dv
    for jt in range(NT):
        nch = NCH - _c0(jt)
        if nch > 1:
            dv += 1
            DRED[jt] = dv
            for gi in range(len(row_groups[jt])):
                dv += 1
                REDG[(jt, gi)] = dv
        dv += 1
        NMX[jt] = dv
        if nch > 1:
            dv += 1
            DENOM[jt] = dv
        dv += 1
        RECIP[jt] = dv
        if jt % 2 == 0:
            dv += 1
            VP8D[jt] = dv
        for pm in docp_due.get(jt, []):
            dv += 1
            DOCP[pm] = dv
    for pm in DOCP_TRAIL:
        dv += 1
        DOCP[pm] = dv

    # ---- Pool plan (sPO): slot-1 memsets only ----
    NMEMSET = 2 * NP

    def st_thr(pm):
        return 16 * (pm + 1)

    def bank_ap(coord, w=512):
        t, slot = coord
        if t == "E":
            return pse[:, slot, 0:w]
        return psRD[:, slot, 0:w]

    def bank_run_ap(coord, ln):
        t, slot = coord
        if t == "E":
            return pse[:, slot : slot + ln, :]
        return psRD[:, slot : slot + ln, :]

    def vp8_wait(te_or_none, jt):
        # (sem, thr) releasing vp8(jt)
        if jt % 2 == 0:
            return ("DV", VP8D[jt])
        return ("AC", VP8A[jt])

    with (
        nc.semaphore("sLv") as sLv,
        nc.semaphore("sLk") as sLk,
        nc.semaphore("sLk2") as sLk2,
        nc.semaphore("sLq") as sLq,
        nc.semaphore("sLc") as sLc,
        nc.semaphore("sLm") as sLm,
        nc.semaphore("sLx0") as sLx0,
        nc.semaphore("sLx1") as sLx1,
        nc.semaphore("sLx2") as sLx2,
        nc.semaphore("sLx3") as sLx3,
        nc.semaphore("sL80") as sL80,
        nc.semaphore("sL81") as sL81,
        nc.semaphore("sL82") as sL82,
        nc.semaphore("sL83") as sL83,
        nc.semaphore("sPE") as sPE,
        nc.semaphore("sDV") as sDV,
        nc.semaphore("sAC") as sAC,
        nc.semaphore("sPO") as sPO,
        nc.semaphore("sST") as sST,
        nc.Block() as block,
    ):
        sLxs = [sLx0, sLx1, sLx2, sLx3]
        sL8s = [sL80, sL81, sL82, sL83]

        @block.sync
        def _(sp):
            def ldx16(ic):
                sp.dma_start(
                    out=xTr[:, :, 512 * ic : 512 * (ic + 1)],
                    in_=x16_d.ap()[:, 512 * ic : 512 * (ic + 1)].rearrange(
                        "(t p) i -> p t i", p=128
                    ),
                ).then_inc(sLxs[ic], 16)

            def ldx8(ic):
                sp.dma_start(
                    out=x8s[:, :, :, 512 * ic : 512 * (ic + 1)],
                    in_=x8_d.ap()[:, :, :, 512 * ic : 512 * (ic + 1)],
                ).then_inc(sL8s[ic], 16)

            wk_re = wk_d.ap().rearrange("(t p) k -> p t k", p=128)
            sp.dma_start(out=wkr[:, :, 0:256], in_=wk_re[:, :, 0:256]).then_inc(
                sLk, 16
            )
            ldx16(0)
            sp.dma_start(out=wv8[:, :, :, :], in_=wv_d.ap()).then_inc(sLv, 16)
            ldx8(0)
            bv_ap = bv_d.ap()
            bv_bcast = bass.AP(
                tensor=bv_ap.tensor, offset=bv_ap.offset, ap=[[0, 128]] + list(bv_ap.ap)
            )
            sp.dma_start(out=bvb[:, :], in_=bv_bcast).then_inc(sLc, 16)
            with nc.allow_non_contiguous_dma(reason="16B/partition bias loads"):
                sp.dma_start(
                    out=bkc[:, :], in_=bk_d.ap().rearrange("(t p) -> p t", p=128)
                ).then_inc(sLc, 16)
                sp.dma_start(
                    out=bqc[:, :], in_=bq_d.ap().rearrange("(t p) -> p t", p=128)
                ).then_inc(sLc, 16)
            sp.dma_start(out=wkr[:, :, 256:512], in_=wk_re[:, :, 256:512]).then_inc(
                sLk2, 16
            )
            sp.dma_start(
                out=wqr[:, :, :],
                in_=wq_d.ap().rearrange("(t p) k -> p t k", p=128),
            ).then_inc(sLq, 16)
            sp.dma_start(out=mask16[:, :], in_=msk_d.ap()).then_inc(sLm, 16)
            sp.dma_start(out=ident16[:, :], in_=idn_d.ap()).then_inc(sLm, 16)
            ldx16(1)
            ldx8(1)
            ldx16(2)
            ldx8(2)
            ldx16(3)
            ldx8(3)
            # paired stores: 256 output rows each
            out_ap = out_d.ap()
            for pm in range(NP):
                if pm % 2 == 0:
                    sp.wait_ge(sAC, AOCP[pm])
                else:
                    sp.wait_ge(sDV, DOCP[pm])
                sp.dma_start(
                    out=out_ap[256 * pm : 256 * (pm + 1), :].rearrange(
                        "(s p) c -> p s c", p=128
                    ),
                    in_=outst[:, pm % 2, :, :],
                ).then_inc(sST, 16)
            sp.wait_ge(sST, 16 * NP)

        @block.tensor
        def _(te):
            waited = set()

            def ldwait(sem, thr=16):
                if sem not in waited:
                    te.wait_ge(sem, thr)
                    waited.add(sem)

            for item in p1_order:
                if item[0] == "v":
                    jt = item[1]
                    ldwait(sLv)
                    ldwait(sL8s[jt // 4])
                    if jt >= 2:
                        te.wait_ge(sDV, VCP[jt - 2])
                    for dm in range(2):
                        mm = te.matmul(
                            pse[:, jt % 2, :],
                            lhsT=x8s[:, dm, :, 128 * jt : 128 * (jt + 1)],
                            rhs=wv8[:, dm, :, :],
                            start=(dm == 0),
                            stop=(dm == 1),
                            perf_mode=DR,
                        )
                        if dm == 1:
                            mm.then_inc(sPE, 1)
                else:
                    kind, kt, ic = item
                    wsb = wkr if kind == "k" else wqr
                    if kind == "k":
                        ldwait(sLk if kt < 2 else sLk2)
                    else:
                        ldwait(sLq)
                    ldwait(sLxs[ic])
                    seq = KQSEQ[(kind, kt, ic)]
                    if seq > 4:
                        te.wait_ge(sAC, seq - 4)
                    for dt_ in range(ND):
                        mm = te.matmul(
                            pse[:, 2 + ((seq - 1) % 4), :],
                            lhsT=wsb[:, dt_, 128 * kt : 128 * (kt + 1)],
                            rhs=xTr[:, dt_, 512 * ic : 512 * (ic + 1)],
                            start=(dt_ == 0),
                            stop=(dt_ == ND - 1),
                        )
                        if dt_ == ND - 1:
                            mm.then_inc(sPE, 1)
            # fused: logits chunks (mask matmul appended to diag groups) +
            # DoubleRow read groups
            for kind, jt in fused_order:
                c0 = _c0(jt)
                if kind == "L":
                    for ic in range(c0, NCH):
                        gg = g_of_chunk[(jt, ic)]
                        diag = ic == c0
                        need_ac = KQSEQ[("q", ND - 1, ic)]
                        need_dv = None
                        pg = prev_user_g(gg)
                        if pg is not None:
                            pj, pic = chunk_of_g[pg]
                            need_ac = max(need_ac, EXPREL[(pj, pic)])
                        elif gg < 2:
                            need_dv = VCP[NT - 2 + gg]
                        elif gg < 6:
                            need_ac = max(need_ac, 2 * KQ)
                        te.wait_ge(sAC, need_ac)
                        if need_dv is not None:
                            te.wait_ge(sDV, need_dv)
                        if diag:
                            ldwait(sLm, 32)
                        w = 512 * (ic + 1) - (128 * jt if diag else 512 * ic)
                        bank = bank_ap(gbank[(jt, ic)], w)
                        ilo = 128 * jt if diag else 512 * ic
                        for kt in range(ND):
                            mm = te.matmul(
                                bank,
                                lhsT=kT[:, kt, 128 * jt : 128 * (jt + 1)],
                                rhs=qT[:, kt, ilo : 512 * (ic + 1)],
                                start=(kt == 0),
                                stop=(kt == ND - 1) and not diag,
                            )
                            if kt == ND - 1 and not diag:
                                mm.then_inc(sPE, 1)
                        if diag:
                            # causal mask accumulated by the PE: identity
                            # stationary x (0/-60000) f16 pattern
                            te.matmul(
                                bank_ap(gbank[(jt, ic)], 128),
                                lhsT=ident16[:, :],
                                rhs=mask16[:, :],
                                start=False,
                                stop=True,
                                skip_group_check=True,
                            ).then_inc(sPE, 1)
                else:
                    npair = (jt + 2) // 2
                    if jt >= 1:
                        sem, thr = vp8_wait(None, jt - 1)
                        te.wait_ge(sPO if sem == "PO" else (sDV if sem == "DV" else sAC), thr)
                    if jt < 2:
                        te.wait_ge(sAC, EXPREL[(1, 2)])
                    else:
                        pm = (jt - 2) // 2
                        if pm % 2 == 0:
                            te.wait_ge(sAC, AOCP[pm])
                        else:
                            te.wait_ge(sDV, DOCP[pm])
                    if jt == 0:
                        te.wait_ge(sPO, NMEMSET)
                    for m in range(npair):
                        if m == npair - 1:
                            sem, thr = vp8_wait(None, jt)
                            te.wait_ge(sDV if sem == "DV" else sAC, thr)
                        mm = te.matmul(
                            psRD[:, jt % 2, :],
                            lhsT=e8s[m][
                                :, :, 128 * jt - 256 * m : 128 * jt - 256 * m + 128
                            ],
                            rhs=vp8[:, m, :, :],
                            start=(m == 0),
                            stop=(m == npair - 1),
                            perf_mode=DR,
                        )
                        if m == npair - 1:
                            mm.then_inc(sPE, 1)

        @block.vector
        def _(ve):
            ndv = 0

            def inc(x):
                nonlocal ndv
                ndv += 1
                x.then_inc(sDV, 1)

            ve.wait_ge(sLc, 16)
            for jt in range(NT):
                ve.wait_ge(sPE, VG[jt])
                inc(
                    ve.tensor_tensor(
                        out=v_sb[:, jt, :], in0=pse[:, jt % 2, :],
                        in1=bvb[:, :], op=AOP.add,
                    )
                )
                assert ndv == VCP[jt]

            for jt in range(NT):
                c0 = _c0(jt)
                nch = NCH - c0
                w0 = 512 * (c0 + 1) - 128 * jt
                pj = jt % 2
                if jt >= 2:
                    # macc/negmax[jt%2] reuse: exps of jt-2 read them
                    ve.wait_ge(sAC, EXP_END[jt - 2])
                ve.wait_ge(sPE, LG[(jt, c0)])
                if nch > 1:
                    inc(
                        ve.reduce_max(
                            macc[:, pj, c0 : c0 + 1],
                            bank_ap(gbank[(jt, c0)], w0),
                            mybir.AxisListType.X,
                        )
                    )
                    assert ndv == DRED[jt]
                    for gi, run in enumerate(row_groups[jt]):
                        ve.wait_ge(sPE, LG[(jt, run[-1])])
                        inc(
                            ve.reduce_max(
                                macc[:, pj, run[0] : run[0] + len(run)],
                                bank_run_ap(gbank[(jt, run[0])], len(run)),
                                mybir.AxisListType.X,
                            )
                        )
                        assert ndv == REDG[(jt, gi)]
                    ve.wait_ge(sDV, REDG[(jt, len(row_groups[jt]) - 1)])
                    inc(
                        ve.reduce_max(
                            negmax[:, pj : pj + 1],
                            macc[:, pj, c0:NCH],
                            mybir.AxisListType.X, negate=True,
                        )
                    )
                else:
                    inc(
                        ve.reduce_max(
                            negmax[:, pj : pj + 1],
                            bank_ap(gbank[(jt, c0)], w0),
                            mybir.AxisListType.X, negate=True,
                        )
                    )
                assert ndv == NMX[jt]
                ve.wait_ge(sAC, EXP_END[jt])
                if nch > 1:
                    inc(
                        ve.reduce_sum(
                            denom[:, pj : pj + 1],
                            dparts[:, pj, 0 : NOPS[jt]],
                            mybir.AxisListType.X,
                        )
                    )
                    assert ndv == DENOM[jt]
                    ve.wait_ge(sDV, DENOM[jt])  # same-engine RAW fence
                    src = denom[:, pj : pj + 1]
                else:
                    src = dparts[:, pj, 0:1]
                inc(ve.reciprocal(rec[:, pj : pj + 1], src))
                assert ndv == RECIP[jt]
                if jt % 2 == 0:
                    ve.wait_ge(sDV, RECIP[jt])  # same-engine RAW fence
                    inc(
                        ve.tensor_scalar(
                            out=vp8[:, jt // 2, jt % 2, :],
                            in0=v_sb[:, jt, :],
                            scalar1=rec[:, pj : pj + 1],
                            scalar2=None,
                            op0=AOP.mult,
                        )
                    )
                    assert ndv == VP8D[jt]
                for pm in docp_due.get(jt, []):
                    ve.wait_ge(sPE, RG[2 * pm + 1])
                    if pm >= 2:
                        ve.wait_ge(sST, st_thr(pm - 2))
                    inc(
                        ve.tensor_scalar_add(
                            out=outst[:, pm % 2, :, :], in0=psRD[:, :, :],
                            scalar1=0.0,
                        )
                    )
                    assert ndv == DOCP[pm]
            for pm in DOCP_TRAIL:
                ve.wait_ge(sPE, RG[2 * pm + 1])
                if pm >= 2:
                    ve.wait_ge(sST, st_thr(pm - 2))
                inc(
                    ve.tensor_scalar_add(
                        out=outst[:, pm % 2, :, :], in0=psRD[:, :, :],
                        scalar1=0.0,
                    )
                )
                assert ndv == DOCP[pm]

        @block.scalar
        def _(ac_):
            ac_.wait_ge(sLc, 48)
            nac = 0
            for ic in range(NCH):
                for wsel, g_tab, bias in ((0, KG, bkc), (1, QG, bqc)):
                    dst = kT if wsel == 0 else qT
                    for kt in range(ND):
                        seq = KQSEQ[("k" if wsel == 0 else "q", kt, ic)]
                        ac_.wait_ge(sPE, g_tab[(kt, ic)])
                        ac_.activation(
                            out=dst[:, kt, 512 * ic : 512 * (ic + 1)],
                            in_=pse[:, 2 + ((seq - 1) % 4), :],
                            func=AFT.Identity,
                            bias=bias[:, kt : kt + 1],
                            scale=1.0,
                        ).then_inc(sAC, 1)
                        nac += 1
            assert nac == 2 * KQ

            def outcopy(pm):
                nonlocal nac
                ac_.wait_ge(sPE, RG[2 * pm + 1])
                if pm >= 2:
                    ac_.wait_ge(sST, st_thr(pm - 2))
                nac += 1
                ac_.activation(
                    out=outst[:, pm % 2, :, :], in_=psRD[:, :, :], func=AFT.Copy
                ).then_inc(sAC, 1)
                assert nac == AOCP[pm]

            for jt in range(NT):
                c0 = _c0(jt)
                pj = jt % 2
                m = jt // 2
                base = 256 * m
                for pm in aocp_due.get(jt, []):
                    outcopy(pm)
                # exps: diag first (frees the rotation bank soonest)
                ac_.wait_ge(sDV, NMX[jt])
                if jt >= 2:
                    # dparts[jt%2] reuse: recip(jt-2) must have read it
                    ac_.wait_ge(sDV, RECIP[jt - 2])
                w0 = 512 * (c0 + 1) - 128 * jt
                nac += 1
                ac_.activation(
                    out=e8s[m][:, jt % 2, 128 * jt - base : 512 * (c0 + 1) - base],
                    in_=bank_ap(gbank[(jt, c0)], w0),
                    func=AFT.Exp,
                    bias=negmax[:, pj : pj + 1],
                    scale=1.0,
                    accum_out=dparts[:, pj, 0:1],
                ).then_inc(sAC, 1)
                assert nac == EXPG[(jt, 0)]
                sidx = 1
                for run in row_groups[jt]:
                    a = 512 * run[0]
                    bcol = 512 * (run[-1] + 1)
                    nac += 1
                    ac_.activation(
                        out=e8s[m][:, jt % 2, a - base : bcol - base],
                        in_=bank_run_ap(gbank[(jt, run[0])], len(run)),
                        func=AFT.Exp,
                        bias=negmax[:, pj : pj + 1],
                        scale=1.0,
                        accum_out=dparts[:, pj, sidx : sidx + 1],
                    ).then_inc(sAC, 1)
                    assert nac == EXPG[(jt, sidx)]
                    sidx += 1
                assert sidx == NOPS[jt]
                if jt % 2 == 1:
                    ac_.wait_ge(sDV, RECIP[jt])
                    nac += 1
                    ac_.activation(
                        out=vp8[:, jt // 2, jt % 2, :],
                        in_=v_sb[:, jt, :],
                        func=AFT.Copy,
                        bias=0.0,
                        scale=rec[:, pj : pj + 1],
                    ).then_inc(sAC, 1)
                    assert nac == VP8A[jt]
            for pm in AOCP_TRAIL:
                outcopy(pm)

        @block.gpsimd
        def _(po):
            npo = 0
            for m in range(NP):
                po.memset(e8s[m][:, 1, 0:128], 0.0).then_inc(sPO, 1)
                npo += 1
                po.memset(vp8[:, m, 1, :], 0.0).then_inc(sPO, 1)
                npo += 1
            assert npo == NMEMSET

    nc.finalize()
    return nc


def _host_inputs(xb, wq16, wk16, wv8h, bq, bk, bv, T):
    # mask class 0 for the (narrowed) diagonal chunk: cols x < p get MASKVAL
    p = np.arange(128, dtype=np.float32)[:, None]
    xx = np.arange(128, dtype=np.float32)[None, :]
    msk = np.where(xx >= p, 0.0, MASKVAL).astype(np.float16)
    idn = np.eye(128, dtype=np.float16)

    xT = np.ascontiguousarray(xb.T)  # [D, T] f32
    x16 = xT.astype(np.float16)
    x8 = np.ascontiguousarray(
        xT.reshape(2, 2, 128, T).transpose(2, 0, 1, 3)
    ).astype(ml_dtypes.float8_e4m3fn)
    return dict(
        x16=x16,
        x8=np.ascontiguousarray(x8),
        wq16=wq16,
        wk16=wk16,
        wv8=wv8h,
        bq=bq,
        bk=bk,
        bv=bv,
        msk16=np.ascontiguousarray(msk),
        idn16=np.ascontiguousarray(idn),
    )


def kernel(x, Wk, bk, Wq, bq, Wv, bv):
    global LAST_RESULTS
    T = 2048
    x = np.ascontiguousarray(np.asarray(x, dtype=np.float32))
    Wk = np.asarray(Wk, dtype=np.float32)
    Wq = np.asarray(Wq, dtype=np.float32)
    Wv = np.asarray(Wv, dtype=np.float32)
    bk = np.ascontiguousarray(np.asarray(bk, dtype=np.float32))
    bq = np.ascontiguousarray(np.asarray(bq, dtype=np.float32))
    bv = np.ascontiguousarray(np.asarray(bv, dtype=np.float32))

    wq16 = np.ascontiguousarray(Wq.T).astype(np.float16)
    wk16 = np.ascontiguousarray(Wk.T).astype(np.float16)
    wvT = Wv.T * np.float32(INV_SQRT_K)  # [D, KS]
    wv8h = np.ascontiguousarray(
        wvT.reshape(2, 2, 128, KS).transpose(2, 0, 1, 3)
    ).astype(ml_dtypes.float8_e4m3fn)
    bvs = np.ascontiguousarray(bv * np.float32(INV_SQRT_K))

    nc = build_nc(T)
    in_maps = [_host_inputs(x[b], wq16, wk16, wv8h, bq, bk, bvs, T) for b in range(B)]
    res = None
    last_exc = None
    for attempt in range(3):
        try:
            res = run_bass_kernel_spmd(nc, in_maps, list(range(B)), trace=TRACE)
            break
        except Exception as e:  # transient NRT device errors; retry fresh
            last_exc = e
            import time as _time
            _time.sleep(10)
            nc = build_nc(T)
    if res is None:
        raise last_exc
    LAST_RESULTS = res
    read = np.stack(
        [np.asarray(res.results[b]["out"]).astype(np.float32) for b in range(B)],
        axis=0,
    )
    return (x + read).astype(np.float32)


# revision 3
# speedup vs baseline: 1.5503x; 1.3004x over previous
"""Trainium2 Bass kernel for nn_AttentionBlock (causal attn, softmax over the
QUERY axis (dim=1), post-softmax 1/sqrt(K) scale, residual add).

Sharding: data-parallel over batch B=8, one batch element per NeuronCore.

v4 design:
- K/Q projections + logits in fp16; V projection and the probability-weighted
  read in fp8e4 with MatmulPerfMode.DoubleRow (two 128-deep contraction tiles
  per matmul).
- The causal mask is applied BY THE PE: each diagonal logits group gets one
  extra 128-wide matmul (identity stationary x f16 mask pattern of 0/-60000)
  accumulated into the PSUM bank.  exp() of -60000-ish underflows to exactly
  0.  No DVE mask-add, no staging: every chunk is max-reduced and exp'd
  straight from its PSUM bank.
- Logits PSUM banks live in ONE 6-bank tensor (pse) plus a 2-bank read tensor
  (psRD).  The first 8 logits chunks use all 8 banks (prologue), then a
  6-bank rotation.  Adjacent-bank non-diag chunk runs are processed by SINGLE
  wide ops: one DVE reduce yields 2-3 column maxes, one ACT exp covers 2-3
  chunks (one f32 accum partial per op).
- Denominator combine + reciprocal on DVE; V'-scales split DVE (even jt) /
  ACT (odd jt); output evacuation is PAIRED: one op copies both read banks
  [128,2,512] -> bf16 outst, one DMA stores 256 output rows; pairs alternate
  ACT/DVE.  Pool does only the startup memsets.
- E8[j,i] strips live in PAIRED key-chunk layout e8s[m][:, slot, :]
  (slot = jt%2, strip base column 256m) feeding DoubleRow reads directly;
  slot-1's first 128 columns are memset 0 (sub-diagonal).
    read[i, :] = sum_j E[j, i] * V'[j, :],  V' = (v + bv) * rec_j / sqrt(K)

Raw Block style with manual semaphores: ONE embedded sync-wait per
instruction; cross-engine deps are standalone wait_ge with statically
computed thresholds; same-engine RAW pairs get explicit fences.
"""

import math
import os
import sys

import numpy as np
import ml_dtypes

for _p in ("/opt/trn_rl_repo", "/root/.axon_site/_ro/trn_rl_repo"):
    if os.path.isdir(_p) and _p not in sys.path:
        sys.path.append(_p)

import concourse.bass as bass
from concourse import mybir
from concourse.bass_utils import run_bass_kernel_spmd

B = 8
D = 512
KS = 512
ND = D // 128  # 4 contraction tiles

F32 = mybir.dt.float32
F16 = mybir.dt.float16
BF16 = mybir.dt.bfloat16
F8 = mybir.dt.float8e4
AOP = mybir.AluOpType
AFT = mybir.ActivationFunctionType
DR = mybir.MatmulPerfMode.DoubleRow

INV_SQRT_K = 1.0 / math.sqrt(KS)
MASKVAL = -60000.0  # fits f16; exp(-60000 + max) == 0 exactly

TRACE = False
LAST_RESULTS = None


def _c0(jt):
    return (128 * jt) // 512


def build_nc(T=2048, debug_dump=False):
    NT = T // 128   # 16 row chunks
    NCH = T // 512  # 4 column chunks
    NP = NT // 2    # 8 key-chunk pairs
    KQ = ND * NCH   # 16 projection output groups for each of q/k

    nc = bass.Bass("TRN2", target_bir_lowering=False, debug=False, num_devices=B)

    # ---- DRAM ----
    x16_d = nc.dram_tensor("x16", [D, T], F16, kind="ExternalInput")
    x8_d = nc.dram_tensor("x8", [128, 2, 2, T], F8, kind="ExternalInput")
    wq_d = nc.dram_tensor("wq16", [D, KS], F16, kind="ExternalInput")
    wk_d = nc.dram_tensor("wk16", [D, KS], F16, kind="ExternalInput")
    wv_d = nc.dram_tensor("wv8", [128, 2, 2, KS], F8, kind="ExternalInput")
    bq_d = nc.dram_tensor("bq", [KS], F32, kind="ExternalInput")
    bk_d = nc.dram_tensor("bk", [KS], F32, kind="ExternalInput")
    bv_d = nc.dram_tensor("bv", [KS], F32, kind="ExternalInput")
    msk_d = nc.dram_tensor("msk16", [128, 128], F16, kind="ExternalInput")
    idn_d = nc.dram_tensor("idn16", [128, 128], F16, kind="ExternalInput")
    out_d = nc.dram_tensor("out", [T, KS], BF16, kind="ExternalOutput")

    # ---- SBUF ----
    xTr = nc.alloc_sbuf_tensor("xTr", [128, ND, T], F16)
    x8s = nc.alloc_sbuf_tensor("x8s", [128, 2, 2, T], F8)
    wkr = nc.alloc_sbuf_tensor("wkr", [128, ND, KS], F16)
    wqr = nc.alloc_sbuf_tensor("wqr", [128, ND, KS], F16)
    wv8 = nc.alloc_sbuf_tensor("wv8s", [128, 2, 2, KS], F8)
    kT = nc.alloc_sbuf_tensor("kT", [128, ND, T], F16)
    qT = nc.alloc_sbuf_tensor("qT", [128, ND, T], F16)
    v_sb = nc.alloc_sbuf_tensor("v_sb", [128, NT, KS], BF16)
    vp8 = nc.alloc_sbuf_tensor("vp8", [128, NP, 2, KS], F8)
    e8s = [
        nc.alloc_sbuf_tensor(f"e8_{m}", [128, 2, T - 256 * m], F8)
        for m in range(NP)
    ]
    outst = nc.alloc_sbuf_tensor("outst", [128, 2, 2, KS], BF16)
    mask16 = nc.alloc_sbuf_tensor("mask16", [128, 128], F16)
    ident16 = nc.alloc_sbuf_tensor("ident16", [128, 128], F16)
    bqc = nc.alloc_sbuf_tensor("bqc", [128, ND], F32)
    bkc = nc.alloc_sbuf_tensor("bkc", [128, ND], F32)
    bvb = nc.alloc_sbuf_tensor("bvb", [128, KS], F32)
    macc = nc.alloc_sbuf_tensor("macc", [128, 2, NCH], F32)
    negmax = nc.alloc_sbuf_tensor("negmax", [128, 2], F32)
    dparts = nc.alloc_sbuf_tensor("dparts", [128, 2, NCH], F32)
    denom = nc.alloc_sbuf_tensor("denom", [128, 2], F32)
    rec = nc.alloc_sbuf_tensor("rec", [128, 2], F32)

    # ---- PSUM: 6-bank logits rotation + 2 read banks ----
    pse = nc.alloc_psum_tensor("pse", [128, 6, 512], F32)
    psRD = nc.alloc_psum_tensor("psRD", [128, 2, 512], F32)
    # phase 1: v -> pse[0..1], k/q -> pse[2..5]

    # ================= static op-index tables =================
    LA = 4  # read lookahead: R(j) sits at fused position j + LA

    def bank_of_g(g):
        if g < 6:
            return ("E", g)
        if g < 8:
            return ("R", g - 6)
        return ("E", (g - 8) % 6)

    def prev_user_g(g):
        if g < 8:
            return None
        return g - 8 if g < 14 else g - 6

    gbank = {}
    g_of_chunk = {}
    chunk_of_g = {}
    g = 0
    for jt in range(NT):
        for ic in range(_c0(jt), NCH):
            gbank[(jt, ic)] = bank_of_g(g)
            g_of_chunk[(jt, ic)] = g
            chunk_of_g[g] = (jt, ic)
            g += 1

    # non-diag chunk groups per row: maximal runs of adjacent banks
    row_groups = {}
    for jt in range(NT):
        c0 = _c0(jt)
        ics = list(range(c0 + 1, NCH))
        groups = []
        i = 0
        while i < len(ics):
            run = [ics[i]]
            while (
                i + 1 < len(ics)
                and len(run) < 3
                and gbank[(jt, ics[i + 1])][0] == gbank[(jt, run[0])][0]
                and gbank[(jt, ics[i + 1])][1]
                == gbank[(jt, run[0])][1] + len(run)
            ):
                run.append(ics[i + 1])
                i += 1
            groups.append(run)
            i += 1
        row_groups[jt] = groups

    # ---- PE plan (sPE counts GROUPS) ----
    VG, KG, QG, KQSEQ, LG, RG = {}, {}, {}, {}, {}, {}
    pe = 0
    kqseq = 0
    p1_order = []
    for b in range(NCH):
        for r in range(4):
            pe += 1
            KG[(r, b)] = pe
            kqseq += 1
            KQSEQ[("k", r, b)] = kqseq
            p1_order.append(("k", r, b))
            jt = 4 * b + r
            if jt < NT:
                pe += 1
                VG[jt] = pe
                p1_order.append(("v", jt))
        for kt in range(ND):
            pe += 1
            QG[(kt, b)] = pe
            kqseq += 1
            KQSEQ[("q", kt, b)] = kqseq
            p1_order.append(("q", kt, b))
    assert pe == NT + 2 * KQ

    fused_order = []
    for m in range(NT):
        fused_order.append(("L", m))
        for j in range(NT):
            if j + LA == m:
                fused_order.append(("R", j))
    for j in range(NT):
        if j + LA >= NT:
            fused_order.append(("R", j))

    for kind, jt in fused_order:
        if kind == "L":
            for ic in range(_c0(jt), NCH):
                pe += 1
                LG[(jt, ic)] = pe
        else:
            pe += 1
            RG[jt] = pe

    # paired outcopies: pair pm covers read rows (2pm, 2pm+1);
    # even pm -> ACT, odd pm -> DVE; due at fused position 2pm+1+LA
    aocp_due, docp_due = {}, {}
    AOCP_TRAIL, DOCP_TRAIL = [], []
    for pm in range(NP):
        due = 2 * pm + 1 + LA
        tgt = aocp_due if pm % 2 == 0 else docp_due
        trail = AOCP_TRAIL if pm % 2 == 0 else DOCP_TRAIL
        if due < NT:
            tgt.setdefault(due, []).append(pm)
        else:
            trail.append(pm)

    # ---- ACT plan (sAC): 32 kq copies, then per jt: due paired outcopies,
    #      exps (diag first, then non-diag groups), odd-jt V'-scale ----
    EXPG = {}      # (jt, gi) -> act idx; gi 0 = diag, 1.. = groups
    EXPREL = {}    # (jt, ic) -> act idx of the exp covering the chunk
    EXP_END = {}
    NOPS = {}
    AOCP, VP8A = {}, {}
    ac = 2 * KQ
    for jt in range(NT):
        c0 = _c0(jt)
        for pm in aocp_due.get(jt, []):
            ac += 1
            AOCP[pm] = ac
        ac += 1
        EXPG[(jt, 0)] = ac  # diag
        EXPREL[(jt, c0)] = ac
        gi = 1
        for run in row_groups[jt]:
            ac += 1
            EXPG[(jt, gi)] = ac
            for ic in run:
                EXPREL[(jt, ic)] = ac
            gi += 1
        EXP_END[jt] = ac
        NOPS[jt] = gi
        if jt >= 1 and (jt - 1) % 2 == 1:
            ac += 1
            VP8A[jt - 1] = ac
    if (NT - 1) % 2 == 1:
        ac += 1
        VP8A[NT - 1] = ac
    for pm in AOCP_TRAIL:
        ac += 1
        AOCP[pm] = ac

    # ---- DVE plan (sDV): 16 v-copies, then per jt: DRED (diag max from
    #      bank; folded into NMX when nch==1), RED groups, NMX, [DENOM],
    #      RECIP, even-jt V'-scale, due paired outcopies ----
    VCP, DRED, REDG, NMX, DENOM, RECIP, VP8D, DOCP = {}, {}, {}, {}, {}, {}, {}, {}
    dv = 0
    for jt in range(NT):
        dv += 1
        VCP[jt] = dv

    def _dve_tail(jt):
        # denominator chain of row jt, emitted one block later
        nonlocal_dv = []
        return nonlocal_dv

    for jt in range(NT + 1):
        if jt < NT:
            nch = NCH - _c0(jt)
            if nch > 1:
                dv += 1
                DRED[jt] = dv
                for gi in range(len(row_groups[jt])):
                    dv += 1
                    REDG[(jt, gi)] = dv
            dv += 1
            NMX[jt] = dv
        pj = jt - 1  # previous row's denominator chain
        if 0 <= pj < NT:
            if NCH - _c0(pj) > 1:
                dv += 1
                DENOM[pj] = dv
            dv += 1
            RECIP[pj] = dv
            if pj % 2 == 0:
                dv += 1
                VP8D[pj] = dv
        if jt < NT:
            for pm in docp_due.get(jt, []):
                dv += 1
                DOCP[pm] = dv
    for pm in DOCP_TRAIL:
        dv += 1
        DOCP[pm] = dv

    # ---- Pool plan (sPO): slot-1 memsets only ----
    NMEMSET = 2 * NP

    def st_thr(pm):
        return 16 * (pm + 1)

    def bank_ap(coord, w=512):
        t, slot = coord
        if t == "E":
            return pse[:, slot, 0:w]
        return psRD[:, slot, 0:w]

    def bank_run_ap(coord, ln):
        t, slot = coord
        if t == "E":
            return pse[:, slot : slot + ln, :]
        return psRD[:, slot : slot + ln, :]

    def vp8_wait(te_or_none, jt):
        # (sem, thr) releasing vp8(jt)
        if jt % 2 == 0:
            return ("DV", VP8D[jt])
        return ("AC", VP8A[jt])

    with (
        nc.semaphore("sLv") as sLv,
        nc.semaphore("sLk") as sLk,
        nc.semaphore("sLk2") as sLk2,
        nc.semaphore("sLq") as sLq,
        nc.semaphore("sLc") as sLc,
        nc.semaphore("sLm") as sLm,
        nc.semaphore("sLx0") as sLx0,
        nc.semaphore("sLx1") as sLx1,
        nc.semaphore("sLx2") as sLx2,
        nc.semaphore("sLx3") as sLx3,
        nc.semaphore("sL80") as sL80,
        nc.semaphore("sL81") as sL81,
        nc.semaphore("sL82") as sL82,
        nc.semaphore("sL83") as sL83,
        nc.semaphore("sPE") as sPE,
        nc.semaphore("sDV") as sDV,
        nc.semaphore("sAC") as sAC,
        nc.semaphore("sPO") as sPO,
        nc.semaphore("sST") as sST,
        nc.Block() as block,
    ):
        sLxs = [sLx0, sLx1, sLx2, sLx3]
        sL8s = [sL80, sL81, sL82, sL83]

        @block.sync
        def _(sp):
            def ldx16(ic):
                sp.dma_start(
                    out=xTr[:, :, 512 * ic : 512 * (ic + 1)],
                    in_=x16_d.ap()[:, 512 * ic : 512 * (ic + 1)].rearrange(
                        "(t p) i -> p t i", p=128
                    ),
                ).then_inc(sLxs[ic], 16)

            def ldx8(ic):
                sp.dma_start(
                    out=x8s[:, :, :, 512 * ic : 512 * (ic + 1)],
                    in_=x8_d.ap()[:, :, :, 512 * ic : 512 * (ic + 1)],
                ).then_inc(sL8s[ic], 16)

            wk_re = wk_d.ap().rearrange("(t p) k -> p t k", p=128)
            sp.dma_start(out=wkr[:, :, 0:256], in_=wk_re[:, :, 0:256]).then_inc(
                sLk, 16
            )
            ldx16(0)
            sp.dma_start(out=wv8[:, :, :, :], in_=wv_d.ap()).then_inc(sLv, 16)
            ldx8(0)
            bv_ap = bv_d.ap()
            bv_bcast = bass.AP(
                tensor=bv_ap.tensor, offset=bv_ap.offset, ap=[[0, 128]] + list(bv_ap.ap)
            )
            sp.dma_start(out=bvb[:, :], in_=bv_bcast).then_inc(sLc, 16)
            with nc.allow_non_contiguous_dma(reason="16B/partition bias loads"):
                sp.dma_start(
                    out=bkc[:, :], in_=bk_d.ap().rearrange("(t p) -> p t", p=128)
                ).then_inc(sLc, 16)
                sp.dma_start(
                    out=bqc[:, :], in_=bq_d.ap().rearrange("(t p) -> p t", p=128)
                ).then_inc(sLc, 16)
            sp.dma_start(out=wkr[:, :, 256:512], in_=wk_re[:, :, 256:512]).then_inc(
                sLk2, 16
            )
            sp.dma_start(
                out=wqr[:, :, :],
                in_=wq_d.ap().rearrange("(t p) k -> p t k", p=128),
            ).then_inc(sLq, 16)
            sp.dma_start(out=mask16[:, :], in_=msk_d.ap()).then_inc(sLm, 16)
            sp.dma_start(out=ident16[:, :], in_=idn_d.ap()).then_inc(sLm, 16)
            ldx16(1)
            ldx8(1)
            ldx16(2)
            ldx8(2)
            ldx16(3)
            ldx8(3)
            # paired stores: 256 output rows each
            out_ap = out_d.ap()
            for pm in range(NP):
                if pm % 2 == 0:
                    sp.wait_ge(sAC, AOCP[pm])
                else:
                    sp.wait_ge(sDV, DOCP[pm])
                sp.dma_start(
                    out=out_ap[256 * pm : 256 * (pm + 1), :].rearrange(
                        "(s p) c -> p s c", p=128
                    ),
                    in_=outst[:, pm % 2, :, :],
                ).then_inc(sST, 16)
            sp.wait_ge(sST, 16 * NP)

        @block.tensor
        def _(te):
            waited = set()

            def ldwait(sem, thr=16):
                if sem not in waited:
                    te.wait_ge(sem, thr)
                    waited.add(sem)

            for item in p1_order:
                if item[0] == "v":
                    jt = item[1]
                    ldwait(sLv)
                    ldwait(sL8s[jt // 4])
                    if jt >= 2:
                        te.wait_ge(sDV, VCP[jt - 2])
                    for dm in range(2):
                        mm = te.matmul(
                            pse[:, jt % 2, :],
                            lhsT=x8s[:, dm, :, 128 * jt : 128 * (jt + 1)],
                            rhs=wv8[:, dm, :, :],
                            start=(dm == 0),
                            stop=(dm == 1),
                            perf_mode=DR,
                        )
                        if dm == 1:
                            mm.then_inc(sPE, 1)
                else:
                    kind, kt, ic = item
                    wsb = wkr if kind == "k" else wqr
                    if kind == "k":
                        ldwait(sLk if kt < 2 else sLk2)
                    else:
                        ldwait(sLq)
                    ldwait(sLxs[ic])
                    seq = KQSEQ[(kind, kt, ic)]
                    if seq > 4:
                        te.wait_ge(sAC, seq - 4)
                    for dt_ in range(ND):
                        mm = te.matmul(
                            pse[:, 2 + ((seq - 1) % 4), :],
                            lhsT=wsb[:, dt_, 128 * kt : 128 * (kt + 1)],
                            rhs=xTr[:, dt_, 512 * ic : 512 * (ic + 1)],
                            start=(dt_ == 0),
                            stop=(dt_ == ND - 1),
                        )
                        if dt_ == ND - 1:
                            mm.then_inc(sPE, 1)
            # fused: logits chunks (mask matmul appended to diag groups) +
            # DoubleRow read groups
            for kind, jt in fused_order:
                c0 = _c0(jt)
                if kind == "L":
                    for ic in range(c0, NCH):
                        gg = g_of_chunk[(jt, ic)]
                        diag = ic == c0
                        need_ac = KQSEQ[("q", ND - 1, ic)]
                        need_dv = None
                        pg = prev_user_g(gg)
                        if pg is not None:
                            pj, pic = chunk_of_g[pg]
                            need_ac = max(need_ac, EXPREL[(pj, pic)])
                        elif gg < 2:
                            need_dv = VCP[NT - 2 + gg]
                        elif gg < 6:
                            need_ac = max(need_ac, 2 * KQ)
                        te.wait_ge(sAC, need_ac)
                        if need_dv is not None:
                            te.wait_ge(sDV, need_dv)
                        if diag:
                            ldwait(sLm, 32)
                        w = 512 * (ic + 1) - (128 * jt if diag else 512 * ic)
                        bank = bank_ap(gbank[(jt, ic)], w)
                        ilo = 128 * jt if diag else 512 * ic
                        for kt in range(ND):
                            mm = te.matmul(
                                bank,
                                lhsT=kT[:, kt, 128 * jt : 128 * (jt + 1)],
                                rhs=qT[:, kt, ilo : 512 * (ic + 1)],
                                start=(kt == 0),
                                stop=(kt == ND - 1) and not diag,
                            )
                            if kt == ND - 1 and not diag:
                                mm.then_inc(sPE, 1)
                        if diag:
                            # causal mask accumulated by the PE: identity
                            # stationary x (0/-60000) f16 pattern
                            te.matmul(
                                bank_ap(gbank[(jt, ic)], 128),
                                lhsT=ident16[:, :],
                                rhs=mask16[:, :],
                                start=False,
                                stop=True,
                                skip_group_check=True,
                            ).then_inc(sPE, 1)
                else:
                    npair = (jt + 2) // 2
                    if jt >= 1:
                        sem, thr = vp8_wait(None, jt - 1)
                        te.wait_ge(sPO if sem == "PO" else (sDV if sem == "DV" else sAC), thr)
                    if jt < 2:
                        te.wait_ge(sAC, EXPREL[(1, 2)])
                    else:
                        pm = (jt - 2) // 2
                        if pm % 2 == 0:
                            te.wait_ge(sAC, AOCP[pm])
                        else:
                            te.wait_ge(sDV, DOCP[pm])
                    if jt == 0:
                        te.wait_ge(sPO, NMEMSET)
                    for m in range(npair):
                        if m == npair - 1:
                            sem, thr = vp8_wait(None, jt)
                            te.wait_ge(sDV if sem == "DV" else sAC, thr)
                        mm = te.matmul(
                            psRD[:, jt % 2, :],
                            lhsT=e8s[m][
                                :, :, 128 * jt - 256 * m : 128 * jt - 256 * m + 128
                            ],
                            rhs=vp8[:, m, :, :],
                            start=(m == 0),
                            stop=(m == npair - 1),
                            perf_mode=DR,
                        )
                        if m == npair - 1:
                            mm.then_inc(sPE, 1)

        @block.vector
        def _(ve):
            ndv = 0

            def inc(x):
                nonlocal ndv
                ndv += 1
                x.then_inc(sDV, 1)

            ve.wait_ge(sLc, 16)
            for jt in range(NT):
                ve.wait_ge(sPE, VG[jt])
                inc(
                    ve.tensor_tensor(
                        out=v_sb[:, jt, :], in0=pse[:, jt % 2, :],
                        in1=bvb[:, :], op=AOP.add,
                    )
                )
                assert ndv == VCP[jt]

            def den_chain(rj):
                # denominator chain of row rj (one block late)
                rp = rj % 2
                ve.wait_ge(sAC, EXP_END[rj])
                if NCH - _c0(rj) > 1:
                    inc(
                        ve.reduce_sum(
                            denom[:, rp : rp + 1],
                            dparts[:, rp, 0 : NOPS[rj]],
                            mybir.AxisListType.X,
                        )
                    )
                    assert ndv == DENOM[rj]
                    ve.wait_ge(sDV, DENOM[rj])  # same-engine RAW fence
                    src = denom[:, rp : rp + 1]
                else:
                    src = dparts[:, rp, 0:1]
                inc(ve.reciprocal(rec[:, rp : rp + 1], src))
                assert ndv == RECIP[rj]
                if rj % 2 == 0:
                    ve.wait_ge(sDV, RECIP[rj])  # same-engine RAW fence
                    inc(
                        ve.tensor_scalar(
                            out=vp8[:, rj // 2, rj % 2, :],
                            in0=v_sb[:, rj, :],
                            scalar1=rec[:, rp : rp + 1],
                            scalar2=None,
                            op0=AOP.mult,
                        )
                    )
                    assert ndv == VP8D[rj]

            for jt in range(NT):
                c0 = _c0(jt)
                nch = NCH - c0
                w0 = 512 * (c0 + 1) - 128 * jt
                pj = jt % 2
                if jt >= 2:
                    # macc/negmax[jt%2] reuse: exps of jt-2 read them
                    ve.wait_ge(sAC, EXP_END[jt - 2])
                ve.wait_ge(sPE, LG[(jt, c0)])
                if nch > 1:
                    inc(
                        ve.reduce_max(
                            macc[:, pj, c0 : c0 + 1],
                            bank_ap(gbank[(jt, c0)], w0),
                            mybir.AxisListType.X,
                        )
                    )
                    assert ndv == DRED[jt]
                    for gi, run in enumerate(row_groups[jt]):
                        ve.wait_ge(sPE, LG[(jt, run[-1])])
                        inc(
                            ve.reduce_max(
                                macc[:, pj, run[0] : run[0] + len(run)],
                                bank_run_ap(gbank[(jt, run[0])], len(run)),
                                mybir.AxisListType.X,
                            )
                        )
                        assert ndv == REDG[(jt, gi)]
                    ve.wait_ge(sDV, REDG[(jt, len(row_groups[jt]) - 1)])
                    inc(
                        ve.reduce_max(
                            negmax[:, pj : pj + 1],
                            macc[:, pj, c0:NCH],
                            mybir.AxisListType.X, negate=True,
                        )
                    )
                else:
                    inc(
                        ve.reduce_max(
                            negmax[:, pj : pj + 1],
                            bank_ap(gbank[(jt, c0)], w0),
                            mybir.AxisListType.X, negate=True,
                        )
                    )
                assert ndv == NMX[jt]
                if jt >= 1:
                    den_chain(jt - 1)
                for pm in docp_due.get(jt, []):
                    ve.wait_ge(sPE, RG[2 * pm + 1])
                    if pm >= 2:
                        ve.wait_ge(sST, st_thr(pm - 2))
                    inc(
                        ve.tensor_scalar_add(
                            out=outst[:, pm % 2, :, :], in0=psRD[:, :, :],
                            scalar1=0.0,
                        )
                    )
                    assert ndv == DOCP[pm]
            den_chain(NT - 1)
            for pm in DOCP_TRAIL:
                ve.wait_ge(sPE, RG[2 * pm + 1])
                if pm >= 2:
                    ve.wait_ge(sST, st_thr(pm - 2))
                inc(
                    ve.tensor_scalar_add(
                        out=outst[:, pm % 2, :, :], in0=psRD[:, :, :],
                        scalar1=0.0,
                    )
                )
                assert ndv == DOCP[pm]

        @block.scalar
        def _(ac_):
            ac_.wait_ge(sLc, 48)
            nac = 0
            for ic in range(NCH):
                for wsel, g_tab, bias in ((0, KG, bkc), (1, QG, bqc)):
                    dst = kT if wsel == 0 else qT
                    for kt in range(ND):
                        seq = KQSEQ[("k" if wsel == 0 else "q", kt, ic)]
                        ac_.wait_ge(sPE, g_tab[(kt, ic)])
                        ac_.activation(
                            out=dst[:, kt, 512 * ic : 512 * (ic + 1)],
                            in_=pse[:, 2 + ((seq - 1) % 4), :],
                            func=AFT.Identity,
                            bias=bias[:, kt : kt + 1],
                            scale=1.0,
                        ).then_inc(sAC, 1)
                        nac += 1
            assert nac == 2 * KQ

            def outcopy(pm):
                nonlocal nac
                ac_.wait_ge(sPE, RG[2 * pm + 1])
                if pm >= 2:
                    ac_.wait_ge(sST, st_thr(pm - 2))
                nac += 1
                ac_.activation(
                    out=outst[:, pm % 2, :, :], in_=psRD[:, :, :], func=AFT.Copy
                ).then_inc(sAC, 1)
                assert nac == AOCP[pm]

            def act_vp8(rj):
                nonlocal nac
                rp = rj % 2
                ac_.wait_ge(sDV, RECIP[rj])
                nac += 1
                ac_.activation(
                    out=vp8[:, rj // 2, rj % 2, :],
                    in_=v_sb[:, rj, :],
                    func=AFT.Copy,
                    bias=0.0,
                    scale=rec[:, rp : rp + 1],
                ).then_inc(sAC, 1)
                assert nac == VP8A[rj]

            for jt in range(NT):
                c0 = _c0(jt)
                pj = jt % 2
                m = jt // 2
                base = 256 * m
                for pm in aocp_due.get(jt, []):
                    outcopy(pm)
                # exps: diag first (frees the rotation bank soonest)
                ac_.wait_ge(sDV, NMX[jt])
                if jt >= 2:
                    # dparts[jt%2] reuse: recip(jt-2) must have read it
                    ac_.wait_ge(sDV, RECIP[jt - 2])
                w0 = 512 * (c0 + 1) - 128 * jt
                nac += 1
                ac_.activation(
                    out=e8s[m][:, jt % 2, 128 * jt - base : 512 * (c0 + 1) - base],
                    in_=bank_ap(gbank[(jt, c0)], w0),
                    func=AFT.Exp,
                    bias=negmax[:, pj : pj + 1],
                    scale=1.0,
                    accum_out=dparts[:, pj, 0:1],
                ).then_inc(sAC, 1)
                assert nac == EXPG[(jt, 0)]
                sidx = 1
                for run in row_groups[jt]:
                    a = 512 * run[0]
                    bcol = 512 * (run[-1] + 1)
                    nac += 1
                    ac_.activation(
                        out=e8s[m][:, jt % 2, a - base : bcol - base],
                        in_=bank_run_ap(gbank[(jt, run[0])], len(run)),
                        func=AFT.Exp,
                        bias=negmax[:, pj : pj + 1],
                        scale=1.0,
                        accum_out=dparts[:, pj, sidx : sidx + 1],
                    ).then_inc(sAC, 1)
                    assert nac == EXPG[(jt, sidx)]
                    sidx += 1
                assert sidx == NOPS[jt]
                if jt >= 1 and (jt - 1) % 2 == 1:
                    act_vp8(jt - 1)
            if (NT - 1) % 2 == 1:
                act_vp8(NT - 1)
            for pm in AOCP_TRAIL:
                outcopy(pm)

        @block.gpsimd
        def _(po):
            npo = 0
            for m in range(NP):
                po.memset(e8s[m][:, 1, 0:128], 0.0).then_inc(sPO, 1)
                npo += 1
                po.memset(vp8[:, m, 1, :], 0.0).then_inc(sPO, 1)
                npo += 1
            assert npo == NMEMSET

    nc.finalize()
    return nc


def _host_inputs(xb, wq16, wk16, wv8h, bq, bk, bv, T):
    # mask class 0 for the (narrowed) diagonal chunk: cols x < p get MASKVAL
    p = np.arange(128, dtype=np.float32)[:, None]
    xx = np.arange(128, dtype=np.float32)[None, :]
    msk = np.where(xx >= p, 0.0, MASKVAL).astype(np.float16)
    idn = np.eye(128, dtype=np.float16)

    xT = np.ascontiguousarray(xb.T)  # [D, T] f32
    x16 = xT.astype(np.float16)
    x8 = np.ascontiguousarray(
        xT.reshape(2, 2, 128, T).transpose(2, 0, 1, 3)
    ).astype(ml_dtypes.float8_e4m3fn)
    return dict(
        x16=x16,
        x8=np.ascontiguousarray(x8),
        wq16=wq16,
        wk16=wk16,
        wv8=wv8h,
        bq=bq,
        bk=bk,
        bv=bv,
        msk16=np.ascontiguousarray(msk),
        idn16=np.ascontiguousarray(idn),
    )


def kernel(x, Wk, bk, Wq, bq, Wv, bv):
    global LAST_RESULTS
    T = 2048
    x = np.ascontiguousarray(np.asarray(x, dtype=np.float32))
    Wk = np.asarray(Wk, dtype=np.float32)
    Wq = np.asarray(Wq, dtype=np.float32)
    Wv = np.asarray(Wv, dtype=np.float32)
    bk = np.ascontiguousarray(np.asarray(bk, dtype=np.float32))
    bq = np.ascontiguousarray(np.asarray(bq, dtype=np.float32))
    bv = np.ascontiguousarray(np.asarray(bv, dtype=np.float32))

    wq16 = np.ascontiguousarray(Wq.T).astype(np.float16)
    wk16 = np.ascontiguousarray(Wk.T).astype(np.float16)
    wvT = Wv.T * np.float32(INV_SQRT_K)  # [D, KS]
    wv8h = np.ascontiguousarray(
        wvT.reshape(2, 2, 128, KS).transpose(2, 0, 1, 3)
    ).astype(ml_dtypes.float8_e4m3fn)
    bvs = np.ascontiguousarray(bv * np.float32(INV_SQRT_K))

    nc = build_nc(T)
    in_maps = [_host_inputs(x[b], wq16, wk16, wv8h, bq, bk, bvs, T) for b in range(B)]
    res = None
    last_exc = None
    for attempt in range(3):
        try:
            res = run_bass_kernel_spmd(nc, in_maps, list(range(B)), trace=TRACE)
            break
        except Exception as e:  # transient NRT device errors; retry fresh
            last_exc = e
            import time as _time
            _time.sleep(10)
            nc = build_nc(T)
    if res is None:
        raise last_exc
    LAST_RESULTS = res
    read = np.stack(
        [np.asarray(res.results[b]["out"]).astype(np.float32) for b in range(B)],
        axis=0,
    )
    return (x + read).astype(np.float32)


# revision 4
# speedup vs baseline: 1.6024x; 1.0336x over previous
"""Trainium2 Bass kernel for nn_AttentionBlock (causal attn, softmax over the
QUERY axis (dim=1), post-softmax 1/sqrt(K) scale, residual add).

Sharding: data-parallel over batch B=8, one batch element per NeuronCore.

v4 design:
- K/Q projections + logits in fp16; V projection and the probability-weighted
  read in fp8e4 with MatmulPerfMode.DoubleRow (two 128-deep contraction tiles
  per matmul).
- The causal mask is applied BY THE PE: each diagonal logits group gets one
  extra 128-wide matmul (identity stationary x f16 mask pattern of 0/-60000)
  accumulated into the PSUM bank.  exp() of -60000-ish underflows to exactly
  0.  No DVE mask-add, no staging: every chunk is max-reduced and exp'd
  straight from its PSUM bank.
- Logits PSUM banks live in ONE 6-bank tensor (pse) plus a 2-bank read tensor
  (psRD).  The first 8 logits chunks use all 8 banks (prologue), then a
  6-bank rotation.  Adjacent-bank non-diag chunk runs are processed by SINGLE
  wide ops: one DVE reduce yields 2-3 column maxes, one ACT exp covers 2-3
  chunks (one f32 accum partial per op).
- Denominator combine + reciprocal on DVE; V'-scales split DVE (even jt) /
  ACT (odd jt); output evacuation is PAIRED: one op copies both read banks
  [128,2,512] -> bf16 outst, one DMA stores 256 output rows; pairs alternate
  ACT/DVE.  Pool does only the startup memsets.
- E8[j,i] strips live in PAIRED key-chunk layout e8s[m][:, slot, :]
  (slot = jt%2, strip base column 256m) feeding DoubleRow reads directly;
  slot-1's first 128 columns are memset 0 (sub-diagonal).
    read[i, :] = sum_j E[j, i] * V'[j, :],  V' = (v + bv) * rec_j / sqrt(K)

Raw Block style with manual semaphores: ONE embedded sync-wait per
instruction; cross-engine deps are standalone wait_ge with statically
computed thresholds; same-engine RAW pairs get explicit fences.
"""

import math
import os
import sys

import numpy as np
import ml_dtypes

for _p in ("/opt/trn_rl_repo", "/root/.axon_site/_ro/trn_rl_repo"):
    if os.path.isdir(_p) and _p not in sys.path:
        sys.path.append(_p)

import concourse.bass as bass
from concourse import mybir
from concourse.bass_utils import run_bass_kernel_spmd

B = 8
D = 512
KS = 512
ND = D // 128  # 4 contraction tiles

F32 = mybir.dt.float32
F16 = mybir.dt.float16
BF16 = mybir.dt.bfloat16
F8 = mybir.dt.float8e4
AOP = mybir.AluOpType
AFT = mybir.ActivationFunctionType
DR = mybir.MatmulPerfMode.DoubleRow

INV_SQRT_K = 1.0 / math.sqrt(KS)
MASKVAL = -60000.0  # fits f16; exp(-60000 + max) == 0 exactly

TRACE = False
LAST_RESULTS = None


def _c0(jt):
    return (128 * jt) // 512


def build_nc(T=2048, debug_dump=False):
    NT = T // 128   # 16 row chunks
    NCH = T // 512  # 4 column chunks
    NP = NT // 2    # 8 key-chunk pairs
    KQ = ND * NCH   # 16 projection output groups for each of q/k

    nc = bass.Bass("TRN2", target_bir_lowering=False, debug=False, num_devices=B)

    # ---- DRAM ----
    x16_d = nc.dram_tensor("x16", [D, T], F16, kind="ExternalInput")
    x8_d = nc.dram_tensor("x8", [128, 2, 2, T], F8, kind="ExternalInput")
    wq_d = nc.dram_tensor("wq16", [D, KS], F16, kind="ExternalInput")
    wk_d = nc.dram_tensor("wk16", [D, KS], F16, kind="ExternalInput")
    wv_d = nc.dram_tensor("wv8", [128, 2, 2, KS], F8, kind="ExternalInput")
    bq_d = nc.dram_tensor("bq", [KS], F32, kind="ExternalInput")
    bk_d = nc.dram_tensor("bk", [KS], F32, kind="ExternalInput")
    bv_d = nc.dram_tensor("bv", [KS], F32, kind="ExternalInput")
    msk_d = nc.dram_tensor("msk16", [128, 128], F16, kind="ExternalInput")
    idn_d = nc.dram_tensor("idn16", [128, 128], F16, kind="ExternalInput")
    out_d = nc.dram_tensor("out", [T, KS], BF16, kind="ExternalOutput")

    # ---- SBUF ----
    xTr = nc.alloc_sbuf_tensor("xTr", [128, ND, T], F16)
    x8s = nc.alloc_sbuf_tensor("x8s", [128, 2, 2, T], F8)
    wkr = nc.alloc_sbuf_tensor("wkr", [128, ND, KS], F16)
    wqr = nc.alloc_sbuf_tensor("wqr", [128, ND, KS], F16)
    wv8 = nc.alloc_sbuf_tensor("wv8s", [128, 2, 2, KS], F8)
    kT = nc.alloc_sbuf_tensor("kT", [128, ND, T], F16)
    qT = nc.alloc_sbuf_tensor("qT", [128, ND, T], F16)
    v_sb = nc.alloc_sbuf_tensor("v_sb", [128, NT, KS], BF16)
    vp8 = nc.alloc_sbuf_tensor("vp8", [128, NP, 2, KS], F8)
    e8s = [
        nc.alloc_sbuf_tensor(f"e8_{m}", [128, 2, T - 256 * m], F8)
        for m in range(NP)
    ]
    outst = nc.alloc_sbuf_tensor("outst", [128, 2, 2, KS], BF16)
    mask16 = nc.alloc_sbuf_tensor("mask16", [128, 128], F16)
    ident16 = nc.alloc_sbuf_tensor("ident16", [128, 128], F16)
    bqc = nc.alloc_sbuf_tensor("bqc", [128, ND], F32)
    bkc = nc.alloc_sbuf_tensor("bkc", [128, ND], F32)
    bvb = nc.alloc_sbuf_tensor("bvb", [128, KS], F32)
    macc = nc.alloc_sbuf_tensor("macc", [128, 4, NCH], F32)
    negmax = nc.alloc_sbuf_tensor("negmax", [128, 4], F32)
    dparts = nc.alloc_sbuf_tensor("dparts", [128, 4, NCH], F32)
    denom = nc.alloc_sbuf_tensor("denom", [128, 4], F32)
    rec = nc.alloc_sbuf_tensor("rec", [128, 4], F32)

    # ---- PSUM: 6-bank logits rotation + 2 read banks ----
    pse = nc.alloc_psum_tensor("pse", [128, 6, 512], F32)
    psRD = nc.alloc_psum_tensor("psRD", [128, 2, 512], F32)
    # phase 1: v -> pse[0..1], k/q -> pse[2..5]

    # ================= static op-index tables =================
    LA = 3  # read lookahead: R(j) sits at fused position j + LA

    def bank_of_g(g):
        if g < 6:
            return ("E", g)
        if g < 8:
            return ("R", g - 6)
        return ("E", (g - 8) % 6)

    def prev_user_g(g):
        if g < 8:
            return None
        return g - 8 if g < 14 else g - 6

    gbank = {}
    g_of_chunk = {}
    chunk_of_g = {}
    g = 0
    for jt in range(NT):
        for ic in range(_c0(jt), NCH):
            gbank[(jt, ic)] = bank_of_g(g)
            g_of_chunk[(jt, ic)] = g
            chunk_of_g[g] = (jt, ic)
            g += 1

    # non-diag chunk groups per row: maximal runs of adjacent banks
    row_groups = {}
    for jt in range(NT):
        c0 = _c0(jt)
        ics = list(range(c0 + 1, NCH))
        groups = []
        i = 0
        while i < len(ics):
            run = [ics[i]]
            while (
                i + 1 < len(ics)
                and len(run) < 3
                and gbank[(jt, ics[i + 1])][0] == gbank[(jt, run[0])][0]
                and gbank[(jt, ics[i + 1])][1]
                == gbank[(jt, run[0])][1] + len(run)
            ):
                run.append(ics[i + 1])
                i += 1
            groups.append(run)
            i += 1
        row_groups[jt] = groups

    # ---- PE plan (sPE counts GROUPS) ----
    VG, KG, QG, KQSEQ, LG, RG = {}, {}, {}, {}, {}, {}
    pe = 0
    kqseq = 0
    p1_order = []
    for b in range(NCH):
        for r in range(4):
            pe += 1
            KG[(r, b)] = pe
            kqseq += 1
            KQSEQ[("k", r, b)] = kqseq
            p1_order.append(("k", r, b))
            jt = 4 * b + r
            if jt < NT:
                pe += 1
                VG[jt] = pe
                p1_order.append(("v", jt))
        for kt in range(ND):
            pe += 1
            QG[(kt, b)] = pe
            kqseq += 1
            KQSEQ[("q", kt, b)] = kqseq
            p1_order.append(("q", kt, b))
    assert pe == NT + 2 * KQ

    fused_order = []
    for m in range(NT):
        fused_order.append(("L", m))
        for j in range(NT):
            if j + LA == m:
                fused_order.append(("R", j))
    for j in range(NT):
        if j + LA >= NT:
            fused_order.append(("R", j))

    for kind, jt in fused_order:
        if kind == "L":
            for ic in range(_c0(jt), NCH):
                pe += 1
                LG[(jt, ic)] = pe
        else:
            pe += 1
            RG[jt] = pe

    # paired outcopies: pair pm covers read rows (2pm, 2pm+1);
    # even pm -> ACT, odd pm -> DVE; due at fused position 2pm+1+LA
    aocp_due, docp_due = {}, {}
    AOCP_TRAIL, DOCP_TRAIL = [], []
    for pm in range(NP):
        due = 2 * pm + 1 + LA
        tgt = aocp_due if pm % 2 == 0 else docp_due
        trail = AOCP_TRAIL if pm % 2 == 0 else DOCP_TRAIL
        if due < NT:
            tgt.setdefault(due, []).append(pm)
        else:
            trail.append(pm)

    # ---- ACT plan (sAC): 32 kq copies, then per jt: due paired outcopies,
    #      exps (diag first, then non-diag groups), odd-jt V'-scale ----
    EXPG = {}      # (jt, gi) -> act idx; gi 0 = diag, 1.. = groups
    EXPREL = {}    # (jt, ic) -> act idx of the exp covering the chunk
    EXP_END = {}
    NOPS = {}
    AOCP, VP8A = {}, {}
    ac = 2 * KQ
    for jt in range(NT):
        c0 = _c0(jt)
        for pm in aocp_due.get(jt, []):
            ac += 1
            AOCP[pm] = ac
        ac += 1
        EXPG[(jt, 0)] = ac  # diag
        EXPREL[(jt, c0)] = ac
        gi = 1
        for run in row_groups[jt]:
            ac += 1
            EXPG[(jt, gi)] = ac
            for ic in run:
                EXPREL[(jt, ic)] = ac
            gi += 1
        EXP_END[jt] = ac
        NOPS[jt] = gi
        if jt >= 1 and (jt - 1) % 2 == 1:
            ac += 1
            VP8A[jt - 1] = ac
    if (NT - 1) % 2 == 1:
        ac += 1
        VP8A[NT - 1] = ac
    for pm in AOCP_TRAIL:
        ac += 1
        AOCP[pm] = ac

    # ---- DVE plan (sDV): 16 v-copies, then per jt: DRED (diag max from
    #      bank; folded into NMX when nch==1), RED groups, NMX, [DENOM],
    #      RECIP, even-jt V'-scale, due paired outcopies ----
    VCP, DRED, REDG, NMX, DENOM, RECIP, VP8D, DOCP = {}, {}, {}, {}, {}, {}, {}, {}
    dv = 0
    for jt in range(NT):
        dv += 1
        VCP[jt] = dv

    def _dve_tail(jt):
        # denominator chain of row jt, emitted one block later
        nonlocal_dv = []
        return nonlocal_dv

    for jt in range(NT + 1):
        if jt < NT:
            nch = NCH - _c0(jt)
            if nch > 1:
                dv += 1
                DRED[jt] = dv
                for gi in range(len(row_groups[jt])):
                    dv += 1
                    REDG[(jt, gi)] = dv
            dv += 1
            NMX[jt] = dv
        pj = jt - 1  # previous row's denominator chain
        if 0 <= pj < NT:
            if NCH - _c0(pj) > 1:
                dv += 1
                DENOM[pj] = dv
            dv += 1
            RECIP[pj] = dv
            if pj % 2 == 0:
                dv += 1
                VP8D[pj] = dv
        if jt < NT:
            for pm in docp_due.get(jt, []):
                dv += 1
                DOCP[pm] = dv
    for pm in DOCP_TRAIL:
        dv += 1
        DOCP[pm] = dv

    # ---- Pool plan (sPO): slot-1 memsets only ----
    NMEMSET = 2 * NP

    def st_thr(pm):
        return 16 * (pm + 1)

    def bank_ap(coord, w=512):
        t, slot = coord
        if t == "E":
            return pse[:, slot, 0:w]
        return psRD[:, slot, 0:w]

    def bank_run_ap(coord, ln):
        t, slot = coord
        if t == "E":
            return pse[:, slot : slot + ln, :]
        return psRD[:, slot : slot + ln, :]

    def vp8_wait(te_or_none, jt):
        # (sem, thr) releasing vp8(jt)
        if jt % 2 == 0:
            return ("DV", VP8D[jt])
        return ("AC", VP8A[jt])

    with (
        nc.semaphore("sLv") as sLv,
        nc.semaphore("sLk") as sLk,
        nc.semaphore("sLk2") as sLk2,
        nc.semaphore("sLq") as sLq,
        nc.semaphore("sLc") as sLc,
        nc.semaphore("sLm") as sLm,
        nc.semaphore("sLx0") as sLx0,
        nc.semaphore("sLx1") as sLx1,
        nc.semaphore("sLx2") as sLx2,
        nc.semaphore("sLx3") as sLx3,
        nc.semaphore("sL80") as sL80,
        nc.semaphore("sL81") as sL81,
        nc.semaphore("sL82") as sL82,
        nc.semaphore("sL83") as sL83,
        nc.semaphore("sPE") as sPE,
        nc.semaphore("sDV") as sDV,
        nc.semaphore("sAC") as sAC,
        nc.semaphore("sPO") as sPO,
        nc.semaphore("sST") as sST,
        nc.Block() as block,
    ):
        sLxs = [sLx0, sLx1, sLx2, sLx3]
        sL8s = [sL80, sL81, sL82, sL83]

        @block.sync
        def _(sp):
            def ldx16(ic):
                sp.dma_start(
                    out=xTr[:, :, 512 * ic : 512 * (ic + 1)],
                    in_=x16_d.ap()[:, 512 * ic : 512 * (ic + 1)].rearrange(
                        "(t p) i -> p t i", p=128
                    ),
                ).then_inc(sLxs[ic], 16)

            def ldx8(ic):
                sp.dma_start(
                    out=x8s[:, :, :, 512 * ic : 512 * (ic + 1)],
                    in_=x8_d.ap()[:, :, :, 512 * ic : 512 * (ic + 1)],
                ).then_inc(sL8s[ic], 16)

            wk_re = wk_d.ap().rearrange("(t p) k -> p t k", p=128)
            sp.dma_start(out=wkr[:, :, 0:256], in_=wk_re[:, :, 0:256]).then_inc(
                sLk, 16
            )
            ldx16(0)
            sp.dma_start(out=wv8[:, :, :, :], in_=wv_d.ap()).then_inc(sLv, 16)
            ldx8(0)
            bv_ap = bv_d.ap()
            bv_bcast = bass.AP(
                tensor=bv_ap.tensor, offset=bv_ap.offset, ap=[[0, 128]] + list(bv_ap.ap)
            )
            sp.dma_start(out=bvb[:, :], in_=bv_bcast).then_inc(sLc, 16)
            with nc.allow_non_contiguous_dma(reason="16B/partition bias loads"):
                sp.dma_start(
                    out=bkc[:, :], in_=bk_d.ap().rearrange("(t p) -> p t", p=128)
                ).then_inc(sLc, 16)
                sp.dma_start(
                    out=bqc[:, :], in_=bq_d.ap().rearrange("(t p) -> p t", p=128)
                ).then_inc(sLc, 16)
            sp.dma_start(out=wkr[:, :, 256:512], in_=wk_re[:, :, 256:512]).then_inc(
                sLk2, 16
            )
            sp.dma_start(
                out=wqr[:, :, :],
                in_=wq_d.ap().rearrange("(t p) k -> p t k", p=128),
            ).then_inc(sLq, 16)
            sp.dma_start(out=mask16[:, :], in_=msk_d.ap()).then_inc(sLm, 16)
            sp.dma_start(out=ident16[:, :], in_=idn_d.ap()).then_inc(sLm, 16)
            ldx16(1)
            ldx8(1)
            ldx16(2)
            ldx8(2)
            ldx16(3)
            ldx8(3)
            # paired stores: 256 output rows each
            out_ap = out_d.ap()
            for pm in range(NP):
                if pm % 2 == 0:
                    sp.wait_ge(sAC, AOCP[pm])
                else:
                    sp.wait_ge(sDV, DOCP[pm])
                sp.dma_start(
                    out=out_ap[256 * pm : 256 * (pm + 1), :].rearrange(
                        "(s p) c -> p s c", p=128
                    ),
                    in_=outst[:, pm % 2, :, :],
                ).then_inc(sST, 16)
            sp.wait_ge(sST, 16 * NP)

        @block.tensor
        def _(te):
            waited = set()

            def ldwait(sem, thr=16):
                if sem not in waited:
                    te.wait_ge(sem, thr)
                    waited.add(sem)

            for item in p1_order:
                if item[0] == "v":
                    jt = item[1]
                    ldwait(sLv)
                    ldwait(sL8s[jt // 4])
                    if jt >= 2:
                        te.wait_ge(sDV, VCP[jt - 2])
                    for dm in range(2):
                        mm = te.matmul(
                            pse[:, jt % 2, :],
                            lhsT=x8s[:, dm, :, 128 * jt : 128 * (jt + 1)],
                            rhs=wv8[:, dm, :, :],
                            start=(dm == 0),
                            stop=(dm == 1),
                            perf_mode=DR,
                        )
                        if dm == 1:
                            mm.then_inc(sPE, 1)
                else:
                    kind, kt, ic = item
                    wsb = wkr if kind == "k" else wqr
                    if kind == "k":
                        ldwait(sLk if kt < 2 else sLk2)
                    else:
                        ldwait(sLq)
                    ldwait(sLxs[ic])
                    seq = KQSEQ[(kind, kt, ic)]
                    if seq > 4:
                        te.wait_ge(sAC, seq - 4)
                    for dt_ in range(ND):
                        mm = te.matmul(
                            pse[:, 2 + ((seq - 1) % 4), :],
                            lhsT=wsb[:, dt_, 128 * kt : 128 * (kt + 1)],
                            rhs=xTr[:, dt_, 512 * ic : 512 * (ic + 1)],
                            start=(dt_ == 0),
                            stop=(dt_ == ND - 1),
                        )
                        if dt_ == ND - 1:
                            mm.then_inc(sPE, 1)
            # fused: logits chunks (mask matmul appended to diag groups) +
            # DoubleRow read groups
            for kind, jt in fused_order:
                c0 = _c0(jt)
                if kind == "L":
                    for ic in range(c0, NCH):
                        gg = g_of_chunk[(jt, ic)]
                        diag = ic == c0
                        need_ac = KQSEQ[("q", ND - 1, ic)]
                        need_dv = None
                        pg = prev_user_g(gg)
                        if pg is not None:
                            pj, pic = chunk_of_g[pg]
                            need_ac = max(need_ac, EXPREL[(pj, pic)])
                        elif gg < 2:
                            need_dv = VCP[NT - 2 + gg]
                        elif gg < 6:
                            need_ac = max(need_ac, 2 * KQ)
                        te.wait_ge(sAC, need_ac)
                        if need_dv is not None:
                            te.wait_ge(sDV, need_dv)
                        if diag:
                            ldwait(sLm, 32)
                        w = 512 * (ic + 1) - (128 * jt if diag else 512 * ic)
                        bank = bank_ap(gbank[(jt, ic)], w)
                        ilo = 128 * jt if diag else 512 * ic
                        for kt in range(ND):
                            mm = te.matmul(
                                bank,
                                lhsT=kT[:, kt, 128 * jt : 128 * (jt + 1)],
                                rhs=qT[:, kt, ilo : 512 * (ic + 1)],
                                start=(kt == 0),
                                stop=(kt == ND - 1) and not diag,
                            )
                            if kt == ND - 1 and not diag:
                                mm.then_inc(sPE, 1)
                        if diag:
                            # causal mask accumulated by the PE: identity
                            # stationary x (0/-60000) f16 pattern
                            te.matmul(
                                bank_ap(gbank[(jt, ic)], 128),
                                lhsT=ident16[:, :],
                                rhs=mask16[:, :],
                                start=False,
                                stop=True,
                                skip_group_check=True,
                            ).then_inc(sPE, 1)
                else:
                    npair = (jt + 2) // 2
                    if jt >= 1:
                        sem, thr = vp8_wait(None, jt - 1)
                        te.wait_ge(sPO if sem == "PO" else (sDV if sem == "DV" else sAC), thr)
                    if jt < 2:
                        te.wait_ge(sAC, EXPREL[(1, 2)])
                    else:
                        pm = (jt - 2) // 2
                        if pm % 2 == 0:
                            te.wait_ge(sAC, AOCP[pm])
                        else:
                            te.wait_ge(sDV, DOCP[pm])
                    if jt == 0:
                        te.wait_ge(sPO, NMEMSET)
                    for m in range(npair):
                        if m == npair - 1:
                            sem, thr = vp8_wait(None, jt)
                            te.wait_ge(sDV if sem == "DV" else sAC, thr)
                        mm = te.matmul(
                            psRD[:, jt % 2, :],
                            lhsT=e8s[m][
                                :, :, 128 * jt - 256 * m : 128 * jt - 256 * m + 128
                            ],
                            rhs=vp8[:, m, :, :],
                            start=(m == 0),
                            stop=(m == npair - 1),
                            perf_mode=DR,
                        )
                        if m == npair - 1:
                            mm.then_inc(sPE, 1)

        @block.vector
        def _(ve):
            ndv = 0

            def inc(x):
                nonlocal ndv
                ndv += 1
                x.then_inc(sDV, 1)

            ve.wait_ge(sLc, 16)
            for jt in range(NT):
                ve.wait_ge(sPE, VG[jt])
                inc(
                    ve.tensor_tensor(
                        out=v_sb[:, jt, :], in0=pse[:, jt % 2, :],
                        in1=bvb[:, :], op=AOP.add,
                    )
                )
                assert ndv == VCP[jt]

            def den_chain(rj):
                # denominator chain of row rj (one block late)
                rp = rj % 4
                ve.wait_ge(sAC, EXP_END[rj])
                if NCH - _c0(rj) > 1:
                    inc(
                        ve.reduce_sum(
                            denom[:, rp : rp + 1],
                            dparts[:, rp, 0 : NOPS[rj]],
                            mybir.AxisListType.X,
                        )
                    )
                    assert ndv == DENOM[rj]
                    ve.wait_ge(sDV, DENOM[rj])  # same-engine RAW fence
                    src = denom[:, rp : rp + 1]
                else:
                    src = dparts[:, rp, 0:1]
                inc(ve.reciprocal(rec[:, rp : rp + 1], src))
                assert ndv == RECIP[rj]
                if rj % 2 == 0:
                    ve.wait_ge(sDV, RECIP[rj])  # same-engine RAW fence
                    inc(
                        ve.tensor_scalar(
                            out=vp8[:, rj // 2, (rj % 2), :],
                            in0=v_sb[:, rj, :],
                            scalar1=rec[:, rp : rp + 1],
                            scalar2=None,
                            op0=AOP.mult,
                        )
                    )
                    assert ndv == VP8D[rj]

            for jt in range(NT):
                c0 = _c0(jt)
                nch = NCH - c0
                w0 = 512 * (c0 + 1) - 128 * jt
                pj = jt % 4
                if jt >= 4:
                    # macc/negmax[jt%4] reuse: exps of jt-4 read them
                    ve.wait_ge(sAC, EXP_END[jt - 4])
                ve.wait_ge(sPE, LG[(jt, c0)])
                if nch > 1:
                    inc(
                        ve.reduce_max(
                            macc[:, pj, c0 : c0 + 1],
                            bank_ap(gbank[(jt, c0)], w0),
                            mybir.AxisListType.X,
                        )
                    )
                    assert ndv == DRED[jt]
                    for gi, run in enumerate(row_groups[jt]):
                        ve.wait_ge(sPE, LG[(jt, run[-1])])
                        inc(
                            ve.reduce_max(
                                macc[:, pj, run[0] : run[0] + len(run)],
                                bank_run_ap(gbank[(jt, run[0])], len(run)),
                                mybir.AxisListType.X,
                            )
                        )
                        assert ndv == REDG[(jt, gi)]
                    ve.wait_ge(sDV, REDG[(jt, len(row_groups[jt]) - 1)])
                    inc(
                        ve.reduce_max(
                            negmax[:, pj : pj + 1],
                            macc[:, pj, c0:NCH],
                            mybir.AxisListType.X, negate=True,
                        )
                    )
                else:
                    inc(
                        ve.reduce_max(
                            negmax[:, pj : pj + 1],
                            bank_ap(gbank[(jt, c0)], w0),
                            mybir.AxisListType.X, negate=True,
                        )
                    )
                assert ndv == NMX[jt]
                if jt >= 1:
                    den_chain(jt - 1)
                for pm in docp_due.get(jt, []):
                    ve.wait_ge(sPE, RG[2 * pm + 1])
                    if pm >= 2:
                        ve.wait_ge(sST, st_thr(pm - 2))
                    inc(
                        ve.tensor_scalar_add(
                            out=outst[:, pm % 2, :, :], in0=psRD[:, :, :],
                            scalar1=0.0,
                        )
                    )
                    assert ndv == DOCP[pm]
            den_chain(NT - 1)
            for pm in DOCP_TRAIL:
                ve.wait_ge(sPE, RG[2 * pm + 1])
                if pm >= 2:
                    ve.wait_ge(sST, st_thr(pm - 2))
                inc(
                    ve.tensor_scalar_add(
                        out=outst[:, pm % 2, :, :], in0=psRD[:, :, :],
                        scalar1=0.0,
                    )
                )
                assert ndv == DOCP[pm]

        @block.scalar
        def _(ac_):
            ac_.wait_ge(sLc, 48)
            nac = 0
            for ic in range(NCH):
                for wsel, g_tab, bias in ((0, KG, bkc), (1, QG, bqc)):
                    dst = kT if wsel == 0 else qT
                    for kt in range(ND):
                        seq = KQSEQ[("k" if wsel == 0 else "q", kt, ic)]
                        ac_.wait_ge(sPE, g_tab[(kt, ic)])
                        ac_.activation(
                            out=dst[:, kt, 512 * ic : 512 * (ic + 1)],
                            in_=pse[:, 2 + ((seq - 1) % 4), :],
                            func=AFT.Identity,
                            bias=bias[:, kt : kt + 1],
                            scale=1.0,
                        ).then_inc(sAC, 1)
                        nac += 1
            assert nac == 2 * KQ

            def outcopy(pm):
                nonlocal nac
                ac_.wait_ge(sPE, RG[2 * pm + 1])
                if pm >= 2:
                    ac_.wait_ge(sST, st_thr(pm - 2))
                nac += 1
                ac_.activation(
                    out=outst[:, pm % 2, :, :], in_=psRD[:, :, :], func=AFT.Copy
                ).then_inc(sAC, 1)
                assert nac == AOCP[pm]

            def act_vp8(rj):
                nonlocal nac
                rp = rj % 4
                ac_.wait_ge(sDV, RECIP[rj])
                nac += 1
                ac_.activation(
                    out=vp8[:, rj // 2, rj % 2, :],
                    in_=v_sb[:, rj, :],
                    func=AFT.Copy,
                    bias=0.0,
                    scale=rec[:, rp : rp + 1],
                ).then_inc(sAC, 1)
                assert nac == VP8A[rj]

            for jt in range(NT):
                c0 = _c0(jt)
                pj = jt % 4
                m = jt // 2
                base = 256 * m
                for pm in aocp_due.get(jt, []):
                    outcopy(pm)
                # exps: diag first (frees the rotation bank soonest)
                ac_.wait_ge(sDV, NMX[jt])
                if jt >= 4:
                    # dparts[jt%4] reuse: recip(jt-4) must have read it
                    ac_.wait_ge(sDV, RECIP[jt - 4])
                w0 = 512 * (c0 + 1) - 128 * jt
                nac += 1
                ac_.activation(
                    out=e8s[m][:, jt % 2, 128 * jt - base : 512 * (c0 + 1) - base],
                    in_=bank_ap(gbank[(jt, c0)], w0),
                    func=AFT.Exp,
                    bias=negmax[:, pj : pj + 1],
                    scale=1.0,
                    accum_out=dparts[:, pj, 0:1],
                ).then_inc(sAC, 1)
                assert nac == EXPG[(jt, 0)]
                sidx = 1
                for run in row_groups[jt]:
                    a = 512 * run[0]
                    bcol = 512 * (run[-1] + 1)
                    nac += 1
                    ac_.activation(
                        out=e8s[m][:, jt % 2, a - base : bcol - base],
                        in_=bank_run_ap(gbank[(jt, run[0])], len(run)),
                        func=AFT.Exp,
                        bias=negmax[:, pj : pj + 1],
                        scale=1.0,
                        accum_out=dparts[:, pj, sidx : sidx + 1],
                    ).then_inc(sAC, 1)
                    assert nac == EXPG[(jt, sidx)]
                    sidx += 1
                assert sidx == NOPS[jt]
                if jt >= 1 and (jt - 1) % 2 == 1:
                    act_vp8(jt - 1)
            if (NT - 1) % 2 == 1:
                act_vp8(NT - 1)
            for pm in AOCP_TRAIL:
                outcopy(pm)

        @block.gpsimd
        def _(po):
            npo = 0
            for m in range(NP):
                po.memset(e8s[m][:, 1, 0:128], 0.0).then_inc(sPO, 1)
                npo += 1
                po.memset(vp8[:, m, 1, :], 0.0).then_inc(sPO, 1)
                npo += 1
            assert npo == NMEMSET

    nc.finalize()
    return nc


def _host_inputs(xb, wq16, wk16, wv8h, bq, bk, bv, T):
    # mask class 0 for the (narrowed) diagonal chunk: cols x < p get MASKVAL
    p = np.arange(128, dtype=np.float32)[:, None]
    xx = np.arange(128, dtype=np.float32)[None, :]
    msk = np.where(xx >= p, 0.0, MASKVAL).astype(np.float16)
    idn = np.eye(128, dtype=np.float16)

    xT = np.ascontiguousarray(xb.T)  # [D, T] f32
    x16 = xT.astype(np.float16)
    x8 = np.ascontiguousarray(
        xT.reshape(2, 2, 128, T).transpose(2, 0, 1, 3)
    ).astype(ml_dtypes.float8_e4m3fn)
    return dict(
        x16=x16,
        x8=np.ascontiguousarray(x8),
        wq16=wq16,
        wk16=wk16,
        wv8=wv8h,
        bq=bq,
        bk=bk,
        bv=bv,
        msk16=np.ascontiguousarray(msk),
        idn16=np.ascontiguousarray(idn),
    )


def kernel(x, Wk, bk, Wq, bq, Wv, bv):
    global LAST_RESULTS
    T = 2048
    x = np.ascontiguousarray(np.asarray(x, dtype=np.float32))
    Wk = np.asarray(Wk, dtype=np.float32)
    Wq = np.asarray(Wq, dtype=np.float32)
    Wv = np.asarray(Wv, dtype=np.float32)
    bk = np.ascontiguousarray(np.asarray(bk, dtype=np.float32))
    bq = np.ascontiguousarray(np.asarray(bq, dtype=np.float32))
    bv = np.ascontiguousarray(np.asarray(bv, dtype=np.float32))

    wq16 = np.ascontiguousarray(Wq.T).astype(np.float16)
    wk16 = np.ascontiguousarray(Wk.T).astype(np.float16)
    wvT = Wv.T * np.float32(INV_SQRT_K)  # [D, KS]
    wv8h = np.ascontiguousarray(
        wvT.reshape(2, 2, 128, KS).transpose(2, 0, 1, 3)
    ).astype(ml_dtypes.float8_e4m3fn)
    bvs = np.ascontiguousarray(bv * np.float32(INV_SQRT_K))

    nc = build_nc(T)
    in_maps = [_host_inputs(x[b], wq16, wk16, wv8h, bq, bk, bvs, T) for b in range(B)]
    res = None
    last_exc = None
    for attempt in range(3):
        try:
            res = run_bass_kernel_spmd(nc, in_maps, list(range(B)), trace=TRACE)
            break
        except Exception as e:  # transient NRT device errors; retry fresh
            last_exc = e
            import time as _time
            _time.sleep(10)
            nc = build_nc(T)
    if res is None:
        raise last_exc
    LAST_RESULTS = res
    read = np.stack(
        [np.asarray(res.results[b]["out"]).astype(np.float32) for b in range(B)],
        axis=0,
    )
    return (x + read).astype(np.float32)


# revision 5
# speedup vs baseline: 1.6690x; 1.0416x over previous
"""Trainium2 Bass kernel for nn_AttentionBlock (causal attn, softmax over the
QUERY axis (dim=1), post-softmax 1/sqrt(K) scale, residual add).

Sharding: data-parallel over batch B=8, one batch element per NeuronCore.

v4 design:
- K/Q projections + logits in fp16; V projection and the probability-weighted
  read in fp8e4 with MatmulPerfMode.DoubleRow (two 128-deep contraction tiles
  per matmul).
- The causal mask is applied BY THE PE: each diagonal logits group gets one
  extra 128-wide matmul (identity stationary x f16 mask pattern of 0/-60000)
  accumulated into the PSUM bank.  exp() of -60000-ish underflows to exactly
  0.  No DVE mask-add, no staging: every chunk is max-reduced and exp'd
  straight from its PSUM bank.
- Logits PSUM banks live in ONE 6-bank tensor (pse) plus a 2-bank read tensor
  (psRD).  The first 8 logits chunks use all 8 banks (prologue), then a
  6-bank rotation.  Adjacent-bank non-diag chunk runs are processed by SINGLE
  wide ops: one DVE reduce yields 2-3 column maxes, one ACT exp covers 2-3
  chunks (one f32 accum partial per op).
- Denominator combine + reciprocal on DVE; V'-scales split DVE (even jt) /
  ACT (odd jt); output evacuation is PAIRED: one op copies both read banks
  [128,2,512] -> bf16 outst, one DMA stores 256 output rows; pairs alternate
  ACT/DVE.  Pool does only the startup memsets.
- E8[j,i] strips live in PAIRED key-chunk layout e8s[m][:, slot, :]
  (slot = jt%2, strip base column 256m) feeding DoubleRow reads directly;
  slot-1's first 128 columns are memset 0 (sub-diagonal).
    read[i, :] = sum_j E[j, i] * V'[j, :],  V' = (v + bv) * rec_j / sqrt(K)

Raw Block style with manual semaphores: ONE embedded sync-wait per
instruction; cross-engine deps are standalone wait_ge with statically
computed thresholds; same-engine RAW pairs get explicit fences.
"""

import math
import os
import sys

import numpy as np
import ml_dtypes

for _p in ("/opt/trn_rl_repo", "/root/.axon_site/_ro/trn_rl_repo"):
    if os.path.isdir(_p) and _p not in sys.path:
        sys.path.append(_p)

import concourse.bass as bass
from concourse import mybir
from concourse.bass_utils import run_bass_kernel_spmd

B = 8
D = 512
KS = 512
ND = D // 128  # 4 contraction tiles

F32 = mybir.dt.float32
F16 = mybir.dt.float16
BF16 = mybir.dt.bfloat16
F8 = mybir.dt.float8e4
AOP = mybir.AluOpType
AFT = mybir.ActivationFunctionType
DR = mybir.MatmulPerfMode.DoubleRow

INV_SQRT_K = 1.0 / math.sqrt(KS)
MASKVAL = -60000.0  # fits f16; exp(-60000 + max) == 0 exactly

TRACE = False
LAST_RESULTS = None


def _c0(jt):
    return (128 * jt) // 512


def build_nc(T=2048, debug_dump=False):
    NT = T // 128   # 16 row chunks
    NCH = T // 512  # 4 column chunks
    NP = NT // 2    # 8 key-chunk pairs
    KQ = ND * NCH   # 16 projection output groups for each of q/k

    nc = bass.Bass("TRN2", target_bir_lowering=False, debug=False, num_devices=B)

    # ---- DRAM ----
    x16_d = nc.dram_tensor("x16", [D, T], F16, kind="ExternalInput")
    x8_d = nc.dram_tensor("x8", [128, 2, 2, T], F8, kind="ExternalInput")
    wq_d = nc.dram_tensor("wq16", [D, KS], F16, kind="ExternalInput")
    wk_d = nc.dram_tensor("wk16", [D, KS], F16, kind="ExternalInput")
    wv_d = nc.dram_tensor("wv8", [128, 2, 2, KS], F8, kind="ExternalInput")
    bq_d = nc.dram_tensor("bq", [KS], F32, kind="ExternalInput")
    bk_d = nc.dram_tensor("bk", [KS], F32, kind="ExternalInput")
    bv_d = nc.dram_tensor("bv", [KS], F32, kind="ExternalInput")
    msk_d = nc.dram_tensor("msk16", [128, 128], F16, kind="ExternalInput")
    idn_d = nc.dram_tensor("idn16", [128, 128], F16, kind="ExternalInput")
    out_d = nc.dram_tensor("out", [T, KS], BF16, kind="ExternalOutput")

    # ---- SBUF ----
    xTr = nc.alloc_sbuf_tensor("xTr", [128, ND, T], F16)
    x8s = nc.alloc_sbuf_tensor("x8s", [128, 2, 2, T], F8)
    wkr = nc.alloc_sbuf_tensor("wkr", [128, ND, KS], F16)
    wqr = nc.alloc_sbuf_tensor("wqr", [128, ND, KS], F16)
    wv8 = nc.alloc_sbuf_tensor("wv8s", [128, 2, 2, KS], F8)
    kT = nc.alloc_sbuf_tensor("kT", [128, ND, T], F16)
    qT = nc.alloc_sbuf_tensor("qT", [128, ND, T], F16)
    v_sb = nc.alloc_sbuf_tensor("v_sb", [128, NT, KS], BF16)
    vp8 = nc.alloc_sbuf_tensor("vp8", [128, NP, 2, KS], F8)
    e8s = [
        nc.alloc_sbuf_tensor(f"e8_{m}", [128, 2, T - 256 * m], F8)
        for m in range(NP)
    ]
    outst = nc.alloc_sbuf_tensor("outst", [128, 2, 2, KS], BF16)
    mask16 = nc.alloc_sbuf_tensor("mask16", [128, 128], F16)
    ident16 = nc.alloc_sbuf_tensor("ident16", [128, 128], F16)
    bqc = nc.alloc_sbuf_tensor("bqc", [128, ND], F32)
    bkc = nc.alloc_sbuf_tensor("bkc", [128, ND], F32)
    bvb = nc.alloc_sbuf_tensor("bvb", [128, KS], F32)
    macc = nc.alloc_sbuf_tensor("macc", [128, 4, NCH], F32)
    negmax = nc.alloc_sbuf_tensor("negmax", [128, 4], F32)
    dparts = nc.alloc_sbuf_tensor("dparts", [128, 4, NCH], F32)
    denom = nc.alloc_sbuf_tensor("denom", [128, 4], F32)
    rec = nc.alloc_sbuf_tensor("rec", [128, 4], F32)

    # ---- PSUM: 6-bank logits rotation + 2 read banks ----
    pse = nc.alloc_psum_tensor("pse", [128, 6, 512], F32)
    psRD = nc.alloc_psum_tensor("psRD", [128, 2, 512], F32)
    # phase 1: v -> pse[0..1], k/q -> pse[2..5]

    # ================= static op-index tables =================
    LA = 3  # read lookahead: R(j) sits at fused position j + LA

    def bank_of_g(g):
        if g < 6:
            return ("E", g)
        if g < 8:
            return ("R", g - 6)
        return ("E", (g - 8) % 6)

    def prev_user_g(g):
        if g < 8:
            return None
        return g - 8 if g < 14 else g - 6

    gbank = {}
    g_of_chunk = {}
    chunk_of_g = {}
    g = 0
    for jt in range(NT):
        for ic in range(_c0(jt), NCH):
            gbank[(jt, ic)] = bank_of_g(g)
            g_of_chunk[(jt, ic)] = g
            chunk_of_g[g] = (jt, ic)
            g += 1

    # non-diag chunk groups per row: maximal runs of adjacent banks
    row_groups = {}
    for jt in range(NT):
        c0 = _c0(jt)
        ics = list(range(c0 + 1, NCH))
        groups = []
        i = 0
        while i < len(ics):
            run = [ics[i]]
            while (
                i + 1 < len(ics)
                and len(run) < 3
                and gbank[(jt, ics[i + 1])][0] == gbank[(jt, run[0])][0]
                and gbank[(jt, ics[i + 1])][1]
                == gbank[(jt, run[0])][1] + len(run)
            ):
                run.append(ics[i + 1])
                i += 1
            groups.append(run)
            i += 1
        row_groups[jt] = groups

    # ---- PE plan (sPE counts GROUPS) ----
    VG, KG, QG, KQSEQ, LG, RG = {}, {}, {}, {}, {}, {}
    pe = 0
    kqseq = 0
    p1_order = []
    for b in range(NCH):
        for r in range(4):
            pe += 1
            KG[(r, b)] = pe
            kqseq += 1
            KQSEQ[("k", r, b)] = kqseq
            p1_order.append(("k", r, b))
            jt = 4 * b + r
            if jt < NT:
                pe += 1
                VG[jt] = pe
                p1_order.append(("v", jt))
        for kt in range(ND):
            pe += 1
            QG[(kt, b)] = pe
            kqseq += 1
            KQSEQ[("q", kt, b)] = kqseq
            p1_order.append(("q", kt, b))
    assert pe == NT + 2 * KQ

    fused_order = []
    for m in range(NT):
        fused_order.append(("L", m))
        for j in range(NT):
            if j + LA == m:
                fused_order.append(("R", j))
    for j in range(NT):
        if j + LA >= NT:
            fused_order.append(("R", j))

    for kind, jt in fused_order:
        if kind == "L":
            for ic in range(_c0(jt), NCH):
                pe += 1
                LG[(jt, ic)] = pe
        else:
            pe += 1
            RG[jt] = pe

    # paired outcopies: pair pm covers read rows (2pm, 2pm+1);
    # even pm -> ACT, odd pm -> DVE; due at fused position 2pm+1+LA
    aocp_due, docp_due = {}, {}
    AOCP_TRAIL, DOCP_TRAIL = [], []
    NPAIRED = 0  # pairs 0..5 cover rows 0..11; rows 12..15 get single OCs
    for pm in range(NPAIRED):
        due = 2 * pm + 1 + LA
        tgt = aocp_due if pm % 2 == 0 else docp_due
        trail = AOCP_TRAIL if pm % 2 == 0 else DOCP_TRAIL
        if due < NT:
            tgt.setdefault(due, []).append(pm)
        else:
            trail.append(pm)
    # single outcopies for rows 12..15: ACT even rows, DVE odd rows
    aoc1_due, doc1_due = {}, {}
    AOC1_TRAIL, DOC1_TRAIL = [], []
    NSING0 = 2 * NPAIRED  # first single-OC row
    for j in range(NSING0, NT):
        due = j + LA
        tgt = aoc1_due if j % 2 == 0 else doc1_due
        trail = AOC1_TRAIL if j % 2 == 0 else DOC1_TRAIL
        if due < NT:
            tgt.setdefault(due, []).append(j)
        else:
            trail.append(j)

    # ---- ACT plan (sAC): 32 kq copies, then per jt: due paired outcopies,
    #      exps (diag first, then non-diag groups), odd-jt V'-scale ----
    EXPG = {}      # (jt, gi) -> act idx; gi 0 = diag, 1.. = groups
    EXPREL = {}    # (jt, ic) -> act idx of the exp covering the chunk
    EXP_END = {}
    NOPS = {}
    AOCP, VP8A, AOC1 = {}, {}, {}
    ac = 2 * KQ
    for jt in range(NT):
        c0 = _c0(jt)
        for pm in aocp_due.get(jt, []):
            ac += 1
            AOCP[pm] = ac
        ac += 1
        EXPG[(jt, 0)] = ac  # diag
        EXPREL[(jt, c0)] = ac
        gi = 1
        for run in row_groups[jt]:
            ac += 1
            EXPG[(jt, gi)] = ac
            for ic in run:
                EXPREL[(jt, ic)] = ac
            gi += 1
        EXP_END[jt] = ac
        NOPS[jt] = gi
        if jt >= 1 and (jt - 1) % 2 == 1:
            ac += 1
            VP8A[jt - 1] = ac
        for j in aoc1_due.get(jt, []):
            ac += 1
            AOC1[j] = ac
    if (NT - 1) % 2 == 1:
        ac += 1
        VP8A[NT - 1] = ac
    for pm in AOCP_TRAIL:
        ac += 1
        AOCP[pm] = ac
    for j in AOC1_TRAIL:
        ac += 1
        AOC1[j] = ac

    # ---- DVE plan (sDV): 16 v-copies, then per jt: DRED (diag max from
    #      bank; folded into NMX when nch==1), RED groups, NMX, [DENOM],
    #      RECIP, even-jt V'-scale, due paired outcopies ----
    VCP, DRED, REDG, NMX, DENOM, RECIP, VP8D, DOCP, DOC1 = {}, {}, {}, {}, {}, {}, {}, {}, {}
    dv = 0
    for jt in range(NT):
        dv += 1
        VCP[jt] = dv

    def _dve_tail(jt):
        # denominator chain of row jt, emitted one block later
        nonlocal_dv = []
        return nonlocal_dv

    for jt in range(NT + 1):
        if jt < NT:
            nch = NCH - _c0(jt)
            if nch > 1:
                dv += 1
                DRED[jt] = dv
                for gi in range(len(row_groups[jt])):
                    dv += 1
                    REDG[(jt, gi)] = dv
            dv += 1
            NMX[jt] = dv
        pj = jt - 1  # previous row's denominator chain
        if 0 <= pj < NT:
            if NCH - _c0(pj) > 1:
                dv += 1
                DENOM[pj] = dv
            dv += 1
            RECIP[pj] = dv
            if pj % 2 == 0:
                dv += 1
                VP8D[pj] = dv
        if jt < NT:
            for pm in docp_due.get(jt, []):
                dv += 1
                DOCP[pm] = dv
            for j in doc1_due.get(jt, []):
                dv += 1
                DOC1[j] = dv
    for pm in DOCP_TRAIL:
        dv += 1
        DOCP[pm] = dv
    for j in DOC1_TRAIL:
        dv += 1
        DOC1[j] = dv

    # ---- Pool plan (sPO): slot-1 memsets only ----
    NMEMSET = 2 * NP

    def st_thr(pm):
        # paired store pm is the (pm+1)-th store
        return 16 * (pm + 1)

    def st1_thr(j):
        # single store for row j is the (NPAIRED + j - NSING0 + 1)-th store
        return 16 * (NPAIRED + j - NSING0 + 1)

    def oc1_st_wait(j):
        # outst slot (s2=(j//2)%2, s1=j%2) previously used by single j-4 or
        # by pair (j-4)//2
        if j - 4 >= NSING0:
            return st1_thr(j - 4)
        return st_thr((j - 4) // 2)

    def bank_ap(coord, w=512):
        t, slot = coord
        if t == "E":
            return pse[:, slot, 0:w]
        return psRD[:, slot, 0:w]

    def bank_run_ap(coord, ln):
        t, slot = coord
        if t == "E":
            return pse[:, slot : slot + ln, :]
        return psRD[:, slot : slot + ln, :]

    def vp8_wait(te_or_none, jt):
        # (sem, thr) releasing vp8(jt)
        if jt % 2 == 0:
            return ("DV", VP8D[jt])
        return ("AC", VP8A[jt])

    with (
        nc.semaphore("sLv") as sLv,
        nc.semaphore("sLk") as sLk,
        nc.semaphore("sLk2") as sLk2,
        nc.semaphore("sLq") as sLq,
        nc.semaphore("sLc") as sLc,
        nc.semaphore("sLm") as sLm,
        nc.semaphore("sLx0") as sLx0,
        nc.semaphore("sLxb") as sLxb,
        nc.semaphore("sLx1") as sLx1,
        nc.semaphore("sLx2") as sLx2,
        nc.semaphore("sLx3") as sLx3,
        nc.semaphore("sL80") as sL80,
        nc.semaphore("sL81") as sL81,
        nc.semaphore("sL82") as sL82,
        nc.semaphore("sL83") as sL83,
        nc.semaphore("sPE") as sPE,
        nc.semaphore("sDV") as sDV,
        nc.semaphore("sAC") as sAC,
        nc.semaphore("sPO") as sPO,
        nc.semaphore("sST") as sST,
        nc.Block() as block,
    ):
        sLxs = [sLx0, sLx1, sLx2, sLx3]
        sL8s = [sL80, sL81, sL82, sL83]

        @block.sync
        def _(sp):
            def ldx16(ic):
                sp.dma_start(
                    out=xTr[:, :, 512 * ic : 512 * (ic + 1)],
                    in_=x16_d.ap()[:, 512 * ic : 512 * (ic + 1)].rearrange(
                        "(t p) i -> p t i", p=128
                    ),
                ).then_inc(sLxs[ic], 16)

            def ldx8(ic):
                sp.dma_start(
                    out=x8s[:, :, :, 512 * ic : 512 * (ic + 1)],
                    in_=x8_d.ap()[:, :, :, 512 * ic : 512 * (ic + 1)],
                ).then_inc(sL8s[ic], 16)

            wk_re = wk_d.ap().rearrange("(t p) k -> p t k", p=128)
            sp.dma_start(out=wkr[:, :, 0:256], in_=wk_re[:, :, 0:256]).then_inc(
                sLk, 16
            )
            x16_re0 = x16_d.ap()[:, 0:512].rearrange("(t p) i -> p t i", p=128)
            sp.dma_start(out=xTr[:, :, 0:256], in_=x16_re0[:, :, 0:256]).then_inc(
                sLx0, 16
            )
            sp.dma_start(out=xTr[:, :, 256:512], in_=x16_re0[:, :, 256:512]).then_inc(
                sLxb, 16
            )
            sp.dma_start(out=wv8[:, :, :, :], in_=wv_d.ap()).then_inc(sLv, 16)
            ldx8(0)
            bv_ap = bv_d.ap()
            bv_bcast = bass.AP(
                tensor=bv_ap.tensor, offset=bv_ap.offset, ap=[[0, 128]] + list(bv_ap.ap)
            )
            sp.dma_start(out=bvb[:, :], in_=bv_bcast).then_inc(sLc, 16)
            with nc.allow_non_contiguous_dma(reason="16B/partition bias loads"):
                sp.dma_start(
                    out=bkc[:, :], in_=bk_d.ap().rearrange("(t p) -> p t", p=128)
                ).then_inc(sLc, 16)
                sp.dma_start(
                    out=bqc[:, :], in_=bq_d.ap().rearrange("(t p) -> p t", p=128)
                ).then_inc(sLc, 16)
            sp.dma_start(out=wkr[:, :, 256:512], in_=wk_re[:, :, 256:512]).then_inc(
                sLk2, 16
            )
            sp.dma_start(
                out=wqr[:, :, :],
                in_=wq_d.ap().rearrange("(t p) k -> p t k", p=128),
            ).then_inc(sLq, 16)
            sp.dma_start(out=mask16[:, :], in_=msk_d.ap()).then_inc(sLm, 16)
            sp.dma_start(out=ident16[:, :], in_=idn_d.ap()).then_inc(sLm, 16)
            ldx16(1)
            ldx8(1)
            ldx16(2)
            ldx8(2)
            ldx16(3)
            ldx8(3)
            # paired stores: 256 output rows each
            out_ap = out_d.ap()
            for pm in range(NPAIRED):
                if pm % 2 == 0:
                    sp.wait_ge(sAC, AOCP[pm])
                else:
                    sp.wait_ge(sDV, DOCP[pm])
                sp.dma_start(
                    out=out_ap[256 * pm : 256 * (pm + 1), :].rearrange(
                        "(s p) c -> p s c", p=128
                    ),
                    in_=outst[:, pm % 2, :, :],
                ).then_inc(sST, 16)
            for j in range(NSING0, NT):
                if j % 2 == 0:
                    sp.wait_ge(sAC, AOC1[j])
                else:
                    sp.wait_ge(sDV, DOC1[j])
                sp.dma_start(
                    out=out_ap[128 * j : 128 * (j + 1), :],
                    in_=outst[:, (j // 2) % 2, j % 2, :],
                ).then_inc(sST, 16)
            sp.wait_ge(sST, 16 * (NPAIRED + NT - NSING0))

        @block.tensor
        def _(te):
            waited = set()

            def ldwait(sem, thr=16):
                if sem not in waited:
                    te.wait_ge(sem, thr)
                    waited.add(sem)

            for item in p1_order:
                if item[0] == "v":
                    jt = item[1]
                    ldwait(sLv)
                    ldwait(sL8s[jt // 4])
                    if jt >= 2:
                        te.wait_ge(sDV, VCP[jt - 2])
                    for dm in range(2):
                        mm = te.matmul(
                            pse[:, jt % 2, :],
                            lhsT=x8s[:, dm, :, 128 * jt : 128 * (jt + 1)],
                            rhs=wv8[:, dm, :, :],
                            start=(dm == 0),
                            stop=(dm == 1),
                            perf_mode=DR,
                        )
                        if dm == 1:
                            mm.then_inc(sPE, 1)
                else:
                    kind, kt, ic = item
                    wsb = wkr if kind == "k" else wqr
                    if kind == "k":
                        ldwait(sLk if kt < 2 else sLk2)
                    else:
                        ldwait(sLq)
                    ldwait(sLxs[ic])
                    if ic == 0 and not (kind == "k" and kt == 0):
                        ldwait(sLxb)
                    seq = KQSEQ[(kind, kt, ic)]
                    if seq > 4:
                        te.wait_ge(sAC, seq - 4)
                    if kind == "k" and ic == 0:
                        # two half-width groups so the first k's start on the
                        # first half-load of x16 strip 0
                        for h in range(2):
                            if h == 1:
                                ldwait(sLxb)
                            for dt_ in range(ND):
                                mm = te.matmul(
                                    pse[:, 2 + ((seq - 1) % 4), 256 * h : 256 * (h + 1)],
                                    lhsT=wsb[:, dt_, 128 * kt : 128 * (kt + 1)],
                                    rhs=xTr[:, dt_, 256 * h : 256 * (h + 1)],
                                    start=(dt_ == 0),
                                    stop=(dt_ == ND - 1),
                                )
                                if h == 1 and dt_ == ND - 1:
                                    mm.then_inc(sPE, 1)
                    else:
                        for dt_ in range(ND):
                            mm = te.matmul(
                                pse[:, 2 + ((seq - 1) % 4), :],
                                lhsT=wsb[:, dt_, 128 * kt : 128 * (kt + 1)],
                                rhs=xTr[:, dt_, 512 * ic : 512 * (ic + 1)],
                                start=(dt_ == 0),
                                stop=(dt_ == ND - 1),
                            )
                            if dt_ == ND - 1:
                                mm.then_inc(sPE, 1)
            # fused: logits chunks (mask matmul appended to diag groups) +
            # DoubleRow read groups
            for kind, jt in fused_order:
                c0 = _c0(jt)
                if kind == "L":
                    for ic in range(c0, NCH):
                        gg = g_of_chunk[(jt, ic)]
                        diag = ic == c0
                        need_ac = KQSEQ[("q", ND - 1, ic)]
                        need_dv = None
                        pg = prev_user_g(gg)
                        if pg is not None:
                            pj, pic = chunk_of_g[pg]
                            need_ac = max(need_ac, EXPREL[(pj, pic)])
                        elif gg < 2:
                            need_dv = VCP[NT - 2 + gg]
                        elif gg < 6:
                            need_ac = max(need_ac, 2 * KQ)
                        te.wait_ge(sAC, need_ac)
                        if need_dv is not None:
                            te.wait_ge(sDV, need_dv)
                        if diag:
                            ldwait(sLm, 32)
                        w = 512 * (ic + 1) - (128 * jt if diag else 512 * ic)
                        bank = bank_ap(gbank[(jt, ic)], w)
                        ilo = 128 * jt if diag else 512 * ic
                        for kt in range(ND):
                            mm = te.matmul(
                                bank,
                                lhsT=kT[:, kt, 128 * jt : 128 * (jt + 1)],
                                rhs=qT[:, kt, ilo : 512 * (ic + 1)],
                                start=(kt == 0),
                                stop=(kt == ND - 1) and not diag,
                            )
                            if kt == ND - 1 and not diag:
                                mm.then_inc(sPE, 1)
                        if diag:
                            # causal mask accumulated by the PE: identity
                            # stationary x (0/-60000) f16 pattern
                            te.matmul(
                                bank_ap(gbank[(jt, ic)], 128),
                                lhsT=ident16[:, :],
                                rhs=mask16[:, :],
                                start=False,
                                stop=True,
                                skip_group_check=True,
                            ).then_inc(sPE, 1)
                else:
                    npair = (jt + 2) // 2
                    if jt >= 1:
                        sem, thr = vp8_wait(None, jt - 1)
                        te.wait_ge(sPO if sem == "PO" else (sDV if sem == "DV" else sAC), thr)
                    if jt < 2:
                        te.wait_ge(sAC, EXPREL[(1, 2)])
                    elif jt >= NSING0 + 2:
                        if jt % 2 == 0:
                            te.wait_ge(sAC, AOC1[jt - 2])
                        else:
                            te.wait_ge(sDV, DOC1[jt - 2])
                    else:
                        pm = (jt - 2) // 2
                        if pm % 2 == 0:
                            te.wait_ge(sAC, AOCP[pm])
                        else:
                            te.wait_ge(sDV, DOCP[pm])
                    if jt == 0:
                        te.wait_ge(sPO, NMEMSET)
                    for m in range(npair):
                        if m == npair - 1:
                            sem, thr = vp8_wait(None, jt)
                            te.wait_ge(sDV if sem == "DV" else sAC, thr)
                        mm = te.matmul(
                            psRD[:, jt % 2, :],
                            lhsT=e8s[m][
                                :, :, 128 * jt - 256 * m : 128 * jt - 256 * m + 128
                            ],
                            rhs=vp8[:, m, :, :],
                            start=(m == 0),
                            stop=(m == npair - 1),
                            perf_mode=DR,
                        )
                        if m == npair - 1:
                            mm.then_inc(sPE, 1)

        @block.vector
        def _(ve):
            ndv = 0

            def inc(x):
                nonlocal ndv
                ndv += 1
                x.then_inc(sDV, 1)

            ve.wait_ge(sLc, 16)
            for jt in range(NT):
                ve.wait_ge(sPE, VG[jt])
                inc(
                    ve.tensor_tensor(
                        out=v_sb[:, jt, :], in0=pse[:, jt % 2, :],
                        in1=bvb[:, :], op=AOP.add,
                    )
                )
                assert ndv == VCP[jt]

            def den_chain(rj):
                # denominator chain of row rj (one block late)
                rp = rj % 4
                ve.wait_ge(sAC, EXP_END[rj])
                if NCH - _c0(rj) > 1:
                    inc(
                        ve.reduce_sum(
                            denom[:, rp : rp + 1],
                            dparts[:, rp, 0 : NOPS[rj]],
                            mybir.AxisListType.X,
                        )
                    )
                    assert ndv == DENOM[rj]
                    ve.wait_ge(sDV, DENOM[rj])  # same-engine RAW fence
                    src = denom[:, rp : rp + 1]
                else:
                    src = dparts[:, rp, 0:1]
                inc(ve.reciprocal(rec[:, rp : rp + 1], src))
                assert ndv == RECIP[rj]
                if rj % 2 == 0:
                    ve.wait_ge(sDV, RECIP[rj])  # same-engine RAW fence
                    inc(
                        ve.tensor_scalar(
                            out=vp8[:, rj // 2, (rj % 2), :],
                            in0=v_sb[:, rj, :],
                            scalar1=rec[:, rp : rp + 1],
                            scalar2=None,
                            op0=AOP.mult,
                        )
                    )
                    assert ndv == VP8D[rj]

            for jt in range(NT):
                c0 = _c0(jt)
                nch = NCH - c0
                w0 = 512 * (c0 + 1) - 128 * jt
                pj = jt % 4
                if jt >= 4:
                    # macc/negmax[jt%4] reuse: exps of jt-4 read them
                    ve.wait_ge(sAC, EXP_END[jt - 4])
                ve.wait_ge(sPE, LG[(jt, c0)])
                if nch > 1:
                    inc(
                        ve.reduce_max(
                            macc[:, pj, c0 : c0 + 1],
                            bank_ap(gbank[(jt, c0)], w0),
                            mybir.AxisListType.X,
                        )
                    )
                    assert ndv == DRED[jt]
                    for gi, run in enumerate(row_groups[jt]):
                        ve.wait_ge(sPE, LG[(jt, run[-1])])
                        inc(
                            ve.reduce_max(
                                macc[:, pj, run[0] : run[0] + len(run)],
                                bank_run_ap(gbank[(jt, run[0])], len(run)),
                                mybir.AxisListType.X,
                            )
                        )
                        assert ndv == REDG[(jt, gi)]
                    ve.wait_ge(sDV, REDG[(jt, len(row_groups[jt]) - 1)])
                    inc(
                        ve.reduce_max(
                            negmax[:, pj : pj + 1],
                            macc[:, pj, c0:NCH],
                            mybir.AxisListType.X, negate=True,
                        )
                    )
                else:
                    inc(
                        ve.reduce_max(
                            negmax[:, pj : pj + 1],
                            bank_ap(gbank[(jt, c0)], w0),
                            mybir.AxisListType.X, negate=True,
                        )
                    )
                assert ndv == NMX[jt]
                if jt >= 1:
                    den_chain(jt - 1)
                for pm in docp_due.get(jt, []):
                    ve.wait_ge(sPE, RG[2 * pm + 1])
                    if pm >= 2:
                        ve.wait_ge(sST, st_thr(pm - 2))
                    inc(
                        ve.tensor_scalar_add(
                            out=outst[:, pm % 2, :, :], in0=psRD[:, :, :],
                            scalar1=0.0,
                        )
                    )
                    assert ndv == DOCP[pm]
                for j in doc1_due.get(jt, []):
                    ve.wait_ge(sPE, RG[j])
                    if j >= 4:
                        ve.wait_ge(sST, oc1_st_wait(j))
                    inc(
                        ve.tensor_scalar_add(
                            out=outst[:, (j // 2) % 2, j % 2, :],
                            in0=psRD[:, j % 2, :],
                            scalar1=0.0,
                        )
                    )
                    assert ndv == DOC1[j]
            den_chain(NT - 1)
            for pm in DOCP_TRAIL:
                ve.wait_ge(sPE, RG[2 * pm + 1])
                if pm >= 2:
                    ve.wait_ge(sST, st_thr(pm - 2))
                inc(
                    ve.tensor_scalar_add(
                        out=outst[:, pm % 2, :, :], in0=psRD[:, :, :],
                        scalar1=0.0,
                    )
                )
                assert ndv == DOCP[pm]
            for j in DOC1_TRAIL:
                ve.wait_ge(sPE, RG[j])
                if j >= 4:
                    ve.wait_ge(sST, oc1_st_wait(j))
                inc(
                    ve.tensor_scalar_add(
                        out=outst[:, (j // 2) % 2, j % 2, :],
                        in0=psRD[:, j % 2, :],
                        scalar1=0.0,
                    )
                )
                assert ndv == DOC1[j]

        @block.scalar
        def _(ac_):
            ac_.wait_ge(sLc, 48)
            nac = 0
            for ic in range(NCH):
                for wsel, g_tab, bias in ((0, KG, bkc), (1, QG, bqc)):
                    dst = kT if wsel == 0 else qT
                    for kt in range(ND):
                        seq = KQSEQ[("k" if wsel == 0 else "q", kt, ic)]
                        ac_.wait_ge(sPE, g_tab[(kt, ic)])
                        ac_.activation(
                            out=dst[:, kt, 512 * ic : 512 * (ic + 1)],
                            in_=pse[:, 2 + ((seq - 1) % 4), :],
                            func=AFT.Identity,
                            bias=bias[:, kt : kt + 1],
                            scale=1.0,
                        ).then_inc(sAC, 1)
                        nac += 1
            assert nac == 2 * KQ

            def outcopy(pm):
                nonlocal nac
                ac_.wait_ge(sPE, RG[2 * pm + 1])
                if pm >= 2:
                    ac_.wait_ge(sST, st_thr(pm - 2))
                nac += 1
                ac_.activation(
                    out=outst[:, pm % 2, :, :], in_=psRD[:, :, :], func=AFT.Copy
                ).then_inc(sAC, 1)
                assert nac == AOCP[pm]

            def outcopy1(j):
                nonlocal nac
                ac_.wait_ge(sPE, RG[j])
                if j >= 4:
                    ac_.wait_ge(sST, oc1_st_wait(j))
                nac += 1
                ac_.activation(
                    out=outst[:, (j // 2) % 2, j % 2, :],
                    in_=psRD[:, j % 2, :],
                    func=AFT.Copy,
                ).then_inc(sAC, 1)
                assert nac == AOC1[j]

            def act_vp8(rj):
                nonlocal nac
                rp = rj % 4
                ac_.wait_ge(sDV, RECIP[rj])
                nac += 1
                ac_.activation(
                    out=vp8[:, rj // 2, rj % 2, :],
                    in_=v_sb[:, rj, :],
                    func=AFT.Copy,
                    bias=0.0,
                    scale=rec[:, rp : rp + 1],
                ).then_inc(sAC, 1)
                assert nac == VP8A[rj]

            for jt in range(NT):
                c0 = _c0(jt)
                pj = jt % 4
                m = jt // 2
                base = 256 * m
                for pm in aocp_due.get(jt, []):
                    outcopy(pm)
                # exps: diag first (frees the rotation bank soonest)
                ac_.wait_ge(sDV, NMX[jt])
                if jt >= 4:
                    # dparts[jt%4] reuse: recip(jt-4) must have read it
                    ac_.wait_ge(sDV, RECIP[jt - 4])
                w0 = 512 * (c0 + 1) - 128 * jt
                nac += 1
                ac_.activation(
                    out=e8s[m][:, jt % 2, 128 * jt - base : 512 * (c0 + 1) - base],
                    in_=bank_ap(gbank[(jt, c0)], w0),
                    func=AFT.Exp,
                    bias=negmax[:, pj : pj + 1],
                    scale=1.0,
                    accum_out=dparts[:, pj, 0:1],
                ).then_inc(sAC, 1)
                assert nac == EXPG[(jt, 0)]
                sidx = 1
                for run in row_groups[jt]:
                    a = 512 * run[0]
                    bcol = 512 * (run[-1] + 1)
                    nac += 1
                    ac_.activation(
                        out=e8s[m][:, jt % 2, a - base : bcol - base],
                        in_=bank_run_ap(gbank[(jt, run[0])], len(run)),
                        func=AFT.Exp,
                        bias=negmax[:, pj : pj + 1],
                        scale=1.0,
                        accum_out=dparts[:, pj, sidx : sidx + 1],
                    ).then_inc(sAC, 1)
                    assert nac == EXPG[(jt, sidx)]
                    sidx += 1
                assert sidx == NOPS[jt]
                if jt >= 1 and (jt - 1) % 2 == 1:
                    act_vp8(jt - 1)
                for j in aoc1_due.get(jt, []):
                    outcopy1(j)
            if (NT - 1) % 2 == 1:
                act_vp8(NT - 1)
            for pm in AOCP_TRAIL:
                outcopy(pm)
            for j in AOC1_TRAIL:
                outcopy1(j)

        @block.gpsimd
        def _(po):
            npo = 0
            for m in range(NP):
                po.memset(e8s[m][:, 1, 0:128], 0.0).then_inc(sPO, 1)
                npo += 1
                po.memset(vp8[:, m, 1, :], 0.0).then_inc(sPO, 1)
                npo += 1
            assert npo == NMEMSET

    nc.finalize()
    return nc


def _host_inputs(xb, wq16, wk16, wv8h, bq, bk, bv, T):
    # mask class 0 for the (narrowed) diagonal chunk: cols x < p get MASKVAL
    p = np.arange(128, dtype=np.float32)[:, None]
    xx = np.arange(128, dtype=np.float32)[None, :]
    msk = np.where(xx >= p, 0.0, MASKVAL).astype(np.float16)
    idn = np.eye(128, dtype=np.float16)

    xT = np.ascontiguousarray(xb.T)  # [D, T] f32
    x16 = xT.astype(np.float16)
    x8 = np.ascontiguousarray(
        xT.reshape(2, 2, 128, T).transpose(2, 0, 1, 3)
    ).astype(ml_dtypes.float8_e4m3fn)
    return dict(
        x16=x16,
        x8=np.ascontiguousarray(x8),
        wq16=wq16,
        wk16=wk16,
        wv8=wv8h,
        bq=bq,
        bk=bk,
        bv=bv,
        msk16=np.ascontiguousarray(msk),
        idn16=np.ascontiguousarray(idn),
    )


def kernel(x, Wk, bk, Wq, bq, Wv, bv):
    global LAST_RESULTS
    T = 2048
    x = np.ascontiguousarray(np.asarray(x, dtype=np.float32))
    Wk = np.asarray(Wk, dtype=np.float32)
    Wq = np.asarray(Wq, dtype=np.float32)
    Wv = np.asarray(Wv, dtype=np.float32)
    bk = np.ascontiguousarray(np.asarray(bk, dtype=np.float32))
    bq = np.ascontiguousarray(np.asarray(bq, dtype=np.float32))
    bv = np.ascontiguousarray(np.asarray(bv, dtype=np.float32))

    wq16 = np.ascontiguousarray(Wq.T).astype(np.float16)
    wk16 = np.ascontiguousarray(Wk.T).astype(np.float16)
    wvT = Wv.T * np.float32(INV_SQRT_K)  # [D, KS]
    wv8h = np.ascontiguousarray(
        wvT.reshape(2, 2, 128, KS).transpose(2, 0, 1, 3)
    ).astype(ml_dtypes.float8_e4m3fn)
    bvs = np.ascontiguousarray(bv * np.float32(INV_SQRT_K))

    nc = build_nc(T)
    in_maps = [_host_inputs(x[b], wq16, wk16, wv8h, bq, bk, bvs, T) for b in range(B)]
    res = None
    last_exc = None
    for attempt in range(3):
        try:
            res = run_bass_kernel_spmd(nc, in_maps, list(range(B)), trace=TRACE)
            break
        except Exception as e:  # transient NRT device errors; retry fresh
            last_exc = e
            import time as _time
            _time.sleep(10)
            nc = build_nc(T)
    if res is None:
        raise last_exc
    LAST_RESULTS = res
    read = np.stack(
        [np.asarray(res.results[b]["out"]).astype(np.float32) for b in range(B)],
        axis=0,
    )
    return (x + read).astype(np.float32)


# revision 8
# speedup vs baseline: 1.7126x; 1.0261x over previous
"""Trainium2 Bass kernel for nn_AttentionBlock (causal attn, softmax over the
QUERY axis (dim=1), post-softmax 1/sqrt(K) scale, residual add).

Sharding: data-parallel over batch B=8, one batch element per NeuronCore.

v4 design:
- K/Q projections + logits in fp16; V projection and the probability-weighted
  read in fp8e4 with MatmulPerfMode.DoubleRow (two 128-deep contraction tiles
  per matmul).
- The causal mask is applied BY THE PE: each diagonal logits group gets one
  extra 128-wide matmul (identity stationary x f16 mask pattern of 0/-60000)
  accumulated into the PSUM bank.  exp() of -60000-ish underflows to exactly
  0.  No DVE mask-add, no staging: every chunk is max-reduced and exp'd
  straight from its PSUM bank.
- Logits PSUM banks live in ONE 6-bank tensor (pse) plus a 2-bank read tensor
  (psRD).  The first 8 logits chunks use all 8 banks (prologue), then a
  6-bank rotation.  Adjacent-bank non-diag chunk runs are processed by SINGLE
  wide ops: one DVE reduce yields 2-3 column maxes, one ACT exp covers 2-3
  chunks (one f32 accum partial per op).
- Denominator combine + reciprocal on DVE; V'-scales split DVE (even jt) /
  ACT (odd jt); output evacuation is PAIRED: one op copies both read banks
  [128,2,512] -> bf16 outst, one DMA stores 256 output rows; pairs alternate
  ACT/DVE.  Pool does only the startup memsets.
- E8[j,i] strips live in PAIRED key-chunk layout e8s[m][:, slot, :]
  (slot = jt%2, strip base column 256m) feeding DoubleRow reads directly;
  slot-1's first 128 columns are memset 0 (sub-diagonal).
    read[i, :] = sum_j E[j, i] * V'[j, :],  V' = (v + bv) * rec_j / sqrt(K)

Raw Block style with manual semaphores: ONE embedded sync-wait per
instruction; cross-engine deps are standalone wait_ge with statically
computed thresholds; same-engine RAW pairs get explicit fences.
"""

import math
import os
import sys

import numpy as np
import ml_dtypes

for _p in ("/opt/trn_rl_repo", "/root/.axon_site/_ro/trn_rl_repo"):
    if os.path.isdir(_p) and _p not in sys.path:
        sys.path.append(_p)

import concourse.bass as bass
from concourse import mybir
from concourse.bass_utils import run_bass_kernel_spmd

B = 8
D = 512
KS = 512
ND = D // 128  # 4 contraction tiles

F32 = mybir.dt.float32
F16 = mybir.dt.float16
BF16 = mybir.dt.bfloat16
F8 = mybir.dt.float8e4
AOP = mybir.AluOpType
AFT = mybir.ActivationFunctionType
DR = mybir.MatmulPerfMode.DoubleRow

INV_SQRT_K = 1.0 / math.sqrt(KS)
MASKVAL = -60000.0  # fits f16; exp(-60000 + max) == 0 exactly

TRACE = False
LAST_RESULTS = None


def _c0(jt):
    return (128 * jt) // 512


def build_nc(T=2048, debug_dump=False):
    NT = T // 128   # 16 row chunks
    NCH = T // 512  # 4 column chunks
    NP = NT // 2    # 8 key-chunk pairs
    KQ = ND * NCH   # 16 projection output groups for each of q/k

    nc = bass.Bass("TRN2", target_bir_lowering=False, debug=False, num_devices=B)

    # ---- DRAM ----
    x16_d = nc.dram_tensor("x16", [D, T], F16, kind="ExternalInput")
    x8_d = nc.dram_tensor("x8", [128, 2, 2, T], F8, kind="ExternalInput")
    wq_d = nc.dram_tensor("wq16", [D, KS], F16, kind="ExternalInput")
    wk_d = nc.dram_tensor("wk16", [D, KS], F16, kind="ExternalInput")
    wv_d = nc.dram_tensor("wv8", [128, 2, 2, KS], F8, kind="ExternalInput")
    bq_d = nc.dram_tensor("bq", [KS], F32, kind="ExternalInput")
    bk_d = nc.dram_tensor("bk", [KS], F32, kind="ExternalInput")
    bv_d = nc.dram_tensor("bv", [KS], F32, kind="ExternalInput")
    msk_d = nc.dram_tensor("msk16", [128, 512], F16, kind="ExternalInput")
    idn_d = nc.dram_tensor("idn16", [128, 128], F16, kind="ExternalInput")
    out_d = nc.dram_tensor("out", [T, KS], BF16, kind="ExternalOutput")

    # ---- SBUF ----
    xTr = nc.alloc_sbuf_tensor("xTr", [128, ND, T], F16)
    x8s = nc.alloc_sbuf_tensor("x8s", [128, 2, 2, T], F8)
    wkr = nc.alloc_sbuf_tensor("wkr", [128, ND, KS], F16)
    wqr = nc.alloc_sbuf_tensor("wqr", [128, ND, KS], F16)
    wv8 = nc.alloc_sbuf_tensor("wv8s", [128, 2, 2, KS], F8)
    kT = nc.alloc_sbuf_tensor("kT", [128, ND, T], F16)
    qT = nc.alloc_sbuf_tensor("qT", [128, ND, T], F16)
    v_sb = nc.alloc_sbuf_tensor("v_sb", [128, NT, KS], BF16)
    vp8 = nc.alloc_sbuf_tensor("vp8", [128, NP, 2, KS], F8)
    e8s = [
        nc.alloc_sbuf_tensor(f"e8_{m}", [128, 2, T - (1024 if m == 5 else 256 * m)], F8)
        for m in range(NP)
    ]
    outst = nc.alloc_sbuf_tensor("outst", [128, 2, 2, KS], BF16)
    mask16 = nc.alloc_sbuf_tensor("mask16", [128, 512], F16)
    ident16 = nc.alloc_sbuf_tensor("ident16", [128, 128], F16)
    bqc = nc.alloc_sbuf_tensor("bqc", [128, ND], F32)
    bkc = nc.alloc_sbuf_tensor("bkc", [128, ND], F32)
    bvb = nc.alloc_sbuf_tensor("bvb", [128, KS], F32)
    macc = nc.alloc_sbuf_tensor("macc", [128, 4, NCH], F32)
    negmax = nc.alloc_sbuf_tensor("negmax", [128, 4], F32)
    dparts = nc.alloc_sbuf_tensor("dparts", [128, 4, NCH], F32)
    denom = nc.alloc_sbuf_tensor("denom", [128, 4], F32)
    rec = nc.alloc_sbuf_tensor("rec", [128, 4], F32)

    # ---- PSUM: 6-bank logits rotation + 2 read banks ----
    pse = nc.alloc_psum_tensor("pse", [128, 6, 512], F32)
    psRD = nc.alloc_psum_tensor("psRD", [128, 2, 512], F32)
    # phase 1: v -> pse[0..1], k/q -> pse[2..5]

    # ================= static op-index tables =================
    LA = 3  # read lookahead: R(j) sits at fused position j + LA_OF(j)
    LATE_LA = 3

    def LA_OF(j):
        return LA if j < 8 else LATE_LA

    def bank_of_g(g):
        if g < 6:
            return ("E", g)
        if g < 8:
            return ("R", g - 6)
        return ("E", (g - 8) % 6)

    def prev_user_g(g):
        if g < 8:
            return None
        return g - 8 if g < 14 else g - 6

    gbank = {}
    g_of_chunk = {}
    chunk_of_g = {}
    g = 0
    for jt in range(NT):
        for ic in range(_c0(jt), NCH):
            gbank[(jt, ic)] = bank_of_g(g)
            g_of_chunk[(jt, ic)] = g
            chunk_of_g[g] = (jt, ic)
            g += 1

    # rows whose diagonal chunk is computed FULL-width so diag+nondiag form
    # one adjacent-bank pair handled by single wide reduce/exp ops
    FULL = {8, 9, 10, 11}
    # e8 strip base columns (pair 5 starts at row 10's full-width diag)
    BASE = [256 * m for m in range(NP)]
    BASE[5] = 1024

    # non-diag chunk groups per row: maximal runs of adjacent banks
    row_groups = {}
    for jt in range(NT):
        c0 = _c0(jt)
        ics = list(range(c0 + 1, NCH))
        groups = []
        i = 0
        while i < len(ics):
            run = [ics[i]]
            while (
                i + 1 < len(ics)
                and len(run) < 3
                and gbank[(jt, ics[i + 1])][0] == gbank[(jt, run[0])][0]
                and gbank[(jt, ics[i + 1])][1]
                == gbank[(jt, run[0])][1] + len(run)
            ):
                run.append(ics[i + 1])
                i += 1
            groups.append(run)
            i += 1
        row_groups[jt] = groups

    # ---- PE plan (sPE counts GROUPS) ----
    VG, KG, QG, KQSEQ, LG, RG = {}, {}, {}, {}, {}, {}
    pe = 0
    kqseq = 0
    p1_order = []
    for b in range(NCH):
        for r in range(4):
            pe += 1
            KG[(r, b)] = pe
            kqseq += 1
            KQSEQ[("k", r, b)] = kqseq
            p1_order.append(("k", r, b))
            jt = 4 * b + r
            if jt < NT:
                pe += 1
                VG[jt] = pe
                p1_order.append(("v", jt))
        for kt in range(ND):
            pe += 1
            QG[(kt, b)] = pe
            kqseq += 1
            KQSEQ[("q", kt, b)] = kqseq
            p1_order.append(("q", kt, b))
    assert pe == NT + 2 * KQ

    fused_order = []
    for m in range(NT):
        fused_order.append(("L", m))
        for j in range(NT):
            if j + LA_OF(j) == m:
                fused_order.append(("R", j))
    for j in range(NT):
        if j + LA_OF(j) >= NT:
            fused_order.append(("R", j))

    for kind, jt in fused_order:
        if kind == "L":
            for ic in range(_c0(jt), NCH):
                pe += 1
                LG[(jt, ic)] = pe
        else:
            pe += 1
            RG[jt] = pe

    # paired outcopies: pair pm covers read rows (2pm, 2pm+1);
    # even pm -> ACT, odd pm -> DVE; due at fused position 2pm+1+LA
    aocp_due, docp_due = {}, {}
    AOCP_TRAIL, DOCP_TRAIL = [], []
    NPAIRED = 0  # pairs 0..5 cover rows 0..11; rows 12..15 get single OCs
    for pm in range(NPAIRED):
        due = 2 * pm + 1 + LA_OF(2 * pm + 1)
        tgt = aocp_due if pm % 2 == 0 else docp_due
        trail = AOCP_TRAIL if pm % 2 == 0 else DOCP_TRAIL
        if due < NT:
            tgt.setdefault(due, []).append(pm)
        else:
            trail.append(pm)
    # single outcopies for rows 12..15: ACT even rows, DVE odd rows
    aoc1_due, doc1_due = {}, {}
    AOC1_TRAIL, DOC1_TRAIL = [], []
    NSING0 = 2 * NPAIRED  # first single-OC row
    for j in range(NSING0, NT):
        due = j + LA_OF(j)
        tgt = aoc1_due if j % 2 == 0 else doc1_due
        trail = AOC1_TRAIL if j % 2 == 0 else DOC1_TRAIL
        if due < NT:
            tgt.setdefault(due, []).append(j)
        else:
            trail.append(j)

    # ---- ACT plan (sAC): 32 kq copies, then per jt: due paired outcopies,
    #      exps (diag first, then non-diag groups), odd-jt V'-scale ----
    EXPG = {}      # (jt, gi) -> act idx; gi 0 = diag, 1.. = groups
    EXPREL = {}    # (jt, ic) -> act idx of the exp covering the chunk
    EXP_END = {}
    NOPS = {}
    AOCP, VP8A, AOC1 = {}, {}, {}
    ac = 2 * KQ
    for jt in range(NT):
        c0 = _c0(jt)
        for pm in aocp_due.get(jt, []):
            ac += 1
            AOCP[pm] = ac
        if jt in FULL:
            ac += 1
            EXPG[(jt, 0)] = ac  # merged diag+nondiag
            for ic in range(c0, NCH):
                EXPREL[(jt, ic)] = ac
            EXP_END[jt] = ac
            NOPS[jt] = 1
        else:
            ac += 1
            EXPG[(jt, 0)] = ac  # diag
            EXPREL[(jt, c0)] = ac
            gi = 1
            for run in row_groups[jt]:
                ac += 1
                EXPG[(jt, gi)] = ac
                for ic in run:
                    EXPREL[(jt, ic)] = ac
                gi += 1
            EXP_END[jt] = ac
            NOPS[jt] = gi
        if jt >= 1 and (jt - 1) % 2 == 1:
            ac += 1
            VP8A[jt - 1] = ac
        for j in aoc1_due.get(jt, []):
            ac += 1
            AOC1[j] = ac
    if (NT - 1) % 2 == 1:
        ac += 1
        VP8A[NT - 1] = ac
    for pm in AOCP_TRAIL:
        ac += 1
        AOCP[pm] = ac
    for j in AOC1_TRAIL:
        ac += 1
        AOC1[j] = ac

    # ---- DVE plan (sDV): 16 v-copies, then per jt: DRED (diag max from
    #      bank; folded into NMX when nch==1), RED groups, NMX, [DENOM],
    #      RECIP, even-jt V'-scale, due paired outcopies ----
    VCP, DRED, REDG, NMX, DENOM, RECIP, VP8D, DOCP, DOC1 = {}, {}, {}, {}, {}, {}, {}, {}, {}
    dv = 0
    for jt in range(NT):
        dv += 1
        VCP[jt] = dv

    def _dve_tail(jt):
        # denominator chain of row jt, emitted one block later
        nonlocal_dv = []
        return nonlocal_dv

    for jt in range(NT + 1):
        if jt < NT:
            nch = NCH - _c0(jt)
            if jt in FULL:
                dv += 1
                DRED[jt] = dv  # merged 2-bank reduce
            elif nch > 1:
                dv += 1
                DRED[jt] = dv
                for ric in range(_c0(jt) + 1, NCH):
                    dv += 1
                    REDG[(jt, ric)] = dv
            dv += 1
            NMX[jt] = dv
        pj = jt - 1  # previous row's denominator chain
        if 0 <= pj < NT:
            if NOPS[pj] > 1:
                dv += 1
                DENOM[pj] = dv
            dv += 1
            RECIP[pj] = dv
            if pj % 2 == 0:
                dv += 1
                VP8D[pj] = dv
        if jt < NT:
            for pm in docp_due.get(jt, []):
                dv += 1
                DOCP[pm] = dv
            for j in doc1_due.get(jt, []):
                dv += 1
                DOC1[j] = dv
    for pm in DOCP_TRAIL:
        dv += 1
        DOCP[pm] = dv
    for j in DOC1_TRAIL:
        dv += 1
        DOC1[j] = dv

    # ---- Pool plan (sPO): slot-1 memsets only ----
    NMEMSET = 2 * NP

    def st_thr(pm):
        # paired store pm is the (pm+1)-th store
        return 16 * (pm + 1)

    def st1_thr(j):
        # single store for row j is the (NPAIRED + j - NSING0 + 1)-th store
        return 16 * (NPAIRED + j - NSING0 + 1)

    def oc1_st_wait(j):
        # outst slot (s2=(j//2)%2, s1=j%2) previously used by single j-4 or
        # by pair (j-4)//2
        if j - 4 >= NSING0:
            return st1_thr(j - 4)
        return st_thr((j - 4) // 2)

    def bank_ap(coord, w=512):
        t, slot = coord
        if t == "E":
            return pse[:, slot, 0:w]
        return psRD[:, slot, 0:w]

    def bank_run_ap(coord, ln):
        t, slot = coord
        if t == "E":
            return pse[:, slot : slot + ln, :]
        return psRD[:, slot : slot + ln, :]

    def vp8_wait(te_or_none, jt):
        # (sem, thr) releasing vp8(jt)
        if jt % 2 == 0:
            return ("DV", VP8D[jt])
        return ("AC", VP8A[jt])

    with (
        nc.semaphore("sLv") as sLv,
        nc.semaphore("sLk") as sLk,
        nc.semaphore("sLk2") as sLk2,
        nc.semaphore("sLq") as sLq,
        nc.semaphore("sLc") as sLc,
        nc.semaphore("sLm") as sLm,
        nc.semaphore("sLx0") as sLx0,
        nc.semaphore("sLxb") as sLxb,
        nc.semaphore("sLx1") as sLx1,
        nc.semaphore("sLx2") as sLx2,
        nc.semaphore("sLx3") as sLx3,
        nc.semaphore("sL80") as sL80,
        nc.semaphore("sL81") as sL81,
        nc.semaphore("sL82") as sL82,
        nc.semaphore("sL83") as sL83,
        nc.semaphore("sPE") as sPE,
        nc.semaphore("sDV") as sDV,
        nc.semaphore("sAC") as sAC,
        nc.semaphore("sPO") as sPO,
        nc.semaphore("sST") as sST,
        nc.Block() as block,
    ):
        sLxs = [sLx0, sLx1, sLx2, sLx3]
        sL8s = [sL80, sL81, sL82, sL83]

        @block.sync
        def _(sp):
            def ldx16(ic):
                sp.dma_start(
                    out=xTr[:, :, 512 * ic : 512 * (ic + 1)],
                    in_=x16_d.ap()[:, 512 * ic : 512 * (ic + 1)].rearrange(
                        "(t p) i -> p t i", p=128
                    ),
                ).then_inc(sLxs[ic], 16)

            def ldx8(ic):
                sp.dma_start(
                    out=x8s[:, :, :, 512 * ic : 512 * (ic + 1)],
                    in_=x8_d.ap()[:, :, :, 512 * ic : 512 * (ic + 1)],
                ).then_inc(sL8s[ic], 16)

            wk_re = wk_d.ap().rearrange("(t p) k -> p t k", p=128)
            sp.dma_start(out=wkr[:, :, 0:256], in_=wk_re[:, :, 0:256]).then_inc(
                sLk, 16
            )
            x16_re0 = x16_d.ap()[:, 0:512].rearrange("(t p) i -> p t i", p=128)
            sp.dma_start(out=xTr[:, :, 0:256], in_=x16_re0[:, :, 0:256]).then_inc(
                sLx0, 16
            )
            sp.dma_start(out=xTr[:, :, 256:512], in_=x16_re0[:, :, 256:512]).then_inc(
                sLxb, 16
            )
            sp.dma_start(out=wv8[:, :, :, :], in_=wv_d.ap()).then_inc(sLv, 16)
            ldx8(0)
            bv_ap = bv_d.ap()
            bv_bcast = bass.AP(
                tensor=bv_ap.tensor, offset=bv_ap.offset, ap=[[0, 128]] + list(bv_ap.ap)
            )
            sp.dma_start(out=bvb[:, :], in_=bv_bcast).then_inc(sLc, 16)
            with nc.allow_non_contiguous_dma(reason="16B/partition bias loads"):
                sp.dma_start(
                    out=bkc[:, :], in_=bk_d.ap().rearrange("(t p) -> p t", p=128)
                ).then_inc(sLc, 16)
                sp.dma_start(
                    out=bqc[:, :], in_=bq_d.ap().rearrange("(t p) -> p t", p=128)
                ).then_inc(sLc, 16)
            sp.dma_start(out=wkr[:, :, 256:512], in_=wk_re[:, :, 256:512]).then_inc(
                sLk2, 16
            )
            sp.dma_start(
                out=wqr[:, :, :],
                in_=wq_d.ap().rearrange("(t p) k -> p t k", p=128),
            ).then_inc(sLq, 16)
            sp.dma_start(out=mask16[:, :], in_=msk_d.ap()).then_inc(sLm, 16)
            sp.dma_start(out=ident16[:, :], in_=idn_d.ap()).then_inc(sLm, 16)
            ldx16(1)
            ldx8(1)
            ldx16(2)
            ldx8(2)
            ldx16(3)
            ldx8(3)
            # paired stores: 256 output rows each
            out_ap = out_d.ap()
            for pm in range(NPAIRED):
                if pm % 2 == 0:
                    sp.wait_ge(sAC, AOCP[pm])
                else:
                    sp.wait_ge(sDV, DOCP[pm])
                sp.dma_start(
                    out=out_ap[256 * pm : 256 * (pm + 1), :].rearrange(
                        "(s p) c -> p s c", p=128
                    ),
                    in_=outst[:, pm % 2, :, :],
                ).then_inc(sST, 16)
            for j in range(NSING0, NT):
                if j % 2 == 0:
                    sp.wait_ge(sAC, AOC1[j])
                else:
                    sp.wait_ge(sDV, DOC1[j])
                sp.dma_start(
                    out=out_ap[128 * j : 128 * (j + 1), :],
                    in_=outst[:, (j // 2) % 2, j % 2, :],
                ).then_inc(sST, 16)
            sp.wait_ge(sST, 16 * (NPAIRED + NT - NSING0))

        @block.tensor
        def _(te):
            waited = set()

            def ldwait(sem, thr=16):
                if sem not in waited:
                    te.wait_ge(sem, thr)
                    waited.add(sem)

            for item in p1_order:
                if item[0] == "v":
                    jt = item[1]
                    ldwait(sLv)
                    ldwait(sL8s[jt // 4])
                    if jt >= 2:
                        te.wait_ge(sDV, VCP[jt - 2])
                    for dm in range(2):
                        mm = te.matmul(
                            pse[:, jt % 2, :],
                            lhsT=x8s[:, dm, :, 128 * jt : 128 * (jt + 1)],
                            rhs=wv8[:, dm, :, :],
                            start=(dm == 0),
                            stop=(dm == 1),
                            perf_mode=DR,
                        )
                        if dm == 1:
                            mm.then_inc(sPE, 1)
                else:
                    kind, kt, ic = item
                    wsb = wkr if kind == "k" else wqr
                    if kind == "k":
                        ldwait(sLk if kt < 2 else sLk2)
                    else:
                        ldwait(sLq)
                    ldwait(sLxs[ic])
                    if ic == 0 and not (kind == "k" and kt == 0):
                        ldwait(sLxb)
                    seq = KQSEQ[(kind, kt, ic)]
                    if seq > 4:
                        te.wait_ge(sAC, seq - 4)
                    if kind == "k" and ic == 0:
                        # two half-width groups so the first k's start on the
                        # first half-load of x16 strip 0
                        for h in range(2):
                            if h == 1:
                                ldwait(sLxb)
                            for dt_ in range(ND):
                                mm = te.matmul(
                                    pse[:, 2 + ((seq - 1) % 4), 256 * h : 256 * (h + 1)],
                                    lhsT=wsb[:, dt_, 128 * kt : 128 * (kt + 1)],
                                    rhs=xTr[:, dt_, 256 * h : 256 * (h + 1)],
                                    start=(dt_ == 0),
                                    stop=(dt_ == ND - 1),
                                )
                                if h == 1 and dt_ == ND - 1:
                                    mm.then_inc(sPE, 1)
                    else:
                        for dt_ in range(ND):
                            mm = te.matmul(
                                pse[:, 2 + ((seq - 1) % 4), :],
                                lhsT=wsb[:, dt_, 128 * kt : 128 * (kt + 1)],
                                rhs=xTr[:, dt_, 512 * ic : 512 * (ic + 1)],
                                start=(dt_ == 0),
                                stop=(dt_ == ND - 1),
                            )
                            if dt_ == ND - 1:
                                mm.then_inc(sPE, 1)
            # fused: logits chunks (mask matmul appended to diag groups) +
            # DoubleRow read groups
            for kind, jt in fused_order:
                c0 = _c0(jt)
                if kind == "L":
                    for ic in range(c0, NCH):
                        gg = g_of_chunk[(jt, ic)]
                        diag = ic == c0
                        need_ac = KQSEQ[("q", ND - 1, ic)]
                        need_dv = None
                        pg = prev_user_g(gg)
                        if pg is not None:
                            pj, pic = chunk_of_g[pg]
                            need_ac = max(need_ac, EXPREL[(pj, pic)])
                        elif gg < 2:
                            need_dv = VCP[NT - 2 + gg]
                        elif gg < 6:
                            need_ac = max(need_ac, 2 * KQ)
                        te.wait_ge(sAC, need_ac)
                        if need_dv is not None:
                            te.wait_ge(sDV, need_dv)
                        if diag:
                            ldwait(sLm, 32)
                        dlo = 512 * c0 if jt in FULL else 128 * jt
                        w = 512 * (ic + 1) - (dlo if diag else 512 * ic)
                        bank = bank_ap(gbank[(jt, ic)], w)
                        ilo = dlo if diag else 512 * ic
                        for kt in range(ND):
                            mm = te.matmul(
                                bank,
                                lhsT=kT[:, kt, 128 * jt : 128 * (jt + 1)],
                                rhs=qT[:, kt, ilo : 512 * (ic + 1)],
                                start=(kt == 0),
                                stop=(kt == ND - 1) and not diag,
                            )
                            if kt == ND - 1 and not diag:
                                mm.then_inc(sPE, 1)
                        if diag:
                            # causal mask accumulated by the PE: identity
                            # stationary x (0/-60000) f16 pattern, class
                            # slice of the class-3 mask
                            cls = (jt % 4) if jt in FULL else 0
                            te.matmul(
                                bank_ap(gbank[(jt, ic)], 128 * (cls + 1)),
                                lhsT=ident16[:, :],
                                rhs=mask16[:, 512 - 128 * (cls + 1) : 512],
                                start=False,
                                stop=True,
                                skip_group_check=True,
                            ).then_inc(sPE, 1)
                else:
                    npair = (jt + 2) // 2
                    if jt >= 1:
                        sem, thr = vp8_wait(None, jt - 1)
                        te.wait_ge(sPO if sem == "PO" else (sDV if sem == "DV" else sAC), thr)
                    if jt < 2:
                        te.wait_ge(sAC, EXPREL[(1, 2)])
                    elif jt >= NSING0 + 2:
                        if jt % 2 == 0:
                            te.wait_ge(sAC, AOC1[jt - 2])
                        else:
                            te.wait_ge(sDV, DOC1[jt - 2])
                    else:
                        pm = (jt - 2) // 2
                        if pm % 2 == 0:
                            te.wait_ge(sAC, AOCP[pm])
                        else:
                            te.wait_ge(sDV, DOCP[pm])
                    if jt == 0:
                        te.wait_ge(sPO, NMEMSET)
                    for m in range(npair):
                        if m == npair - 1:
                            sem, thr = vp8_wait(None, jt)
                            te.wait_ge(sDV if sem == "DV" else sAC, thr)
                        mm = te.matmul(
                            psRD[:, jt % 2, :],
                            lhsT=e8s[m][
                                :, :, 128 * jt - BASE[m] : 128 * jt - BASE[m] + 128
                            ],
                            rhs=vp8[:, m, :, :],
                            start=(m == 0),
                            stop=(m == npair - 1),
                            perf_mode=DR,
                        )
                        if m == npair - 1:
                            mm.then_inc(sPE, 1)

        @block.vector
        def _(ve):
            ndv = 0

            def inc(x):
                nonlocal ndv
                ndv += 1
                x.then_inc(sDV, 1)

            ve.wait_ge(sLc, 16)
            for jt in range(NT):
                ve.wait_ge(sPE, VG[jt])
                inc(
                    ve.tensor_tensor(
                        out=v_sb[:, jt, :], in0=pse[:, jt % 2, :],
                        in1=bvb[:, :], op=AOP.add,
                    )
                )
                assert ndv == VCP[jt]

            def den_chain(rj):
                # denominator chain of row rj (one block late)
                rp = rj % 4
                ve.wait_ge(sAC, EXP_END[rj])
                if NOPS[rj] > 1:
                    inc(
                        ve.reduce_sum(
                            denom[:, rp : rp + 1],
                            dparts[:, rp, 0 : NOPS[rj]],
                            mybir.AxisListType.X,
                        )
                    )
                    assert ndv == DENOM[rj]
                    ve.wait_ge(sDV, DENOM[rj])  # same-engine RAW fence
                    src = denom[:, rp : rp + 1]
                else:
                    src = dparts[:, rp, 0:1]
                inc(ve.reciprocal(rec[:, rp : rp + 1], src))
                assert ndv == RECIP[rj]
                if rj % 2 == 0:
                    ve.wait_ge(sDV, RECIP[rj])  # same-engine RAW fence
                    inc(
                        ve.tensor_scalar(
                            out=vp8[:, rj // 2, (rj % 2), :],
                            in0=v_sb[:, rj, :],
                            scalar1=rec[:, rp : rp + 1],
                            scalar2=None,
                            op0=AOP.mult,
                        )
                    )
                    assert ndv == VP8D[rj]

            for jt in range(NT):
                c0 = _c0(jt)
                nch = NCH - c0
                w0 = 512 * (c0 + 1) - 128 * jt
                pj = jt % 4
                if jt >= 4:
                    # macc/negmax[jt%4] reuse: exps of jt-4 read them
                    ve.wait_ge(sAC, EXP_END[jt - 4])
                ve.wait_ge(sPE, LG[(jt, c0)])
                if jt in FULL:
                    ve.wait_ge(sPE, LG[(jt, c0 + 1)])
                    inc(
                        ve.reduce_max(
                            macc[:, pj, c0 : c0 + 2],
                            bank_run_ap(gbank[(jt, c0)], 2),
                            mybir.AxisListType.X,
                        )
                    )
                    assert ndv == DRED[jt]
                    ve.wait_ge(sDV, DRED[jt])
                    inc(
                        ve.reduce_max(
                            negmax[:, pj : pj + 1],
                            macc[:, pj, c0:NCH],
                            mybir.AxisListType.X, negate=True,
                        )
                    )
                elif nch > 1:
                    inc(
                        ve.reduce_max(
                            macc[:, pj, c0 : c0 + 1],
                            bank_ap(gbank[(jt, c0)], w0),
                            mybir.AxisListType.X,
                        )
                    )
                    assert ndv == DRED[jt]
                    for ric in range(c0 + 1, NCH):
                        ve.wait_ge(sPE, LG[(jt, ric)])
                        inc(
                            ve.reduce_max(
                                macc[:, pj, ric : ric + 1],
                                bank_ap(gbank[(jt, ric)], 512),
                                mybir.AxisListType.X,
                            )
                        )
                        assert ndv == REDG[(jt, ric)]
                    ve.wait_ge(sDV, REDG[(jt, NCH - 1)])
                    inc(
                        ve.reduce_max(
                            negmax[:, pj : pj + 1],
                            macc[:, pj, c0:NCH],
                            mybir.AxisListType.X, negate=True,
                        )
                    )
                else:
                    inc(
                        ve.reduce_max(
                            negmax[:, pj : pj + 1],
                            bank_ap(gbank[(jt, c0)], w0),
                            mybir.AxisListType.X, negate=True,
                        )
                    )
                assert ndv == NMX[jt]
                if jt >= 1:
                    den_chain(jt - 1)
                for pm in docp_due.get(jt, []):
                    ve.wait_ge(sPE, RG[2 * pm + 1])
                    if pm >= 2:
                        ve.wait_ge(sST, st_thr(pm - 2))
                    inc(
                        ve.tensor_scalar_add(
                            out=outst[:, pm % 2, :, :], in0=psRD[:, :, :],
                            scalar1=0.0,
                        )
                    )
                    assert ndv == DOCP[pm]
                for j in doc1_due.get(jt, []):
                    ve.wait_ge(sPE, RG[j])
                    if j >= 4:
                        ve.wait_ge(sST, oc1_st_wait(j))
                    inc(
                        ve.tensor_scalar_add(
                            out=outst[:, (j // 2) % 2, j % 2, :],
                            in0=psRD[:, j % 2, :],
                            scalar1=0.0,
                        )
                    )
                    assert ndv == DOC1[j]
            den_chain(NT - 1)
            for pm in DOCP_TRAIL:
                ve.wait_ge(sPE, RG[2 * pm + 1])
                if pm >= 2:
                    ve.wait_ge(sST, st_thr(pm - 2))
                inc(
                    ve.tensor_scalar_add(
                        out=outst[:, pm % 2, :, :], in0=psRD[:, :, :],
                        scalar1=0.0,
                    )
                )
                assert ndv == DOCP[pm]
            for j in DOC1_TRAIL:
                ve.wait_ge(sPE, RG[j])
                if j >= 4:
                    ve.wait_ge(sST, oc1_st_wait(j))
                inc(
                    ve.tensor_scalar_add(
                        out=outst[:, (j // 2) % 2, j % 2, :],
                        in0=psRD[:, j % 2, :],
                        scalar1=0.0,
                    )
                )
                assert ndv == DOC1[j]

        @block.scalar
        def _(ac_):
            ac_.wait_ge(sLc, 48)
            nac = 0
            for ic in range(NCH):
                for wsel, g_tab, bias in ((0, KG, bkc), (1, QG, bqc)):
                    dst = kT if wsel == 0 else qT
                    for kt in range(ND):
                        seq = KQSEQ[("k" if wsel == 0 else "q", kt, ic)]
                        ac_.wait_ge(sPE, g_tab[(kt, ic)])
                        ac_.activation(
                            out=dst[:, kt, 512 * ic : 512 * (ic + 1)],
                            in_=pse[:, 2 + ((seq - 1) % 4), :],
                            func=AFT.Identity,
                            bias=bias[:, kt : kt + 1],
                            scale=1.0,
                        ).then_inc(sAC, 1)
                        nac += 1
            assert nac == 2 * KQ

            def outcopy(pm):
                nonlocal nac
                ac_.wait_ge(sPE, RG[2 * pm + 1])
                if pm >= 2:
                    ac_.wait_ge(sST, st_thr(pm - 2))
                nac += 1
                ac_.activation(
                    out=outst[:, pm % 2, :, :], in_=psRD[:, :, :], func=AFT.Copy
                ).then_inc(sAC, 1)
                assert nac == AOCP[pm]

            def outcopy1(j):
                nonlocal nac
                ac_.wait_ge(sPE, RG[j])
                if j >= 4:
                    ac_.wait_ge(sST, oc1_st_wait(j))
                nac += 1
                ac_.activation(
                    out=outst[:, (j // 2) % 2, j % 2, :],
                    in_=psRD[:, j % 2, :],
                    func=AFT.Copy,
                ).then_inc(sAC, 1)
                assert nac == AOC1[j]

            def act_vp8(rj):
                nonlocal nac
                rp = rj % 4
                ac_.wait_ge(sDV, RECIP[rj])
                nac += 1
                ac_.activation(
                    out=vp8[:, rj // 2, rj % 2, :],
                    in_=v_sb[:, rj, :],
                    func=AFT.Copy,
                    bias=0.0,
                    scale=rec[:, rp : rp + 1],
                ).then_inc(sAC, 1)
                assert nac == VP8A[rj]

            for jt in range(NT):
                c0 = _c0(jt)
                pj = jt % 4
                m = jt // 2
                base = BASE[m]
                for pm in aocp_due.get(jt, []):
                    outcopy(pm)
                ac_.wait_ge(sDV, NMX[jt])
                if jt >= 4:
                    # dparts[jt%4] reuse: recip(jt-4) must have read it
                    ac_.wait_ge(sDV, RECIP[jt - 4])
                if jt in FULL:
                    # one exp over the adjacent diag+nondiag bank pair
                    nac += 1
                    ac_.activation(
                        out=e8s[m][
                            :, jt % 2, 512 * c0 - base : 512 * (c0 + 2) - base
                        ],
                        in_=bank_run_ap(gbank[(jt, c0)], 2),
                        func=AFT.Exp,
                        bias=negmax[:, pj : pj + 1],
                        scale=1.0,
                        accum_out=dparts[:, pj, 0:1],
                    ).then_inc(sAC, 1)
                    assert nac == EXPG[(jt, 0)]
                else:
                    # exps: diag first (frees the rotation bank soonest)
                    w0 = 512 * (c0 + 1) - 128 * jt
                    nac += 1
                    ac_.activation(
                        out=e8s[m][
                            :, jt % 2, 128 * jt - base : 512 * (c0 + 1) - base
                        ],
                        in_=bank_ap(gbank[(jt, c0)], w0),
                        func=AFT.Exp,
                        bias=negmax[:, pj : pj + 1],
                        scale=1.0,
                        accum_out=dparts[:, pj, 0:1],
                    ).then_inc(sAC, 1)
                    assert nac == EXPG[(jt, 0)]
                    sidx = 1
                    for run in row_groups[jt]:
                        a = 512 * run[0]
                        bcol = 512 * (run[-1] + 1)
                        nac += 1
                        ac_.activation(
                            out=e8s[m][:, jt % 2, a - base : bcol - base],
                            in_=bank_run_ap(gbank[(jt, run[0])], len(run)),
                            func=AFT.Exp,
                            bias=negmax[:, pj : pj + 1],
                            scale=1.0,
                            accum_out=dparts[:, pj, sidx : sidx + 1],
                        ).then_inc(sAC, 1)
                        assert nac == EXPG[(jt, sidx)]
                        sidx += 1
                    assert sidx == NOPS[jt]
                if jt >= 1 and (jt - 1) % 2 == 1:
                    act_vp8(jt - 1)
                for j in aoc1_due.get(jt, []):
                    outcopy1(j)
            if (NT - 1) % 2 == 1:
                act_vp8(NT - 1)
            for pm in AOCP_TRAIL:
                outcopy(pm)
            for j in AOC1_TRAIL:
                outcopy1(j)

        @block.gpsimd
        def _(po):
            npo = 0
            for m in range(NP):
                zw = 384 if m == 5 else 128
                po.memset(e8s[m][:, 1, 0:zw], 0.0).then_inc(sPO, 1)
                npo += 1
                po.memset(vp8[:, m, 1, :], 0.0).then_inc(sPO, 1)
                npo += 1
            assert npo == NMEMSET

    nc.finalize()
    return nc


def _host_inputs(xb, wq16, wk16, wv8h, bq, bk, bv, T):
    # mask class 3: cols x < 384 + p get MASKVAL; class c uses the slice
    # [512-128*(c+1) : 512]
    p = np.arange(128, dtype=np.float32)[:, None]
    xx = np.arange(512, dtype=np.float32)[None, :]
    msk = np.where(xx >= 384.0 + p, 0.0, MASKVAL).astype(np.float16)
    idn = np.eye(128, dtype=np.float16)

    xT = np.ascontiguousarray(xb.T)  # [D, T] f32
    x16 = xT.astype(np.float16)
    x8 = np.ascontiguousarray(
        xT.reshape(2, 2, 128, T).transpose(2, 0, 1, 3)
    ).astype(ml_dtypes.float8_e4m3fn)
    return dict(
        x16=x16,
        x8=np.ascontiguousarray(x8),
        wq16=wq16,
        wk16=wk16,
        wv8=wv8h,
        bq=bq,
        bk=bk,
        bv=bv,
        msk16=np.ascontiguousarray(msk),
        idn16=np.ascontiguousarray(idn),
    )


def kernel(x, Wk, bk, Wq, bq, Wv, bv):
    global LAST_RESULTS
    T = 2048
    x = np.ascontiguousarray(np.asarray(x, dtype=np.float32))
    Wk = np.asarray(Wk, dtype=np.float32)
    Wq = np.asarray(Wq, dtype=np.float32)
    Wv = np.asarray(Wv, dtype=np.float32)
    bk = np.ascontiguousarray(np.asarray(bk, dtype=np.float32))
    bq = np.ascontiguousarray(np.asarray(bq, dtype=np.float32))
    bv = np.ascontiguousarray(np.asarray(bv, dtype=np.float32))

    wq16 = np.ascontiguousarray(Wq.T).astype(np.float16)
    wk16 = np.ascontiguousarray(Wk.T).astype(np.float16)
    wvT = Wv.T * np.float32(INV_SQRT_K)  # [D, KS]
    wv8h = np.ascontiguousarray(
        wvT.reshape(2, 2, 128, KS).transpose(2, 0, 1, 3)
    ).astype(ml_dtypes.float8_e4m3fn)
    bvs = np.ascontiguousarray(bv * np.float32(INV_SQRT_K))

    nc = build_nc(T)
    in_maps = [_host_inputs(x[b], wq16, wk16, wv8h, bq, bk, bvs, T) for b in range(B)]
    res = None
    read = None
    last_exc = None
    for attempt in range(4):
        try:
            res = run_bass_kernel_spmd(nc, in_maps, list(range(B)), trace=TRACE)
            read = np.stack(
                [
                    np.asarray(res.results[b]["out"]).astype(np.float32)
                    for b in range(B)
                ],
                axis=0,
            )
            if np.isfinite(read).all():
                break
            read = None  # flaky device run produced NaN/inf; retry
        except Exception as e:  # transient NRT device errors; retry fresh
            last_exc = e
        import time as _time
        _time.sleep(5)
        nc = build_nc(T)
    if read is None:
        if last_exc is not None:
            raise last_exc
        raise RuntimeError("device produced non-finite output on all attempts")
    LAST_RESULTS = res
    return (x + read).astype(np.float32)


# revision 9
# speedup vs baseline: 1.7137x; 1.0006x over previous
"""Trainium2 Bass kernel for nn_AttentionBlock (causal attn, softmax over the
QUERY axis (dim=1), post-softmax 1/sqrt(K) scale, residual add).

Sharding: data-parallel over batch B=8, one batch element per NeuronCore.

v4 design:
- K/Q projections + logits in fp16; V projection and the probability-weighted
  read in fp8e4 with MatmulPerfMode.DoubleRow (two 128-deep contraction tiles
  per matmul).
- The causal mask is applied BY THE PE: each diagonal logits group gets one
  extra 128-wide matmul (identity stationary x f16 mask pattern of 0/-60000)
  accumulated into the PSUM bank.  exp() of -60000-ish underflows to exactly
  0.  No DVE mask-add, no staging: every chunk is max-reduced and exp'd
  straight from its PSUM bank.
- Logits PSUM banks live in ONE 6-bank tensor (pse) plus a 2-bank read tensor
  (psRD).  The first 8 logits chunks use all 8 banks (prologue), then a
  6-bank rotation.  Adjacent-bank non-diag chunk runs are processed by SINGLE
  wide ops: one DVE reduce yields 2-3 column maxes, one ACT exp covers 2-3
  chunks (one f32 accum partial per op).
- Denominator combine + reciprocal on DVE; V'-scales split DVE (even jt) /
  ACT (odd jt); output evacuation is PAIRED: one op copies both read banks
  [128,2,512] -> bf16 outst, one DMA stores 256 output rows; pairs alternate
  ACT/DVE.  Pool does only the startup memsets.
- E8[j,i] strips live in PAIRED key-chunk layout e8s[m][:, slot, :]
  (slot = jt%2, strip base column 256m) feeding DoubleRow reads directly;
  slot-1's first 128 columns are memset 0 (sub-diagonal).
    read[i, :] = sum_j E[j, i] * V'[j, :],  V' = (v + bv) * rec_j / sqrt(K)

Raw Block style with manual semaphores: ONE embedded sync-wait per
instruction; cross-engine deps are standalone wait_ge with statically
computed thresholds; same-engine RAW pairs get explicit fences.
"""

import math
import os
import sys

import numpy as np
import ml_dtypes

for _p in ("/opt/trn_rl_repo", "/root/.axon_site/_ro/trn_rl_repo"):
    if os.path.isdir(_p) and _p not in sys.path:
        sys.path.append(_p)

import concourse.bass as bass
from concourse import mybir
from concourse.bass_utils import run_bass_kernel_spmd

B = 8
D = 512
KS = 512
ND = D // 128  # 4 contraction tiles

F32 = mybir.dt.float32
F16 = mybir.dt.float16
BF16 = mybir.dt.bfloat16
F8 = mybir.dt.float8e4
AOP = mybir.AluOpType
AFT = mybir.ActivationFunctionType
DR = mybir.MatmulPerfMode.DoubleRow

INV_SQRT_K = 1.0 / math.sqrt(KS)
MASKVAL = -60000.0  # fits f16; exp(-60000 + max) == 0 exactly

TRACE = False
LAST_RESULTS = None


def _c0(jt):
    return (128 * jt) // 512


def build_nc(T=2048, debug_dump=False):
    NT = T // 128   # 16 row chunks
    NCH = T // 512  # 4 column chunks
    NP = NT // 2    # 8 key-chunk pairs
    KQ = ND * NCH   # 16 projection output groups for each of q/k

    nc = bass.Bass("TRN2", target_bir_lowering=False, debug=False, num_devices=B)

    # ---- DRAM ----
    x16_d = nc.dram_tensor("x16", [D, T], F16, kind="ExternalInput")
    x8_d = nc.dram_tensor("x8", [128, 2, 2, T], F8, kind="ExternalInput")
    wq_d = nc.dram_tensor("wq16", [D, KS], F16, kind="ExternalInput")
    wk_d = nc.dram_tensor("wk16", [D, KS], F16, kind="ExternalInput")
    wv_d = nc.dram_tensor("wv8", [128, 2, 2, KS], F8, kind="ExternalInput")
    bq_d = nc.dram_tensor("bq", [KS], F32, kind="ExternalInput")
    bk_d = nc.dram_tensor("bk", [KS], F32, kind="ExternalInput")
    bv_d = nc.dram_tensor("bv", [KS], F32, kind="ExternalInput")
    msk_d = nc.dram_tensor("msk16", [128, 512], F16, kind="ExternalInput")
    idn_d = nc.dram_tensor("idn16", [128, 128], F16, kind="ExternalInput")
    out_d = nc.dram_tensor("out", [T, KS], BF16, kind="ExternalOutput")

    # ---- SBUF ----
    xTr = nc.alloc_sbuf_tensor("xTr", [128, ND, T], F16)
    x8s = nc.alloc_sbuf_tensor("x8s", [128, 2, 2, T], F8)
    wkr = nc.alloc_sbuf_tensor("wkr", [128, ND, KS], F16)
    wqr = nc.alloc_sbuf_tensor("wqr", [128, ND, KS], F16)
    wv8 = nc.alloc_sbuf_tensor("wv8s", [128, 2, 2, KS], F8)
    kT = nc.alloc_sbuf_tensor("kT", [128, ND, T], F16)
    qT = nc.alloc_sbuf_tensor("qT", [128, ND, T], F16)
    v_sb = nc.alloc_sbuf_tensor("v_sb", [128, NT, KS], BF16)
    vp8 = nc.alloc_sbuf_tensor("vp8", [128, NP, 2, KS], F8)
    e8s = [
        nc.alloc_sbuf_tensor(f"e8_{m}", [128, 2, T - (1024 if m == 5 else 256 * m)], F8)
        for m in range(NP)
    ]
    outst = nc.alloc_sbuf_tensor("outst", [128, 2, 2, KS], BF16)
    mask16 = nc.alloc_sbuf_tensor("mask16", [128, 512], F16)
    ident16 = nc.alloc_sbuf_tensor("ident16", [128, 128], F16)
    bqc = nc.alloc_sbuf_tensor("bqc", [128, ND], F32)
    bkc = nc.alloc_sbuf_tensor("bkc", [128, ND], F32)
    bvb = nc.alloc_sbuf_tensor("bvb", [128, KS], F32)
    macc = nc.alloc_sbuf_tensor("macc", [128, 16, NCH], F32)
    negmax = nc.alloc_sbuf_tensor("negmax", [128, 16], F32)
    dparts = nc.alloc_sbuf_tensor("dparts", [128, 16, NCH], F32)
    denom = nc.alloc_sbuf_tensor("denom", [128, 16], F32)
    rec = nc.alloc_sbuf_tensor("rec", [128, 16], F32)

    # ---- PSUM: 6-bank logits rotation + 2 read banks ----
    pse = nc.alloc_psum_tensor("pse", [128, 6, 512], F32)
    psRD = nc.alloc_psum_tensor("psRD", [128, 2, 512], F32)
    # phase 1: v -> pse[0..1], k/q -> pse[2..5]

    # ================= static op-index tables =================
    LA = 3  # read lookahead: R(j) sits at fused position j + LA_OF(j)
    LATE_LA = 3

    def LA_OF(j):
        return LA if j < 8 else LATE_LA

    def bank_of_g(g):
        if g < 6:
            return ("E", g)
        if g < 8:
            return ("R", g - 6)
        return ("E", (g - 8) % 6)

    def prev_user_g(g):
        if g < 8:
            return None
        return g - 8 if g < 14 else g - 6

    gbank = {}
    g_of_chunk = {}
    chunk_of_g = {}
    g = 0
    for jt in range(NT):
        for ic in range(_c0(jt), NCH):
            gbank[(jt, ic)] = bank_of_g(g)
            g_of_chunk[(jt, ic)] = g
            chunk_of_g[g] = (jt, ic)
            g += 1

    # rows whose diagonal chunk is computed FULL-width so diag+nondiag form
    # one adjacent-bank pair handled by single wide reduce/exp ops
    FULL = {8, 9, 10, 11}
    # e8 strip base columns (pair 5 starts at row 10's full-width diag)
    BASE = [256 * m for m in range(NP)]
    BASE[5] = 1024

    # non-diag chunk groups per row: maximal runs of adjacent banks
    row_groups = {}
    for jt in range(NT):
        c0 = _c0(jt)
        ics = list(range(c0 + 1, NCH))
        groups = []
        i = 0
        while i < len(ics):
            run = [ics[i]]
            while (
                i + 1 < len(ics)
                and len(run) < 3
                and gbank[(jt, ics[i + 1])][0] == gbank[(jt, run[0])][0]
                and gbank[(jt, ics[i + 1])][1]
                == gbank[(jt, run[0])][1] + len(run)
            ):
                run.append(ics[i + 1])
                i += 1
            groups.append(run)
            i += 1
        row_groups[jt] = groups

    # ---- PE plan (sPE counts GROUPS) ----
    VG, KG, QG, KQSEQ, LG, RG = {}, {}, {}, {}, {}, {}
    pe = 0
    kqseq = 0
    p1_order = []
    for b in range(NCH):
        for r in range(4):
            pe += 1
            KG[(r, b)] = pe
            kqseq += 1
            KQSEQ[("k", r, b)] = kqseq
            p1_order.append(("k", r, b))
            jt = 4 * b + r
            if jt < NT:
                pe += 1
                VG[jt] = pe
                p1_order.append(("v", jt))
        for kt in range(ND):
            pe += 1
            QG[(kt, b)] = pe
            kqseq += 1
            KQSEQ[("q", kt, b)] = kqseq
            p1_order.append(("q", kt, b))
    assert pe == NT + 2 * KQ

    fused_order = []
    for m in range(NT):
        fused_order.append(("L", m))
        for j in range(NT):
            if j + LA_OF(j) == m:
                fused_order.append(("R", j))
    for j in range(NT):
        if j + LA_OF(j) >= NT:
            fused_order.append(("R", j))

    for kind, jt in fused_order:
        if kind == "L":
            for ic in range(_c0(jt), NCH):
                pe += 1
                LG[(jt, ic)] = pe
        else:
            pe += 1
            RG[jt] = pe

    # paired outcopies: pair pm covers read rows (2pm, 2pm+1);
    # even pm -> ACT, odd pm -> DVE; due at fused position 2pm+1+LA
    aocp_due, docp_due = {}, {}
    AOCP_TRAIL, DOCP_TRAIL = [], []
    NPAIRED = 0  # pairs 0..5 cover rows 0..11; rows 12..15 get single OCs
    for pm in range(NPAIRED):
        due = 2 * pm + 1 + LA_OF(2 * pm + 1)
        tgt = aocp_due if pm % 2 == 0 else docp_due
        trail = AOCP_TRAIL if pm % 2 == 0 else DOCP_TRAIL
        if due < NT:
            tgt.setdefault(due, []).append(pm)
        else:
            trail.append(pm)
    # single outcopies for rows 12..15: ACT even rows, DVE odd rows
    aoc1_due, doc1_due = {}, {}
    AOC1_TRAIL, DOC1_TRAIL = [], []
    NSING0 = 2 * NPAIRED  # first single-OC row
    for j in range(NSING0, NT):
        due = j + LA_OF(j)
        tgt = aoc1_due if j % 2 == 0 else doc1_due
        trail = AOC1_TRAIL if j % 2 == 0 else DOC1_TRAIL
        if due < NT:
            tgt.setdefault(due, []).append(j)
        else:
            trail.append(j)

    # ---- ACT plan (sAC): 32 kq copies, then per jt: due paired outcopies,
    #      exps (diag first, then non-diag groups), odd-jt V'-scale ----
    EXPG = {}      # (jt, gi) -> act idx; gi 0 = diag, 1.. = groups
    EXPREL = {}    # (jt, ic) -> act idx of the exp covering the chunk
    EXP_END = {}
    NOPS = {}
    AOCP, VP8A, AOC1 = {}, {}, {}
    ac = 2 * KQ
    for jt in range(NT):
        c0 = _c0(jt)
        for pm in aocp_due.get(jt, []):
            ac += 1
            AOCP[pm] = ac
        if jt in FULL:
            ac += 1
            EXPG[(jt, 0)] = ac  # merged diag+nondiag
            for ic in range(c0, NCH):
                EXPREL[(jt, ic)] = ac
            EXP_END[jt] = ac
            NOPS[jt] = 1
        else:
            ac += 1
            EXPG[(jt, 0)] = ac  # diag
            EXPREL[(jt, c0)] = ac
            gi = 1
            for run in row_groups[jt]:
                ac += 1
                EXPG[(jt, gi)] = ac
                for ic in run:
                    EXPREL[(jt, ic)] = ac
                gi += 1
            EXP_END[jt] = ac
            NOPS[jt] = gi
        if jt >= 1 and (jt - 1) % 2 == 1:
            ac += 1
            VP8A[jt - 1] = ac
        for j in aoc1_due.get(jt, []):
            ac += 1
            AOC1[j] = ac
    if (NT - 1) % 2 == 1:
        ac += 1
        VP8A[NT - 1] = ac
    for pm in AOCP_TRAIL:
        ac += 1
        AOCP[pm] = ac
    for j in AOC1_TRAIL:
        ac += 1
        AOC1[j] = ac

    # ---- DVE plan (sDV): 16 v-copies, then per jt: DRED (diag max from
    #      bank; folded into NMX when nch==1), RED groups, NMX, [DENOM],
    #      RECIP, even-jt V'-scale, due paired outcopies ----
    VCP, DRED, REDG, NMX, DENOM, RECIP, VP8D, DOCP, DOC1 = {}, {}, {}, {}, {}, {}, {}, {}, {}
    dv = 0
    for jt in range(NT):
        dv += 1
        VCP[jt] = dv

    def _dve_tail(jt):
        # denominator chain of row jt, emitted one block later
        nonlocal_dv = []
        return nonlocal_dv

    for jt in range(NT + 1):
        if jt < NT:
            nch = NCH - _c0(jt)
            if jt in FULL:
                dv += 1
                DRED[jt] = dv  # merged 2-bank reduce
            elif nch > 1:
                dv += 1
                DRED[jt] = dv
                for ric in range(_c0(jt) + 1, NCH):
                    dv += 1
                    REDG[(jt, ric)] = dv
            dv += 1
            NMX[jt] = dv
        pj = jt - 1  # previous row's denominator chain
        if 0 <= pj < NT:
            if NOPS[pj] > 1:
                dv += 1
                DENOM[pj] = dv
            dv += 1
            RECIP[pj] = dv
            if pj % 2 == 0:
                dv += 1
                VP8D[pj] = dv
        if jt < NT:
            for pm in docp_due.get(jt, []):
                dv += 1
                DOCP[pm] = dv
            for j in doc1_due.get(jt, []):
                dv += 1
                DOC1[j] = dv
    for pm in DOCP_TRAIL:
        dv += 1
        DOCP[pm] = dv
    for j in DOC1_TRAIL:
        dv += 1
        DOC1[j] = dv

    # ---- Pool plan (sPO): slot-1 memsets only ----
    NMEMSET = 2 * NP

    def st_thr(pm):
        # paired store pm is the (pm+1)-th store
        return 16 * (pm + 1)

    def st1_thr(j):
        # single store for row j is the (NPAIRED + j - NSING0 + 1)-th store
        return 16 * (NPAIRED + j - NSING0 + 1)

    def oc1_st_wait(j):
        # outst slot (s2=(j//2)%2, s1=j%2) previously used by single j-4 or
        # by pair (j-4)//2
        if j - 4 >= NSING0:
            return st1_thr(j - 4)
        return st_thr((j - 4) // 2)

    def bank_ap(coord, w=512):
        t, slot = coord
        if t == "E":
            return pse[:, slot, 0:w]
        return psRD[:, slot, 0:w]

    def bank_run_ap(coord, ln):
        t, slot = coord
        if t == "E":
            return pse[:, slot : slot + ln, :]
        return psRD[:, slot : slot + ln, :]

    def vp8_wait(te_or_none, jt):
        # (sem, thr) releasing vp8(jt)
        if jt % 2 == 0:
            return ("DV", VP8D[jt])
        return ("AC", VP8A[jt])

    with (
        nc.semaphore("sLv") as sLv,
        nc.semaphore("sLk") as sLk,
        nc.semaphore("sLk2") as sLk2,
        nc.semaphore("sLq") as sLq,
        nc.semaphore("sLc") as sLc,
        nc.semaphore("sLm") as sLm,
        nc.semaphore("sLx0") as sLx0,
        nc.semaphore("sLxb") as sLxb,
        nc.semaphore("sLx1") as sLx1,
        nc.semaphore("sLx2") as sLx2,
        nc.semaphore("sLx3") as sLx3,
        nc.semaphore("sL80") as sL80,
        nc.semaphore("sL81") as sL81,
        nc.semaphore("sL82") as sL82,
        nc.semaphore("sL83") as sL83,
        nc.semaphore("sPE") as sPE,
        nc.semaphore("sDV") as sDV,
        nc.semaphore("sAC") as sAC,
        nc.semaphore("sPO") as sPO,
        nc.semaphore("sST") as sST,
        nc.Block() as block,
    ):
        sLxs = [sLx0, sLx1, sLx2, sLx3]
        sL8s = [sL80, sL81, sL82, sL83]

        @block.sync
        def _(sp):
            def ldx16(ic):
                sp.dma_start(
                    out=xTr[:, :, 512 * ic : 512 * (ic + 1)],
                    in_=x16_d.ap()[:, 512 * ic : 512 * (ic + 1)].rearrange(
                        "(t p) i -> p t i", p=128
                    ),
                ).then_inc(sLxs[ic], 16)

            def ldx8(ic):
                sp.dma_start(
                    out=x8s[:, :, :, 512 * ic : 512 * (ic + 1)],
                    in_=x8_d.ap()[:, :, :, 512 * ic : 512 * (ic + 1)],
                ).then_inc(sL8s[ic], 16)

            wk_re = wk_d.ap().rearrange("(t p) k -> p t k", p=128)
            sp.dma_start(out=wkr[:, :, 0:256], in_=wk_re[:, :, 0:256]).then_inc(
                sLk, 16
            )
            x16_re0 = x16_d.ap()[:, 0:512].rearrange("(t p) i -> p t i", p=128)
            sp.dma_start(out=xTr[:, :, 0:256], in_=x16_re0[:, :, 0:256]).then_inc(
                sLx0, 16
            )
            sp.dma_start(out=xTr[:, :, 256:512], in_=x16_re0[:, :, 256:512]).then_inc(
                sLxb, 16
            )
            sp.dma_start(out=wv8[:, :, :, :], in_=wv_d.ap()).then_inc(sLv, 16)
            ldx8(0)
            bv_ap = bv_d.ap()
            bv_bcast = bass.AP(
                tensor=bv_ap.tensor, offset=bv_ap.offset, ap=[[0, 128]] + list(bv_ap.ap)
            )
            sp.dma_start(out=bvb[:, :], in_=bv_bcast).then_inc(sLc, 16)
            with nc.allow_non_contiguous_dma(reason="16B/partition bias loads"):
                sp.dma_start(
                    out=bkc[:, :], in_=bk_d.ap().rearrange("(t p) -> p t", p=128)
                ).then_inc(sLc, 16)
                sp.dma_start(
                    out=bqc[:, :], in_=bq_d.ap().rearrange("(t p) -> p t", p=128)
                ).then_inc(sLc, 16)
            sp.dma_start(out=wkr[:, :, 256:512], in_=wk_re[:, :, 256:512]).then_inc(
                sLk2, 16
            )
            sp.dma_start(
                out=wqr[:, :, :],
                in_=wq_d.ap().rearrange("(t p) k -> p t k", p=128),
            ).then_inc(sLq, 16)
            sp.dma_start(out=mask16[:, :], in_=msk_d.ap()).then_inc(sLm, 16)
            sp.dma_start(out=ident16[:, :], in_=idn_d.ap()).then_inc(sLm, 16)
            ldx16(1)
            ldx8(1)
            ldx16(2)
            ldx8(2)
            ldx16(3)
            ldx8(3)
            # paired stores: 256 output rows each
            out_ap = out_d.ap()
            for pm in range(NPAIRED):
                if pm % 2 == 0:
                    sp.wait_ge(sAC, AOCP[pm])
                else:
                    sp.wait_ge(sDV, DOCP[pm])
                sp.dma_start(
                    out=out_ap[256 * pm : 256 * (pm + 1), :].rearrange(
                        "(s p) c -> p s c", p=128
                    ),
                    in_=outst[:, pm % 2, :, :],
                ).then_inc(sST, 16)
            for j in range(NSING0, NT):
                if j % 2 == 0:
                    sp.wait_ge(sAC, AOC1[j])
                else:
                    sp.wait_ge(sDV, DOC1[j])
                sp.dma_start(
                    out=out_ap[128 * j : 128 * (j + 1), :],
                    in_=outst[:, (j // 2) % 2, j % 2, :],
                ).then_inc(sST, 16)
            sp.wait_ge(sST, 16 * (NPAIRED + NT - NSING0))

        @block.tensor
        def _(te):
            waited = set()

            def ldwait(sem, thr=16):
                if sem not in waited:
                    te.wait_ge(sem, thr)
                    waited.add(sem)

            for item in p1_order:
                if item[0] == "v":
                    jt = item[1]
                    ldwait(sLv)
                    ldwait(sL8s[jt // 4])
                    if jt >= 2:
                        te.wait_ge(sDV, VCP[jt - 2])
                    for dm in range(2):
                        mm = te.matmul(
                            pse[:, jt % 2, :],
                            lhsT=x8s[:, dm, :, 128 * jt : 128 * (jt + 1)],
                            rhs=wv8[:, dm, :, :],
                            start=(dm == 0),
                            stop=(dm == 1),
                            perf_mode=DR,
                        )
                        if dm == 1:
                            mm.then_inc(sPE, 1)
                else:
                    kind, kt, ic = item
                    wsb = wkr if kind == "k" else wqr
                    if kind == "k":
                        ldwait(sLk if kt < 2 else sLk2)
                    else:
                        ldwait(sLq)
                    ldwait(sLxs[ic])
                    if ic == 0 and not (kind == "k" and kt == 0):
                        ldwait(sLxb)
                    seq = KQSEQ[(kind, kt, ic)]
                    if seq > 4:
                        te.wait_ge(sAC, seq - 4)
                    if kind == "k" and ic == 0:
                        # two half-width groups so the first k's start on the
                        # first half-load of x16 strip 0
                        for h in range(2):
                            if h == 1:
                                ldwait(sLxb)
                            for dt_ in range(ND):
                                mm = te.matmul(
                                    pse[:, 2 + ((seq - 1) % 4), 256 * h : 256 * (h + 1)],
                                    lhsT=wsb[:, dt_, 128 * kt : 128 * (kt + 1)],
                                    rhs=xTr[:, dt_, 256 * h : 256 * (h + 1)],
                                    start=(dt_ == 0),
                                    stop=(dt_ == ND - 1),
                                )
                                if h == 1 and dt_ == ND - 1:
                                    mm.then_inc(sPE, 1)
                    else:
                        for dt_ in range(ND):
                            mm = te.matmul(
                                pse[:, 2 + ((seq - 1) % 4), :],
                                lhsT=wsb[:, dt_, 128 * kt : 128 * (kt + 1)],
                                rhs=xTr[:, dt_, 512 * ic : 512 * (ic + 1)],
                                start=(dt_ == 0),
                                stop=(dt_ == ND - 1),
                            )
                            if dt_ == ND - 1:
                                mm.then_inc(sPE, 1)
            # fused: logits chunks (mask matmul appended to diag groups) +
            # DoubleRow read groups
            for kind, jt in fused_order:
                c0 = _c0(jt)
                if kind == "L":
                    for ic in range(c0, NCH):
                        gg = g_of_chunk[(jt, ic)]
                        diag = ic == c0
                        need_ac = KQSEQ[("q", ND - 1, ic)]
                        need_dv = None
                        pg = prev_user_g(gg)
                        if pg is not None:
                            pj, pic = chunk_of_g[pg]
                            need_ac = max(need_ac, EXPREL[(pj, pic)])
                        elif gg < 2:
                            need_dv = VCP[NT - 2 + gg]
                        elif gg < 6:
                            need_ac = max(need_ac, 2 * KQ)
                        te.wait_ge(sAC, need_ac)
                        if need_dv is not None:
                            te.wait_ge(sDV, need_dv)
                        if diag:
                            ldwait(sLm, 32)
                        dlo = 512 * c0 if jt in FULL else 128 * jt
                        w = 512 * (ic + 1) - (dlo if diag else 512 * ic)
                        bank = bank_ap(gbank[(jt, ic)], w)
                        ilo = dlo if diag else 512 * ic
                        for kt in range(ND):
                            mm = te.matmul(
                                bank,
                                lhsT=kT[:, kt, 128 * jt : 128 * (jt + 1)],
                                rhs=qT[:, kt, ilo : 512 * (ic + 1)],
                                start=(kt == 0),
                                stop=(kt == ND - 1) and not diag,
                            )
                            if kt == ND - 1 and not diag:
                                mm.then_inc(sPE, 1)
                        if diag:
                            # causal mask accumulated by the PE: identity
                            # stationary x (0/-60000) f16 pattern, class
                            # slice of the class-3 mask
                            cls = (jt % 4) if jt in FULL else 0
                            te.matmul(
                                bank_ap(gbank[(jt, ic)], 128 * (cls + 1)),
                                lhsT=ident16[:, :],
                                rhs=mask16[:, 512 - 128 * (cls + 1) : 512],
                                start=False,
                                stop=True,
                                skip_group_check=True,
                            ).then_inc(sPE, 1)
                else:
                    npair = (jt + 2) // 2
                    if jt >= 1:
                        sem, thr = vp8_wait(None, jt - 1)
                        te.wait_ge(sPO if sem == "PO" else (sDV if sem == "DV" else sAC), thr)
                    if jt < 2:
                        te.wait_ge(sAC, EXPREL[(1, 2)])
                    elif jt >= NSING0 + 2:
                        if jt % 2 == 0:
                            te.wait_ge(sAC, AOC1[jt - 2])
                        else:
                            te.wait_ge(sDV, DOC1[jt - 2])
                    else:
                        pm = (jt - 2) // 2
                        if pm % 2 == 0:
                            te.wait_ge(sAC, AOCP[pm])
                        else:
                            te.wait_ge(sDV, DOCP[pm])
                    if jt == 0:
                        te.wait_ge(sPO, NMEMSET)
                    for m in range(npair):
                        if m == npair - 1:
                            sem, thr = vp8_wait(None, jt)
                            te.wait_ge(sDV if sem == "DV" else sAC, thr)
                        mm = te.matmul(
                            psRD[:, jt % 2, :],
                            lhsT=e8s[m][
                                :, :, 128 * jt - BASE[m] : 128 * jt - BASE[m] + 128
                            ],
                            rhs=vp8[:, m, :, :],
                            start=(m == 0),
                            stop=(m == npair - 1),
                            perf_mode=DR,
                        )
                        if m == npair - 1:
                            mm.then_inc(sPE, 1)

        @block.vector
        def _(ve):
            ndv = 0

            def inc(x):
                nonlocal ndv
                ndv += 1
                x.then_inc(sDV, 1)

            ve.wait_ge(sLc, 16)
            for jt in range(NT):
                ve.wait_ge(sPE, VG[jt])
                inc(
                    ve.tensor_tensor(
                        out=v_sb[:, jt, :], in0=pse[:, jt % 2, :],
                        in1=bvb[:, :], op=AOP.add,
                    )
                )
                assert ndv == VCP[jt]

            def den_chain(rj):
                # denominator chain of row rj (one block late)
                rp = rj
                ve.wait_ge(sAC, EXP_END[rj])
                if NOPS[rj] > 1:
                    inc(
                        ve.reduce_sum(
                            denom[:, rp : rp + 1],
                            dparts[:, rp, 0 : NOPS[rj]],
                            mybir.AxisListType.X,
                        )
                    )
                    assert ndv == DENOM[rj]
                    ve.wait_ge(sDV, DENOM[rj])  # same-engine RAW fence
                    src = denom[:, rp : rp + 1]
                else:
                    src = dparts[:, rp, 0:1]
                inc(ve.reciprocal(rec[:, rp : rp + 1], src))
                assert ndv == RECIP[rj]
                if rj % 2 == 0:
                    ve.wait_ge(sDV, RECIP[rj])  # same-engine RAW fence
                    inc(
                        ve.tensor_scalar(
                            out=vp8[:, rj // 2, (rj % 2), :],
                            in0=v_sb[:, rj, :],
                            scalar1=rec[:, rp : rp + 1],
                            scalar2=None,
                            op0=AOP.mult,
                        )
                    )
                    assert ndv == VP8D[rj]

            for jt in range(NT):
                c0 = _c0(jt)
                nch = NCH - c0
                w0 = 512 * (c0 + 1) - 128 * jt
                pj = jt  # dedicated per-row softmax state, no reuse
                ve.wait_ge(sPE, LG[(jt, c0)])
                if jt in FULL:
                    ve.wait_ge(sPE, LG[(jt, c0 + 1)])
                    inc(
                        ve.reduce_max(
                            macc[:, pj, c0 : c0 + 2],
                            bank_run_ap(gbank[(jt, c0)], 2),
                            mybir.AxisListType.X,
                        )
                    )
                    assert ndv == DRED[jt]
                    ve.wait_ge(sDV, DRED[jt])
                    inc(
                        ve.reduce_max(
                            negmax[:, pj : pj + 1],
                            macc[:, pj, c0:NCH],
                            mybir.AxisListType.X, negate=True,
                        )
                    )
                elif nch > 1:
                    inc(
                        ve.reduce_max(
                            macc[:, pj, c0 : c0 + 1],
                            bank_ap(gbank[(jt, c0)], w0),
                            mybir.AxisListType.X,
                        )
                    )
                    assert ndv == DRED[jt]
                    for ric in range(c0 + 1, NCH):
                        ve.wait_ge(sPE, LG[(jt, ric)])
                        inc(
                            ve.reduce_max(
                                macc[:, pj, ric : ric + 1],
                                bank_ap(gbank[(jt, ric)], 512),
                                mybir.AxisListType.X,
                            )
                        )
                        assert ndv == REDG[(jt, ric)]
                    ve.wait_ge(sDV, REDG[(jt, NCH - 1)])
                    inc(
                        ve.reduce_max(
                            negmax[:, pj : pj + 1],
                            macc[:, pj, c0:NCH],
                            mybir.AxisListType.X, negate=True,
                        )
                    )
                else:
                    inc(
                        ve.reduce_max(
                            negmax[:, pj : pj + 1],
                            bank_ap(gbank[(jt, c0)], w0),
                            mybir.AxisListType.X, negate=True,
                        )
                    )
                assert ndv == NMX[jt]
                if jt >= 1:
                    den_chain(jt - 1)
                for pm in docp_due.get(jt, []):
                    ve.wait_ge(sPE, RG[2 * pm + 1])
                    if pm >= 2:
                        ve.wait_ge(sST, st_thr(pm - 2))
                    inc(
                        ve.tensor_scalar_add(
                            out=outst[:, pm % 2, :, :], in0=psRD[:, :, :],
                            scalar1=0.0,
                        )
                    )
                    assert ndv == DOCP[pm]
                for j in doc1_due.get(jt, []):
                    ve.wait_ge(sPE, RG[j])
                    if j >= 4:
                        ve.wait_ge(sST, oc1_st_wait(j))
                    inc(
                        ve.tensor_scalar_add(
                            out=outst[:, (j // 2) % 2, j % 2, :],
                            in0=psRD[:, j % 2, :],
                            scalar1=0.0,
                        )
                    )
                    assert ndv == DOC1[j]
            den_chain(NT - 1)
            for pm in DOCP_TRAIL:
                ve.wait_ge(sPE, RG[2 * pm + 1])
                if pm >= 2:
                    ve.wait_ge(sST, st_thr(pm - 2))
                inc(
                    ve.tensor_scalar_add(
                        out=outst[:, pm % 2, :, :], in0=psRD[:, :, :],
                        scalar1=0.0,
                    )
                )
                assert ndv == DOCP[pm]
            for j in DOC1_TRAIL:
                ve.wait_ge(sPE, RG[j])
                if j >= 4:
                    ve.wait_ge(sST, oc1_st_wait(j))
                inc(
                    ve.tensor_scalar_add(
                        out=outst[:, (j // 2) % 2, j % 2, :],
                        in0=psRD[:, j % 2, :],
                        scalar1=0.0,
                    )
                )
                assert ndv == DOC1[j]

        @block.scalar
        def _(ac_):
            ac_.wait_ge(sLc, 48)
            nac = 0
            for ic in range(NCH):
                for wsel, g_tab, bias in ((0, KG, bkc), (1, QG, bqc)):
                    dst = kT if wsel == 0 else qT
                    for kt in range(ND):
                        seq = KQSEQ[("k" if wsel == 0 else "q", kt, ic)]
                        ac_.wait_ge(sPE, g_tab[(kt, ic)])
                        ac_.activation(
                            out=dst[:, kt, 512 * ic : 512 * (ic + 1)],
                            in_=pse[:, 2 + ((seq - 1) % 4), :],
                            func=AFT.Identity,
                            bias=bias[:, kt : kt + 1],
                            scale=1.0,
                        ).then_inc(sAC, 1)
                        nac += 1
            assert nac == 2 * KQ

            def outcopy(pm):
                nonlocal nac
                ac_.wait_ge(sPE, RG[2 * pm + 1])
                if pm >= 2:
                    ac_.wait_ge(sST, st_thr(pm - 2))
                nac += 1
                ac_.activation(
                    out=outst[:, pm % 2, :, :], in_=psRD[:, :, :], func=AFT.Copy
                ).then_inc(sAC, 1)
                assert nac == AOCP[pm]

            def outcopy1(j):
                nonlocal nac
                ac_.wait_ge(sPE, RG[j])
                if j >= 4:
                    ac_.wait_ge(sST, oc1_st_wait(j))
                nac += 1
                ac_.activation(
                    out=outst[:, (j // 2) % 2, j % 2, :],
                    in_=psRD[:, j % 2, :],
                    func=AFT.Copy,
                ).then_inc(sAC, 1)
                assert nac == AOC1[j]

            def act_vp8(rj):
                nonlocal nac
                rp = rj
                ac_.wait_ge(sDV, RECIP[rj])
                nac += 1
                ac_.activation(
                    out=vp8[:, rj // 2, rj % 2, :],
                    in_=v_sb[:, rj, :],
                    func=AFT.Copy,
                    bias=0.0,
                    scale=rec[:, rp : rp + 1],
                ).then_inc(sAC, 1)
                assert nac == VP8A[rj]

            for jt in range(NT):
                c0 = _c0(jt)
                pj = jt  # dedicated per-row softmax state, no reuse
                m = jt // 2
                base = BASE[m]
                for pm in aocp_due.get(jt, []):
                    outcopy(pm)
                ac_.wait_ge(sDV, NMX[jt])
                if jt in FULL:
                    # one exp over the adjacent diag+nondiag bank pair
                    nac += 1
                    ac_.activation(
                        out=e8s[m][
                            :, jt % 2, 512 * c0 - base : 512 * (c0 + 2) - base
                        ],
                        in_=bank_run_ap(gbank[(jt, c0)], 2),
                        func=AFT.Exp,
                        bias=negmax[:, pj : pj + 1],
                        scale=1.0,
                        accum_out=dparts[:, pj, 0:1],
                    ).then_inc(sAC, 1)
                    assert nac == EXPG[(jt, 0)]
                else:
                    # exps: diag first (frees the rotation bank soonest)
                    w0 = 512 * (c0 + 1) - 128 * jt
                    nac += 1
                    ac_.activation(
                        out=e8s[m][
                            :, jt % 2, 128 * jt - base : 512 * (c0 + 1) - base
                        ],
                        in_=bank_ap(gbank[(jt, c0)], w0),
                        func=AFT.Exp,
                        bias=negmax[:, pj : pj + 1],
                        scale=1.0,
                        accum_out=dparts[:, pj, 0:1],
                    ).then_inc(sAC, 1)
                    assert nac == EXPG[(jt, 0)]
                    sidx = 1
                    for run in row_groups[jt]:
                        a = 512 * run[0]
                        bcol = 512 * (run[-1] + 1)
                        nac += 1
                        ac_.activation(
                            out=e8s[m][:, jt % 2, a - base : bcol - base],
                            in_=bank_run_ap(gbank[(jt, run[0])], len(run)),
                            func=AFT.Exp,
                            bias=negmax[:, pj : pj + 1],
                            scale=1.0,
                            accum_out=dparts[:, pj, sidx : sidx + 1],
                        ).then_inc(sAC, 1)
                        assert nac == EXPG[(jt, sidx)]
                        sidx += 1
                    assert sidx == NOPS[jt]
                if jt >= 1 and (jt - 1) % 2 == 1:
                    act_vp8(jt - 1)
                for j in aoc1_due.get(jt, []):
                    outcopy1(j)
            if (NT - 1) % 2 == 1:
                act_vp8(NT - 1)
            for pm in AOCP_TRAIL:
                outcopy(pm)
            for j in AOC1_TRAIL:
                outcopy1(j)

        @block.gpsimd
        def _(po):
            npo = 0
            for m in range(NP):
                zw = 384 if m == 5 else 128
                po.memset(e8s[m][:, 1, 0:zw], 0.0).then_inc(sPO, 1)
                npo += 1
                po.memset(vp8[:, m, 1, :], 0.0).then_inc(sPO, 1)
                npo += 1
            assert npo == NMEMSET

    nc.finalize()
    return nc


def _host_inputs(xb, wq16, wk16, wv8h, bq, bk, bv, T):
    # mask class 3: cols x < 384 + p get MASKVAL; class c uses the slice
    # [512-128*(c+1) : 512]
    p = np.arange(128, dtype=np.float32)[:, None]
    xx = np.arange(512, dtype=np.float32)[None, :]
    msk = np.where(xx >= 384.0 + p, 0.0, MASKVAL).astype(np.float16)
    idn = np.eye(128, dtype=np.float16)

    xT = np.ascontiguousarray(xb.T)  # [D, T] f32
    x16 = xT.astype(np.float16)
    x8 = np.ascontiguousarray(
        xT.reshape(2, 2, 128, T).transpose(2, 0, 1, 3)
    ).astype(ml_dtypes.float8_e4m3fn)
    return dict(
        x16=x16,
        x8=np.ascontiguousarray(x8),
        wq16=wq16,
        wk16=wk16,
        wv8=wv8h,
        bq=bq,
        bk=bk,
        bv=bv,
        msk16=np.ascontiguousarray(msk),
        idn16=np.ascontiguousarray(idn),
    )


def kernel(x, Wk, bk, Wq, bq, Wv, bv):
    global LAST_RESULTS
    T = 2048
    x = np.ascontiguousarray(np.asarray(x, dtype=np.float32))
    Wk = np.asarray(Wk, dtype=np.float32)
    Wq = np.asarray(Wq, dtype=np.float32)
    Wv = np.asarray(Wv, dtype=np.float32)
    bk = np.ascontiguousarray(np.asarray(bk, dtype=np.float32))
    bq = np.ascontiguousarray(np.asarray(bq, dtype=np.float32))
    bv = np.ascontiguousarray(np.asarray(bv, dtype=np.float32))

    wq16 = np.ascontiguousarray(Wq.T).astype(np.float16)
    wk16 = np.ascontiguousarray(Wk.T).astype(np.float16)
    wvT = Wv.T * np.float32(INV_SQRT_K)  # [D, KS]
    wv8h = np.ascontiguousarray(
        wvT.reshape(2, 2, 128, KS).transpose(2, 0, 1, 3)
    ).astype(ml_dtypes.float8_e4m3fn)
    bvs = np.ascontiguousarray(bv * np.float32(INV_SQRT_K))

    nc = build_nc(T)
    in_maps = [_host_inputs(x[b], wq16, wk16, wv8h, bq, bk, bvs, T) for b in range(B)]
    res = None
    read = None
    last_exc = None
    for attempt in range(4):
        try:
            res = run_bass_kernel_spmd(nc, in_maps, list(range(B)), trace=TRACE)
            read = np.stack(
                [
                    np.asarray(res.results[b]["out"]).astype(np.float32)
                    for b in range(B)
                ],
                axis=0,
            )
            if np.isfinite(read).all():
                break
            read = None  # flaky device run produced NaN/inf; retry
        except Exception as e:  # transient NRT device errors; retry fresh
            last_exc = e
        import time as _time
        _time.sleep(5)
        nc = build_nc(T)
    if read is None:
        if last_exc is not None:
            raise last_exc
        raise RuntimeError("device produced non-finite output on all attempts")
    LAST_RESULTS = res
    return (x + read).astype(np.float32)


# revision 10
# speedup vs baseline: 1.7208x; 1.0041x over previous
"""Trainium2 Bass kernel for nn_AttentionBlock (causal attn, softmax over the
QUERY axis (dim=1), post-softmax 1/sqrt(K) scale, residual add).

Sharding: data-parallel over batch B=8, one batch element per NeuronCore.

v4 design:
- K/Q projections + logits in fp16; V projection and the probability-weighted
  read in fp8e4 with MatmulPerfMode.DoubleRow (two 128-deep contraction tiles
  per matmul).
- The causal mask is applied BY THE PE: each diagonal logits group gets one
  extra 128-wide matmul (identity stationary x f16 mask pattern of 0/-60000)
  accumulated into the PSUM bank.  exp() of -60000-ish underflows to exactly
  0.  No DVE mask-add, no staging: every chunk is max-reduced and exp'd
  straight from its PSUM bank.
- Logits PSUM banks live in ONE 6-bank tensor (pse) plus a 2-bank read tensor
  (psRD).  The first 8 logits chunks use all 8 banks (prologue), then a
  6-bank rotation.  Adjacent-bank non-diag chunk runs are processed by SINGLE
  wide ops: one DVE reduce yields 2-3 column maxes, one ACT exp covers 2-3
  chunks (one f32 accum partial per op).
- Denominator combine + reciprocal on DVE; V'-scales split DVE (even jt) /
  ACT (odd jt); output evacuation is PAIRED: one op copies both read banks
  [128,2,512] -> bf16 outst, one DMA stores 256 output rows; pairs alternate
  ACT/DVE.  Pool does only the startup memsets.
- E8[j,i] strips live in PAIRED key-chunk layout e8s[m][:, slot, :]
  (slot = jt%2, strip base column 256m) feeding DoubleRow reads directly;
  slot-1's first 128 columns are memset 0 (sub-diagonal).
    read[i, :] = sum_j E[j, i] * V'[j, :],  V' = (v + bv) * rec_j / sqrt(K)

Raw Block style with manual semaphores: ONE embedded sync-wait per
instruction; cross-engine deps are standalone wait_ge with statically
computed thresholds; same-engine RAW pairs get explicit fences.
"""

import math
import os
import sys

import numpy as np
import ml_dtypes

for _p in ("/opt/trn_rl_repo", "/root/.axon_site/_ro/trn_rl_repo"):
    if os.path.isdir(_p) and _p not in sys.path:
        sys.path.append(_p)

import concourse.bass as bass
from concourse import mybir
from concourse.bass_utils import run_bass_kernel_spmd

B = 8
D = 512
KS = 512
ND = D // 128  # 4 contraction tiles

F32 = mybir.dt.float32
F16 = mybir.dt.float16
BF16 = mybir.dt.bfloat16
F8 = mybir.dt.float8e4
AOP = mybir.AluOpType
AFT = mybir.ActivationFunctionType
DR = mybir.MatmulPerfMode.DoubleRow

INV_SQRT_K = 1.0 / math.sqrt(KS)
MASKVAL = -60000.0  # fits f16; exp(-60000 + max) == 0 exactly

TRACE = False
LAST_RESULTS = None


def _c0(jt):
    return (128 * jt) // 512


def build_nc(T=2048, debug_dump=False):
    NT = T // 128   # 16 row chunks
    NCH = T // 512  # 4 column chunks
    NP = NT // 2    # 8 key-chunk pairs
    KQ = ND * NCH   # 16 projection output groups for each of q/k

    nc = bass.Bass("TRN2", target_bir_lowering=False, debug=False, num_devices=B)

    # ---- DRAM ----
    x16_d = nc.dram_tensor("x16", [D, T], F16, kind="ExternalInput")
    x8_d = nc.dram_tensor("x8", [128, 2, 2, T], F8, kind="ExternalInput")
    wq_d = nc.dram_tensor("wq16", [D, KS], F16, kind="ExternalInput")
    wk_d = nc.dram_tensor("wk16", [D, KS], F16, kind="ExternalInput")
    wv_d = nc.dram_tensor("wv8", [128, 2, 2, KS], F8, kind="ExternalInput")
    bq_d = nc.dram_tensor("bq", [KS], F32, kind="ExternalInput")
    bk_d = nc.dram_tensor("bk", [KS], F32, kind="ExternalInput")
    bv_d = nc.dram_tensor("bv", [KS], F32, kind="ExternalInput")
    msk_d = nc.dram_tensor("msk16", [128, 512], F16, kind="ExternalInput")
    idn_d = nc.dram_tensor("idn16", [128, 128], F16, kind="ExternalInput")
    out_d = nc.dram_tensor("out", [T, KS], BF16, kind="ExternalOutput")

    # ---- SBUF ----
    xTr = nc.alloc_sbuf_tensor("xTr", [128, ND, T], F16)
    x8s = nc.alloc_sbuf_tensor("x8s", [128, 2, 2, T], F8)
    wkr = nc.alloc_sbuf_tensor("wkr", [128, ND, KS], F16)
    wqr = nc.alloc_sbuf_tensor("wqr", [128, ND, KS], F16)
    wv8 = nc.alloc_sbuf_tensor("wv8s", [128, 2, 2, KS], F8)
    kT = nc.alloc_sbuf_tensor("kT", [128, ND, T], F16)
    qT = nc.alloc_sbuf_tensor("qT", [128, ND, T], F16)
    v_sb = nc.alloc_sbuf_tensor("v_sb", [128, NT, KS], BF16)
    vp8 = nc.alloc_sbuf_tensor("vp8", [128, NP, 2, KS], F8)
    e8s = [
        nc.alloc_sbuf_tensor(f"e8_{m}", [128, 2, T - (1024 if m == 5 else 256 * m)], F8)
        for m in range(NP)
    ]
    outst = nc.alloc_sbuf_tensor("outst", [128, 2, 2, KS], BF16)
    mask16 = nc.alloc_sbuf_tensor("mask16", [128, 512], F16)
    ident16 = nc.alloc_sbuf_tensor("ident16", [128, 128], F16)
    bqc = nc.alloc_sbuf_tensor("bqc", [128, ND], F32)
    bkc = nc.alloc_sbuf_tensor("bkc", [128, ND], F32)
    bvb = nc.alloc_sbuf_tensor("bvb", [128, KS], F32)
    macc = nc.alloc_sbuf_tensor("macc", [128, 16, NCH], F32)
    negmax = nc.alloc_sbuf_tensor("negmax", [128, 16], F32)
    dparts = nc.alloc_sbuf_tensor("dparts", [128, 16, NCH], F32)
    denom = nc.alloc_sbuf_tensor("denom", [128, 16], F32)
    rec = nc.alloc_sbuf_tensor("rec", [128, 16], F32)

    # ---- PSUM: 6-bank logits rotation + 2 read banks ----
    pse = nc.alloc_psum_tensor("pse", [128, 6, 512], F32)
    psRD = nc.alloc_psum_tensor("psRD", [128, 2, 512], F32)
    # phase 1: v -> pse[0..1], k/q -> pse[2..5]

    # ================= static op-index tables =================
    LA = 3  # read lookahead: R(j) sits at fused position j + LA_OF(j)
    LATE_LA = 3

    def LA_OF(j):
        return LA if j < 8 else LATE_LA

    def bank_of_g(g):
        if g < 6:
            return ("E", g)
        if g < 8:
            return ("R", g - 6)
        return ("E", (g - 8) % 6)

    def prev_user_g(g):
        if g < 8:
            return None
        return g - 8 if g < 14 else g - 6

    gbank = {}
    g_of_chunk = {}
    chunk_of_g = {}
    g = 0
    for jt in range(NT):
        for ic in range(_c0(jt), NCH):
            gbank[(jt, ic)] = bank_of_g(g)
            g_of_chunk[(jt, ic)] = g
            chunk_of_g[g] = (jt, ic)
            g += 1

    # rows whose diagonal chunk is computed FULL-width so diag+nondiag form
    # one adjacent-bank pair handled by single wide reduce/exp ops
    FULL = {8, 9}
    # e8 strip base columns (pair 5 starts at row 10's full-width diag)
    BASE = [256 * m for m in range(NP)]
    BASE[5] = 1024

    # non-diag chunk groups per row: maximal runs of adjacent banks
    row_groups = {}
    for jt in range(NT):
        c0 = _c0(jt)
        ics = list(range(c0 + 1, NCH))
        groups = []
        i = 0
        while i < len(ics):
            run = [ics[i]]
            while (
                i + 1 < len(ics)
                and len(run) < 3
                and gbank[(jt, ics[i + 1])][0] == gbank[(jt, run[0])][0]
                and gbank[(jt, ics[i + 1])][1]
                == gbank[(jt, run[0])][1] + len(run)
            ):
                run.append(ics[i + 1])
                i += 1
            groups.append(run)
            i += 1
        row_groups[jt] = groups

    # ---- PE plan (sPE counts GROUPS) ----
    VG, KG, QG, KQSEQ, LG, RG = {}, {}, {}, {}, {}, {}
    pe = 0
    kqseq = 0
    p1_order = []
    for b in range(NCH):
        for r in range(4):
            pe += 1
            KG[(r, b)] = pe
            kqseq += 1
            KQSEQ[("k", r, b)] = kqseq
            p1_order.append(("k", r, b))
            jt = 4 * b + r
            if jt < NT:
                pe += 1
                VG[jt] = pe
                p1_order.append(("v", jt))
        for kt in range(ND):
            pe += 1
            QG[(kt, b)] = pe
            kqseq += 1
            KQSEQ[("q", kt, b)] = kqseq
            p1_order.append(("q", kt, b))
    assert pe == NT + 2 * KQ

    fused_order = []
    for m in range(NT):
        fused_order.append(("L", m))
        for j in range(NT):
            if j + LA_OF(j) == m:
                fused_order.append(("R", j))
    for j in range(NT):
        if j + LA_OF(j) >= NT:
            fused_order.append(("R", j))

    for kind, jt in fused_order:
        if kind == "L":
            for ic in range(_c0(jt), NCH):
                pe += 1
                LG[(jt, ic)] = pe
        else:
            pe += 1
            RG[jt] = pe

    # paired outcopies: pair pm covers read rows (2pm, 2pm+1);
    # even pm -> ACT, odd pm -> DVE; due at fused position 2pm+1+LA
    aocp_due, docp_due = {}, {}
    AOCP_TRAIL, DOCP_TRAIL = [], []
    NPAIRED = 0  # pairs 0..5 cover rows 0..11; rows 12..15 get single OCs
    for pm in range(NPAIRED):
        due = 2 * pm + 1 + LA_OF(2 * pm + 1)
        tgt = aocp_due if pm % 2 == 0 else docp_due
        trail = AOCP_TRAIL if pm % 2 == 0 else DOCP_TRAIL
        if due < NT:
            tgt.setdefault(due, []).append(pm)
        else:
            trail.append(pm)
    # single outcopies for rows 12..15: ACT even rows, DVE odd rows
    aoc1_due, doc1_due = {}, {}
    AOC1_TRAIL, DOC1_TRAIL = [], []
    NSING0 = 2 * NPAIRED  # first single-OC row
    for j in range(NSING0, NT):
        due = j + LA_OF(j)
        tgt = aoc1_due if j % 2 == 0 else doc1_due
        trail = AOC1_TRAIL if j % 2 == 0 else DOC1_TRAIL
        if due < NT:
            tgt.setdefault(due, []).append(j)
        else:
            trail.append(j)

    # ---- ACT plan (sAC): 32 kq copies, then per jt: due paired outcopies,
    #      exps (diag first, then non-diag groups), odd-jt V'-scale ----
    EXPG = {}      # (jt, gi) -> act idx; gi 0 = diag, 1.. = groups
    EXPREL = {}    # (jt, ic) -> act idx of the exp covering the chunk
    EXP_END = {}
    NOPS = {}
    AOCP, VP8A, AOC1 = {}, {}, {}
    ac = 2 * KQ
    for jt in range(NT):
        c0 = _c0(jt)
        for pm in aocp_due.get(jt, []):
            ac += 1
            AOCP[pm] = ac
        if jt in FULL:
            ac += 1
            EXPG[(jt, 0)] = ac  # merged diag+nondiag
            for ic in range(c0, NCH):
                EXPREL[(jt, ic)] = ac
            EXP_END[jt] = ac
            NOPS[jt] = 1
        else:
            ac += 1
            EXPG[(jt, 0)] = ac  # diag
            EXPREL[(jt, c0)] = ac
            gi = 1
            for run in row_groups[jt]:
                ac += 1
                EXPG[(jt, gi)] = ac
                for ic in run:
                    EXPREL[(jt, ic)] = ac
                gi += 1
            EXP_END[jt] = ac
            NOPS[jt] = gi
        if jt >= 1 and (jt - 1) % 2 == 1:
            ac += 1
            VP8A[jt - 1] = ac
        for j in aoc1_due.get(jt, []):
            ac += 1
            AOC1[j] = ac
    if (NT - 1) % 2 == 1:
        ac += 1
        VP8A[NT - 1] = ac
    for pm in AOCP_TRAIL:
        ac += 1
        AOCP[pm] = ac
    for j in AOC1_TRAIL:
        ac += 1
        AOC1[j] = ac

    # ---- DVE plan (sDV): 16 v-copies, then per jt: DRED (diag max from
    #      bank; folded into NMX when nch==1), RED groups, NMX, [DENOM],
    #      RECIP, even-jt V'-scale, due paired outcopies ----
    VCP, DRED, REDG, NMX, DENOM, RECIP, VP8D, DOCP, DOC1 = {}, {}, {}, {}, {}, {}, {}, {}, {}
    dv = 0
    for jt in range(NT):
        dv += 1
        VCP[jt] = dv

    def _dve_tail(jt):
        # denominator chain of row jt, emitted one block later
        nonlocal_dv = []
        return nonlocal_dv

    for jt in range(NT + 1):
        if jt < NT:
            nch = NCH - _c0(jt)
            if jt in FULL:
                dv += 1
                DRED[jt] = dv  # merged 2-bank reduce
            elif nch > 1:
                dv += 1
                DRED[jt] = dv
                for ric in range(_c0(jt) + 1, NCH):
                    dv += 1
                    REDG[(jt, ric)] = dv
            dv += 1
            NMX[jt] = dv
        pj = jt - 1  # previous row's denominator chain
        if 0 <= pj < NT:
            if NOPS[pj] > 1:
                dv += 1
                DENOM[pj] = dv
            dv += 1
            RECIP[pj] = dv
            if pj % 2 == 0:
                dv += 1
                VP8D[pj] = dv
        if jt < NT:
            for pm in docp_due.get(jt, []):
                dv += 1
                DOCP[pm] = dv
            for j in doc1_due.get(jt, []):
                dv += 1
                DOC1[j] = dv
    for pm in DOCP_TRAIL:
        dv += 1
        DOCP[pm] = dv
    for j in DOC1_TRAIL:
        dv += 1
        DOC1[j] = dv

    # ---- Pool plan (sPO): slot-1 memsets only ----
    NMEMSET = 2 * NP

    def st_thr(pm):
        # paired store pm is the (pm+1)-th store
        return 16 * (pm + 1)

    def st1_thr(j):
        # single store for row j is the (NPAIRED + j - NSING0 + 1)-th store
        return 16 * (NPAIRED + j - NSING0 + 1)

    def oc1_st_wait(j):
        # outst slot (s2=(j//2)%2, s1=j%2) previously used by single j-4 or
        # by pair (j-4)//2
        if j - 4 >= NSING0:
            return st1_thr(j - 4)
        return st_thr((j - 4) // 2)

    def bank_ap(coord, w=512):
        t, slot = coord
        if t == "E":
            return pse[:, slot, 0:w]
        return psRD[:, slot, 0:w]

    def bank_run_ap(coord, ln):
        t, slot = coord
        if t == "E":
            return pse[:, slot : slot + ln, :]
        return psRD[:, slot : slot + ln, :]

    def vp8_wait(te_or_none, jt):
        # (sem, thr) releasing vp8(jt)
        if jt % 2 == 0:
            return ("DV", VP8D[jt])
        return ("AC", VP8A[jt])

    with (
        nc.semaphore("sLv") as sLv,
        nc.semaphore("sLk") as sLk,
        nc.semaphore("sLk2") as sLk2,
        nc.semaphore("sLq") as sLq,
        nc.semaphore("sLc") as sLc,
        nc.semaphore("sLm") as sLm,
        nc.semaphore("sLx0") as sLx0,
        nc.semaphore("sLxb") as sLxb,
        nc.semaphore("sLx1") as sLx1,
        nc.semaphore("sLx2") as sLx2,
        nc.semaphore("sLx3") as sLx3,
        nc.semaphore("sL80") as sL80,
        nc.semaphore("sL81") as sL81,
        nc.semaphore("sL82") as sL82,
        nc.semaphore("sL83") as sL83,
        nc.semaphore("sPE") as sPE,
        nc.semaphore("sDV") as sDV,
        nc.semaphore("sAC") as sAC,
        nc.semaphore("sPO") as sPO,
        nc.semaphore("sST") as sST,
        nc.Block() as block,
    ):
        sLxs = [sLx0, sLx1, sLx2, sLx3]
        sL8s = [sL80, sL81, sL82, sL83]

        @block.sync
        def _(sp):
            def ldx16(ic):
                sp.dma_start(
                    out=xTr[:, :, 512 * ic : 512 * (ic + 1)],
                    in_=x16_d.ap()[:, 512 * ic : 512 * (ic + 1)].rearrange(
                        "(t p) i -> p t i", p=128
                    ),
                ).then_inc(sLxs[ic], 16)

            def ldx8(ic):
                sp.dma_start(
                    out=x8s[:, :, :, 512 * ic : 512 * (ic + 1)],
                    in_=x8_d.ap()[:, :, :, 512 * ic : 512 * (ic + 1)],
                ).then_inc(sL8s[ic], 16)

            wk_re = wk_d.ap().rearrange("(t p) k -> p t k", p=128)
            sp.dma_start(out=wkr[:, :, 0:256], in_=wk_re[:, :, 0:256]).then_inc(
                sLk, 16
            )
            x16_re0 = x16_d.ap()[:, 0:512].rearrange("(t p) i -> p t i", p=128)
            sp.dma_start(out=xTr[:, :, 0:256], in_=x16_re0[:, :, 0:256]).then_inc(
                sLx0, 16
            )
            sp.dma_start(out=xTr[:, :, 256:512], in_=x16_re0[:, :, 256:512]).then_inc(
                sLxb, 16
            )
            sp.dma_start(out=wv8[:, :, :, :], in_=wv_d.ap()).then_inc(sLv, 16)
            ldx8(0)
            bv_ap = bv_d.ap()
            bv_bcast = bass.AP(
                tensor=bv_ap.tensor, offset=bv_ap.offset, ap=[[0, 128]] + list(bv_ap.ap)
            )
            sp.dma_start(out=bvb[:, :], in_=bv_bcast).then_inc(sLc, 16)
            with nc.allow_non_contiguous_dma(reason="16B/partition bias loads"):
                sp.dma_start(
                    out=bkc[:, :], in_=bk_d.ap().rearrange("(t p) -> p t", p=128)
                ).then_inc(sLc, 16)
                sp.dma_start(
                    out=bqc[:, :], in_=bq_d.ap().rearrange("(t p) -> p t", p=128)
                ).then_inc(sLc, 16)
            sp.dma_start(out=wkr[:, :, 256:512], in_=wk_re[:, :, 256:512]).then_inc(
                sLk2, 16
            )
            sp.dma_start(
                out=wqr[:, :, :],
                in_=wq_d.ap().rearrange("(t p) k -> p t k", p=128),
            ).then_inc(sLq, 16)
            sp.dma_start(out=mask16[:, :], in_=msk_d.ap()).then_inc(sLm, 16)
            sp.dma_start(out=ident16[:, :], in_=idn_d.ap()).then_inc(sLm, 16)
            ldx16(1)
            ldx8(1)
            ldx16(2)
            ldx8(2)
            ldx16(3)
            ldx8(3)
            # paired stores: 256 output rows each
            out_ap = out_d.ap()
            for pm in range(NPAIRED):
                if pm % 2 == 0:
                    sp.wait_ge(sAC, AOCP[pm])
                else:
                    sp.wait_ge(sDV, DOCP[pm])
                sp.dma_start(
                    out=out_ap[256 * pm : 256 * (pm + 1), :].rearrange(
                        "(s p) c -> p s c", p=128
                    ),
                    in_=outst[:, pm % 2, :, :],
                ).then_inc(sST, 16)
            for j in range(NSING0, NT):
                if j % 2 == 0:
                    sp.wait_ge(sAC, AOC1[j])
                else:
                    sp.wait_ge(sDV, DOC1[j])
                sp.dma_start(
                    out=out_ap[128 * j : 128 * (j + 1), :],
                    in_=outst[:, (j // 2) % 2, j % 2, :],
                ).then_inc(sST, 16)
            sp.wait_ge(sST, 16 * (NPAIRED + NT - NSING0))

        @block.tensor
        def _(te):
            waited = set()

            def ldwait(sem, thr=16):
                if sem not in waited:
                    te.wait_ge(sem, thr)
                    waited.add(sem)

            for item in p1_order:
                if item[0] == "v":
                    jt = item[1]
                    ldwait(sLv)
                    ldwait(sL8s[jt // 4])
                    if jt >= 2:
                        te.wait_ge(sDV, VCP[jt - 2])
                    for dm in range(2):
                        mm = te.matmul(
                            pse[:, jt % 2, :],
                            lhsT=x8s[:, dm, :, 128 * jt : 128 * (jt + 1)],
                            rhs=wv8[:, dm, :, :],
                            start=(dm == 0),
                            stop=(dm == 1),
                            perf_mode=DR,
                        )
                        if dm == 1:
                            mm.then_inc(sPE, 1)
                else:
                    kind, kt, ic = item
                    wsb = wkr if kind == "k" else wqr
                    if kind == "k":
                        ldwait(sLk if kt < 2 else sLk2)
                    else:
                        ldwait(sLq)
                    ldwait(sLxs[ic])
                    if ic == 0 and not (kind == "k" and kt == 0):
                        ldwait(sLxb)
                    seq = KQSEQ[(kind, kt, ic)]
                    if seq > 4:
                        te.wait_ge(sAC, seq - 4)
                    if kind == "k" and ic == 0:
                        # two half-width groups so the first k's start on the
                        # first half-load of x16 strip 0
                        for h in range(2):
                            if h == 1:
                                ldwait(sLxb)
                            for dt_ in range(ND):
                                mm = te.matmul(
                                    pse[:, 2 + ((seq - 1) % 4), 256 * h : 256 * (h + 1)],
                                    lhsT=wsb[:, dt_, 128 * kt : 128 * (kt + 1)],
                                    rhs=xTr[:, dt_, 256 * h : 256 * (h + 1)],
                                    start=(dt_ == 0),
                                    stop=(dt_ == ND - 1),
                                )
                                if h == 1 and dt_ == ND - 1:
                                    mm.then_inc(sPE, 1)
                    else:
                        for dt_ in range(ND):
                            mm = te.matmul(
                                pse[:, 2 + ((seq - 1) % 4), :],
                                lhsT=wsb[:, dt_, 128 * kt : 128 * (kt + 1)],
                                rhs=xTr[:, dt_, 512 * ic : 512 * (ic + 1)],
                                start=(dt_ == 0),
                                stop=(dt_ == ND - 1),
                            )
                            if dt_ == ND - 1:
                                mm.then_inc(sPE, 1)
            # fused: logits chunks (mask matmul appended to diag groups) +
            # DoubleRow read groups
            for kind, jt in fused_order:
                c0 = _c0(jt)
                if kind == "L":
                    for ic in range(c0, NCH):
                        gg = g_of_chunk[(jt, ic)]
                        diag = ic == c0
                        need_ac = KQSEQ[("q", ND - 1, ic)]
                        need_dv = None
                        pg = prev_user_g(gg)
                        if pg is not None:
                            pj, pic = chunk_of_g[pg]
                            need_ac = max(need_ac, EXPREL[(pj, pic)])
                        elif gg < 2:
                            need_dv = VCP[NT - 2 + gg]
                        elif gg < 6:
                            need_ac = max(need_ac, 2 * KQ)
                        te.wait_ge(sAC, need_ac)
                        if need_dv is not None:
                            te.wait_ge(sDV, need_dv)
                        if diag:
                            ldwait(sLm, 32)
                        dlo = 512 * c0 if jt in FULL else 128 * jt
                        w = 512 * (ic + 1) - (dlo if diag else 512 * ic)
                        bank = bank_ap(gbank[(jt, ic)], w)
                        ilo = dlo if diag else 512 * ic
                        for kt in range(ND):
                            mm = te.matmul(
                                bank,
                                lhsT=kT[:, kt, 128 * jt : 128 * (jt + 1)],
                                rhs=qT[:, kt, ilo : 512 * (ic + 1)],
                                start=(kt == 0),
                                stop=(kt == ND - 1) and not diag,
                            )
                            if kt == ND - 1 and not diag:
                                mm.then_inc(sPE, 1)
                        if diag:
                            # causal mask accumulated by the PE: identity
                            # stationary x (0/-60000) f16 pattern, class
                            # slice of the class-3 mask
                            cls = (jt % 4) if jt in FULL else 0
                            te.matmul(
                                bank_ap(gbank[(jt, ic)], 128 * (cls + 1)),
                                lhsT=ident16[:, :],
                                rhs=mask16[:, 512 - 128 * (cls + 1) : 512],
                                start=False,
                                stop=True,
                                skip_group_check=True,
                            ).then_inc(sPE, 1)
                else:
                    npair = (jt + 2) // 2
                    if jt >= 1:
                        sem, thr = vp8_wait(None, jt - 1)
                        te.wait_ge(sPO if sem == "PO" else (sDV if sem == "DV" else sAC), thr)
                    if jt < 2:
                        te.wait_ge(sAC, EXPREL[(1, 2)])
                    elif jt >= NSING0 + 2:
                        if jt % 2 == 0:
                            te.wait_ge(sAC, AOC1[jt - 2])
                        else:
                            te.wait_ge(sDV, DOC1[jt - 2])
                    else:
                        pm = (jt - 2) // 2
                        if pm % 2 == 0:
                            te.wait_ge(sAC, AOCP[pm])
                        else:
                            te.wait_ge(sDV, DOCP[pm])
                    if jt == 0:
                        te.wait_ge(sPO, NMEMSET)
                    for m in range(npair):
                        if m == npair - 1:
                            sem, thr = vp8_wait(None, jt)
                            te.wait_ge(sDV if sem == "DV" else sAC, thr)
                        mm = te.matmul(
                            psRD[:, jt % 2, :],
                            lhsT=e8s[m][
                                :, :, 128 * jt - BASE[m] : 128 * jt - BASE[m] + 128
                            ],
                            rhs=vp8[:, m, :, :],
                            start=(m == 0),
                            stop=(m == npair - 1),
                            perf_mode=DR,
                        )
                        if m == npair - 1:
                            mm.then_inc(sPE, 1)

        @block.vector
        def _(ve):
            ndv = 0

            def inc(x):
                nonlocal ndv
                ndv += 1
                x.then_inc(sDV, 1)

            ve.wait_ge(sLc, 16)
            for jt in range(NT):
                ve.wait_ge(sPE, VG[jt])
                inc(
                    ve.tensor_tensor(
                        out=v_sb[:, jt, :], in0=pse[:, jt % 2, :],
                        in1=bvb[:, :], op=AOP.add,
                    )
                )
                assert ndv == VCP[jt]

            def den_chain(rj):
                # denominator chain of row rj (one block late)
                rp = rj
                ve.wait_ge(sAC, EXP_END[rj])
                if NOPS[rj] > 1:
                    inc(
                        ve.reduce_sum(
                            denom[:, rp : rp + 1],
                            dparts[:, rp, 0 : NOPS[rj]],
                            mybir.AxisListType.X,
                        )
                    )
                    assert ndv == DENOM[rj]
                    ve.wait_ge(sDV, DENOM[rj])  # same-engine RAW fence
                    src = denom[:, rp : rp + 1]
                else:
                    src = dparts[:, rp, 0:1]
                inc(ve.reciprocal(rec[:, rp : rp + 1], src))
                assert ndv == RECIP[rj]
                if rj % 2 == 0:
                    ve.wait_ge(sDV, RECIP[rj])  # same-engine RAW fence
                    inc(
                        ve.tensor_scalar(
                            out=vp8[:, rj // 2, (rj % 2), :],
                            in0=v_sb[:, rj, :],
                            scalar1=rec[:, rp : rp + 1],
                            scalar2=None,
                            op0=AOP.mult,
                        )
                    )
                    assert ndv == VP8D[rj]

            for jt in range(NT):
                c0 = _c0(jt)
                nch = NCH - c0
                w0 = 512 * (c0 + 1) - 128 * jt
                pj = jt  # dedicated per-row softmax state, no reuse
                ve.wait_ge(sPE, LG[(jt, c0)])
                if jt in FULL:
                    ve.wait_ge(sPE, LG[(jt, c0 + 1)])
                    inc(
                        ve.reduce_max(
                            macc[:, pj, c0 : c0 + 2],
                            bank_run_ap(gbank[(jt, c0)], 2),
                            mybir.AxisListType.X,
                        )
                    )
                    assert ndv == DRED[jt]
                    ve.wait_ge(sDV, DRED[jt])
                    inc(
                        ve.reduce_max(
                            negmax[:, pj : pj + 1],
                            macc[:, pj, c0:NCH],
                            mybir.AxisListType.X, negate=True,
                        )
                    )
                elif nch > 1:
                    inc(
                        ve.reduce_max(
                            macc[:, pj, c0 : c0 + 1],
                            bank_ap(gbank[(jt, c0)], w0),
                            mybir.AxisListType.X,
                        )
                    )
                    assert ndv == DRED[jt]
                    for ric in range(c0 + 1, NCH):
                        ve.wait_ge(sPE, LG[(jt, ric)])
                        inc(
                            ve.reduce_max(
                                macc[:, pj, ric : ric + 1],
                                bank_ap(gbank[(jt, ric)], 512),
                                mybir.AxisListType.X,
                            )
                        )
                        assert ndv == REDG[(jt, ric)]
                    ve.wait_ge(sDV, REDG[(jt, NCH - 1)])
                    inc(
                        ve.reduce_max(
                            negmax[:, pj : pj + 1],
                            macc[:, pj, c0:NCH],
                            mybir.AxisListType.X, negate=True,
                        )
                    )
                else:
                    inc(
                        ve.reduce_max(
                            negmax[:, pj : pj + 1],
                            bank_ap(gbank[(jt, c0)], w0),
                            mybir.AxisListType.X, negate=True,
                        )
                    )
                assert ndv == NMX[jt]
                if jt >= 1:
                    den_chain(jt - 1)
                for pm in docp_due.get(jt, []):
                    ve.wait_ge(sPE, RG[2 * pm + 1])
                    if pm >= 2:
                        ve.wait_ge(sST, st_thr(pm - 2))
                    inc(
                        ve.tensor_scalar_add(
                            out=outst[:, pm % 2, :, :], in0=psRD[:, :, :],
                            scalar1=0.0,
                        )
                    )
                    assert ndv == DOCP[pm]
                for j in doc1_due.get(jt, []):
                    ve.wait_ge(sPE, RG[j])
                    if j >= 4:
                        ve.wait_ge(sST, oc1_st_wait(j))
                    inc(
                        ve.tensor_scalar_add(
                            out=outst[:, (j // 2) % 2, j % 2, :],
                            in0=psRD[:, j % 2, :],
                            scalar1=0.0,
                        )
                    )
                    assert ndv == DOC1[j]
            den_chain(NT - 1)
            for pm in DOCP_TRAIL:
                ve.wait_ge(sPE, RG[2 * pm + 1])
                if pm >= 2:
                    ve.wait_ge(sST, st_thr(pm - 2))
                inc(
                    ve.tensor_scalar_add(
                        out=outst[:, pm % 2, :, :], in0=psRD[:, :, :],
                        scalar1=0.0,
                    )
                )
                assert ndv == DOCP[pm]
            for j in DOC1_TRAIL:
                ve.wait_ge(sPE, RG[j])
                if j >= 4:
                    ve.wait_ge(sST, oc1_st_wait(j))
                inc(
                    ve.tensor_scalar_add(
                        out=outst[:, (j // 2) % 2, j % 2, :],
                        in0=psRD[:, j % 2, :],
                        scalar1=0.0,
                    )
                )
                assert ndv == DOC1[j]

        @block.scalar
        def _(ac_):
            ac_.wait_ge(sLc, 48)
            nac = 0
            for ic in range(NCH):
                for wsel, g_tab, bias in ((0, KG, bkc), (1, QG, bqc)):
                    dst = kT if wsel == 0 else qT
                    for kt in range(ND):
                        seq = KQSEQ[("k" if wsel == 0 else "q", kt, ic)]
                        ac_.wait_ge(sPE, g_tab[(kt, ic)])
                        ac_.activation(
                            out=dst[:, kt, 512 * ic : 512 * (ic + 1)],
                            in_=pse[:, 2 + ((seq - 1) % 4), :],
                            func=AFT.Identity,
                            bias=bias[:, kt : kt + 1],
                            scale=1.0,
                        ).then_inc(sAC, 1)
                        nac += 1
            assert nac == 2 * KQ

            def outcopy(pm):
                nonlocal nac
                ac_.wait_ge(sPE, RG[2 * pm + 1])
                if pm >= 2:
                    ac_.wait_ge(sST, st_thr(pm - 2))
                nac += 1
                ac_.activation(
                    out=outst[:, pm % 2, :, :], in_=psRD[:, :, :], func=AFT.Copy
                ).then_inc(sAC, 1)
                assert nac == AOCP[pm]

            def outcopy1(j):
                nonlocal nac
                ac_.wait_ge(sPE, RG[j])
                if j >= 4:
                    ac_.wait_ge(sST, oc1_st_wait(j))
                nac += 1
                ac_.activation(
                    out=outst[:, (j // 2) % 2, j % 2, :],
                    in_=psRD[:, j % 2, :],
                    func=AFT.Copy,
                ).then_inc(sAC, 1)
                assert nac == AOC1[j]

            def act_vp8(rj):
                nonlocal nac
                rp = rj
                ac_.wait_ge(sDV, RECIP[rj])
                nac += 1
                ac_.activation(
                    out=vp8[:, rj // 2, rj % 2, :],
                    in_=v_sb[:, rj, :],
                    func=AFT.Copy,
                    bias=0.0,
                    scale=rec[:, rp : rp + 1],
                ).then_inc(sAC, 1)
                assert nac == VP8A[rj]

            for jt in range(NT):
                c0 = _c0(jt)
                pj = jt  # dedicated per-row softmax state, no reuse
                m = jt // 2
                base = BASE[m]
                for pm in aocp_due.get(jt, []):
                    outcopy(pm)
                ac_.wait_ge(sDV, NMX[jt])
                if jt in FULL:
                    # one exp over the adjacent diag+nondiag bank pair
                    nac += 1
                    ac_.activation(
                        out=e8s[m][
                            :, jt % 2, 512 * c0 - base : 512 * (c0 + 2) - base
                        ],
                        in_=bank_run_ap(gbank[(jt, c0)], 2),
                        func=AFT.Exp,
                        bias=negmax[:, pj : pj + 1],
                        scale=1.0,
                        accum_out=dparts[:, pj, 0:1],
                    ).then_inc(sAC, 1)
                    assert nac == EXPG[(jt, 0)]
                else:
                    # exps: diag first (frees the rotation bank soonest)
                    w0 = 512 * (c0 + 1) - 128 * jt
                    nac += 1
                    ac_.activation(
                        out=e8s[m][
                            :, jt % 2, 128 * jt - base : 512 * (c0 + 1) - base
                        ],
                        in_=bank_ap(gbank[(jt, c0)], w0),
                        func=AFT.Exp,
                        bias=negmax[:, pj : pj + 1],
                        scale=1.0,
                        accum_out=dparts[:, pj, 0:1],
                    ).then_inc(sAC, 1)
                    assert nac == EXPG[(jt, 0)]
                    sidx = 1
                    for run in row_groups[jt]:
                        a = 512 * run[0]
                        bcol = 512 * (run[-1] + 1)
                        nac += 1
                        ac_.activation(
                            out=e8s[m][:, jt % 2, a - base : bcol - base],
                            in_=bank_run_ap(gbank[(jt, run[0])], len(run)),
                            func=AFT.Exp,
                            bias=negmax[:, pj : pj + 1],
                            scale=1.0,
                            accum_out=dparts[:, pj, sidx : sidx + 1],
                        ).then_inc(sAC, 1)
                        assert nac == EXPG[(jt, sidx)]
                        sidx += 1
                    assert sidx == NOPS[jt]
                if jt >= 1 and (jt - 1) % 2 == 1:
                    act_vp8(jt - 1)
                for j in aoc1_due.get(jt, []):
                    outcopy1(j)
            if (NT - 1) % 2 == 1:
                act_vp8(NT - 1)
            for pm in AOCP_TRAIL:
                outcopy(pm)
            for j in AOC1_TRAIL:
                outcopy1(j)

        @block.gpsimd
        def _(po):
            npo = 0
            for m in range(NP):
                zw = 384 if m == 5 else 128
                po.memset(e8s[m][:, 1, 0:zw], 0.0).then_inc(sPO, 1)
                npo += 1
                po.memset(vp8[:, m, 1, :], 0.0).then_inc(sPO, 1)
                npo += 1
            assert npo == NMEMSET

    nc.finalize()
    return nc


def _host_inputs(xb, wq16, wk16, wv8h, bq, bk, bv, T):
    # mask class 3: cols x < 384 + p get MASKVAL; class c uses the slice
    # [512-128*(c+1) : 512]
    p = np.arange(128, dtype=np.float32)[:, None]
    xx = np.arange(512, dtype=np.float32)[None, :]
    msk = np.where(xx >= 384.0 + p, 0.0, MASKVAL).astype(np.float16)
    idn = np.eye(128, dtype=np.float16)

    xT = np.ascontiguousarray(xb.T)  # [D, T] f32
    x16 = xT.astype(np.float16)
    x8 = np.ascontiguousarray(
        xT.reshape(2, 2, 128, T).transpose(2, 0, 1, 3)
    ).astype(ml_dtypes.float8_e4m3fn)
    return dict(
        x16=x16,
        x8=np.ascontiguousarray(x8),
        wq16=wq16,
        wk16=wk16,
        wv8=wv8h,
        bq=bq,
        bk=bk,
        bv=bv,
        msk16=np.ascontiguousarray(msk),
        idn16=np.ascontiguousarray(idn),
    )


def kernel(x, Wk, bk, Wq, bq, Wv, bv):
    global LAST_RESULTS
    T = 2048
    x = np.ascontiguousarray(np.asarray(x, dtype=np.float32))
    Wk = np.asarray(Wk, dtype=np.float32)
    Wq = np.asarray(Wq, dtype=np.float32)
    Wv = np.asarray(Wv, dtype=np.float32)
    bk = np.ascontiguousarray(np.asarray(bk, dtype=np.float32))
    bq = np.ascontiguousarray(np.asarray(bq, dtype=np.float32))
    bv = np.ascontiguousarray(np.asarray(bv, dtype=np.float32))

    wq16 = np.ascontiguousarray(Wq.T).astype(np.float16)
    wk16 = np.ascontiguousarray(Wk.T).astype(np.float16)
    wvT = Wv.T * np.float32(INV_SQRT_K)  # [D, KS]
    wv8h = np.ascontiguousarray(
        wvT.reshape(2, 2, 128, KS).transpose(2, 0, 1, 3)
    ).astype(ml_dtypes.float8_e4m3fn)
    bvs = np.ascontiguousarray(bv * np.float32(INV_SQRT_K))

    nc = build_nc(T)
    in_maps = [_host_inputs(x[b], wq16, wk16, wv8h, bq, bk, bvs, T) for b in range(B)]
    res = None
    read = None
    last_exc = None
    for attempt in range(4):
        try:
            res = run_bass_kernel_spmd(nc, in_maps, list(range(B)), trace=TRACE)
            read = np.stack(
                [
                    np.asarray(res.results[b]["out"]).astype(np.float32)
                    for b in range(B)
                ],
                axis=0,
            )
            if np.isfinite(read).all():
                break
            read = None  # flaky device run produced NaN/inf; retry
        except Exception as e:  # transient NRT device errors; retry fresh
            last_exc = e
        import time as _time
        _time.sleep(5)
        nc = build_nc(T)
    if read is None:
        if last_exc is not None:
            raise last_exc
        raise RuntimeError("device produced non-finite output on all attempts")
    LAST_RESULTS = res
    return (x + read).astype(np.float32)


# revision 11
# speedup vs baseline: 1.7221x; 1.0007x over previous
"""Trainium2 Bass kernel for nn_AttentionBlock (causal attn, softmax over the
QUERY axis (dim=1), post-softmax 1/sqrt(K) scale, residual add).

Sharding: data-parallel over batch B=8, one batch element per NeuronCore.

v4 design:
- K/Q projections + logits in fp16; V projection and the probability-weighted
  read in fp8e4 with MatmulPerfMode.DoubleRow (two 128-deep contraction tiles
  per matmul).
- The causal mask is applied BY THE PE: each diagonal logits group gets one
  extra 128-wide matmul (identity stationary x f16 mask pattern of 0/-60000)
  accumulated into the PSUM bank.  exp() of -60000-ish underflows to exactly
  0.  No DVE mask-add, no staging: every chunk is max-reduced and exp'd
  straight from its PSUM bank.
- Logits PSUM banks live in ONE 6-bank tensor (pse) plus a 2-bank read tensor
  (psRD).  The first 8 logits chunks use all 8 banks (prologue), then a
  6-bank rotation.  Adjacent-bank non-diag chunk runs are processed by SINGLE
  wide ops: one DVE reduce yields 2-3 column maxes, one ACT exp covers 2-3
  chunks (one f32 accum partial per op).
- Denominator combine + reciprocal on DVE; V'-scales split DVE (even jt) /
  ACT (odd jt); output evacuation is PAIRED: one op copies both read banks
  [128,2,512] -> bf16 outst, one DMA stores 256 output rows; pairs alternate
  ACT/DVE.  Pool does only the startup memsets.
- E8[j,i] strips live in PAIRED key-chunk layout e8s[m][:, slot, :]
  (slot = jt%2, strip base column 256m) feeding DoubleRow reads directly;
  slot-1's first 128 columns are memset 0 (sub-diagonal).
    read[i, :] = sum_j E[j, i] * V'[j, :],  V' = (v + bv) * rec_j / sqrt(K)

Raw Block style with manual semaphores: ONE embedded sync-wait per
instruction; cross-engine deps are standalone wait_ge with statically
computed thresholds; same-engine RAW pairs get explicit fences.
"""

import math
import os
import sys

import numpy as np
import ml_dtypes

for _p in ("/opt/trn_rl_repo", "/root/.axon_site/_ro/trn_rl_repo"):
    if os.path.isdir(_p) and _p not in sys.path:
        sys.path.append(_p)

import concourse.bass as bass
from concourse import mybir
from concourse.bass_utils import run_bass_kernel_spmd

B = 8
D = 512
KS = 512
ND = D // 128  # 4 contraction tiles

F32 = mybir.dt.float32
F16 = mybir.dt.float16
BF16 = mybir.dt.bfloat16
F8 = mybir.dt.float8e4
AOP = mybir.AluOpType
AFT = mybir.ActivationFunctionType
DR = mybir.MatmulPerfMode.DoubleRow

INV_SQRT_K = 1.0 / math.sqrt(KS)
MASKVAL = -60000.0  # fits f16; exp(-60000 + max) == 0 exactly

TRACE = False
LAST_RESULTS = None


def _c0(jt):
    return (128 * jt) // 512


def build_nc(T=2048, debug_dump=False):
    NT = T // 128   # 16 row chunks
    NCH = T // 512  # 4 column chunks
    NP = NT // 2    # 8 key-chunk pairs
    KQ = ND * NCH   # 16 projection output groups for each of q/k

    nc = bass.Bass("TRN2", target_bir_lowering=False, debug=False, num_devices=B)

    # ---- DRAM ----
    x16_d = nc.dram_tensor("x16", [D, T], F16, kind="ExternalInput")
    x8_d = nc.dram_tensor("x8", [128, 2, 2, T], F8, kind="ExternalInput")
    wq_d = nc.dram_tensor("wq16", [D, KS], F16, kind="ExternalInput")
    wk_d = nc.dram_tensor("wk16", [D, KS], F16, kind="ExternalInput")
    wv_d = nc.dram_tensor("wv8", [128, 2, 2, KS], F8, kind="ExternalInput")
    bq_d = nc.dram_tensor("bq", [KS], F32, kind="ExternalInput")
    bk_d = nc.dram_tensor("bk", [KS], F32, kind="ExternalInput")
    bv_d = nc.dram_tensor("bv", [KS], F32, kind="ExternalInput")
    msk_d = nc.dram_tensor("msk16", [128, 512], F16, kind="ExternalInput")
    idn_d = nc.dram_tensor("idn16", [128, 128], F16, kind="ExternalInput")
    out_d = nc.dram_tensor("out", [T, KS], BF16, kind="ExternalOutput")

    # ---- SBUF ----
    xTr = nc.alloc_sbuf_tensor("xTr", [128, ND, T], F16)
    x8s = nc.alloc_sbuf_tensor("x8s", [128, 2, 2, T], F8)
    wkr = nc.alloc_sbuf_tensor("wkr", [128, ND, KS], F16)
    wqr = nc.alloc_sbuf_tensor("wqr", [128, ND, KS], F16)
    wv8 = nc.alloc_sbuf_tensor("wv8s", [128, 2, 2, KS], F8)
    kT = nc.alloc_sbuf_tensor("kT", [128, ND, T], F16)
    qT = nc.alloc_sbuf_tensor("qT", [128, ND, T], F16)
    v_sb = nc.alloc_sbuf_tensor("v_sb", [128, NT, KS], BF16)
    vp8 = nc.alloc_sbuf_tensor("vp8", [128, NP, 2, KS], F8)
    e8s = [
        nc.alloc_sbuf_tensor(f"e8_{m}", [128, 2, T - (1024 if m == 5 else 256 * m)], F8)
        for m in range(NP)
    ]
    outst = nc.alloc_sbuf_tensor("outst", [128, 2, 2, KS], BF16)
    mask16 = nc.alloc_sbuf_tensor("mask16", [128, 512], F16)
    ident16 = nc.alloc_sbuf_tensor("ident16", [128, 128], F16)
    bqc = nc.alloc_sbuf_tensor("bqc", [128, ND], F32)
    bkc = nc.alloc_sbuf_tensor("bkc", [128, ND], F32)
    bvb = nc.alloc_sbuf_tensor("bvb", [128, KS], F32)
    macc = nc.alloc_sbuf_tensor("macc", [128, 16, NCH], F32)
    negmax = nc.alloc_sbuf_tensor("negmax", [128, 16], F32)
    dparts = nc.alloc_sbuf_tensor("dparts", [128, 16, NCH], F32)
    denom = nc.alloc_sbuf_tensor("denom", [128, 16], F32)
    rec = nc.alloc_sbuf_tensor("rec", [128, 16], F32)

    # ---- PSUM: 6-bank logits rotation + 2 read banks ----
    pse = nc.alloc_psum_tensor("pse", [128, 6, 512], F32)
    psRD = nc.alloc_psum_tensor("psRD", [128, 2, 512], F32)
    # phase 1: v -> pse[0..1], k/q -> pse[2..5]

    # ================= static op-index tables =================
    LA = 3  # read lookahead: R(j) sits at fused position j + LA_OF(j)
    LATE_LA = 3

    def LA_OF(j):
        return LA if j < 8 else LATE_LA

    def bank_of_g(g):
        if g < 6:
            return ("E", g)
        if g < 8:
            return ("R", g - 6)
        return ("E", (g - 8) % 6)

    def prev_user_g(g):
        if g < 8:
            return None
        return g - 8 if g < 14 else g - 6

    gbank = {}
    g_of_chunk = {}
    chunk_of_g = {}
    g = 0
    for jt in range(NT):
        for ic in range(_c0(jt), NCH):
            gbank[(jt, ic)] = bank_of_g(g)
            g_of_chunk[(jt, ic)] = g
            chunk_of_g[g] = (jt, ic)
            g += 1

    # rows whose diagonal chunk is computed FULL-width so diag+nondiag form
    # one adjacent-bank pair handled by single wide reduce/exp ops
    FULL = {8, 9}
    # e8 strip base columns (pair 5 starts at row 10's full-width diag)
    BASE = [256 * m for m in range(NP)]
    BASE[5] = 1024

    # non-diag chunk groups per row: maximal runs of adjacent banks
    row_groups = {}
    for jt in range(NT):
        c0 = _c0(jt)
        ics = list(range(c0 + 1, NCH))
        groups = []
        i = 0
        while i < len(ics):
            run = [ics[i]]
            while (
                i + 1 < len(ics)
                and len(run) < 3
                and gbank[(jt, ics[i + 1])][0] == gbank[(jt, run[0])][0]
                and gbank[(jt, ics[i + 1])][1]
                == gbank[(jt, run[0])][1] + len(run)
            ):
                run.append(ics[i + 1])
                i += 1
            groups.append(run)
            i += 1
        row_groups[jt] = groups

    # ---- PE plan (sPE counts GROUPS) ----
    VG, KG, QG, KQSEQ, LG, RG = {}, {}, {}, {}, {}, {}
    pe = 0
    kqseq = 0
    p1_order = []
    for b in range(NCH):
        for r in range(4):
            pe += 1
            KG[(r, b)] = pe
            kqseq += 1
            KQSEQ[("k", r, b)] = kqseq
            p1_order.append(("k", r, b))
            jt = 4 * b + r
            if jt < NT:
                pe += 1
                VG[jt] = pe
                p1_order.append(("v", jt))
        for kt in range(ND):
            pe += 1
            QG[(kt, b)] = pe
            kqseq += 1
            KQSEQ[("q", kt, b)] = kqseq
            p1_order.append(("q", kt, b))
    assert pe == NT + 2 * KQ

    fused_order = []
    for m in range(NT):
        fused_order.append(("L", m))
        for j in range(NT):
            if j + LA_OF(j) == m:
                fused_order.append(("R", j))
    for j in range(NT):
        if j + LA_OF(j) >= NT:
            fused_order.append(("R", j))

    for kind, jt in fused_order:
        if kind == "L":
            for ic in range(_c0(jt), NCH):
                pe += 1
                LG[(jt, ic)] = pe
        else:
            pe += 1
            RG[jt] = pe

    # paired outcopies: pair pm covers read rows (2pm, 2pm+1);
    # even pm -> ACT, odd pm -> DVE; due at fused position 2pm+1+LA
    aocp_due, docp_due = {}, {}
    AOCP_TRAIL, DOCP_TRAIL = [], []
    NPAIRED = 0  # pairs 0..5 cover rows 0..11; rows 12..15 get single OCs
    for pm in range(NPAIRED):
        due = 2 * pm + 1 + LA_OF(2 * pm + 1)
        tgt = aocp_due if pm % 2 == 0 else docp_due
        trail = AOCP_TRAIL if pm % 2 == 0 else DOCP_TRAIL
        if due < NT:
            tgt.setdefault(due, []).append(pm)
        else:
            trail.append(pm)
    # single outcopies for rows 12..15: ACT even rows, DVE odd rows
    aoc1_due, doc1_due = {}, {}
    AOC1_TRAIL, DOC1_TRAIL = [], []
    NSING0 = 2 * NPAIRED  # first single-OC row
    for j in range(NSING0, NT):
        due = j + LA_OF(j)
        tgt = aoc1_due if j % 2 == 0 else doc1_due
        trail = AOC1_TRAIL if j % 2 == 0 else DOC1_TRAIL
        if due < NT:
            tgt.setdefault(due, []).append(j)
        else:
            trail.append(j)

    # ---- ACT plan (sAC): 32 kq copies, then per jt: due paired outcopies,
    #      exps (diag first, then non-diag groups), odd-jt V'-scale ----
    EXPG = {}      # (jt, gi) -> act idx; gi 0 = diag, 1.. = groups
    EXPREL = {}    # (jt, ic) -> act idx of the exp covering the chunk
    EXP_END = {}
    NOPS = {}
    AOCP, VP8A, AOC1 = {}, {}, {}
    ac = 2 * KQ
    for jt in range(NT):
        c0 = _c0(jt)
        for pm in aocp_due.get(jt, []):
            ac += 1
            AOCP[pm] = ac
        if jt in FULL:
            ac += 1
            EXPG[(jt, 0)] = ac  # merged diag+nondiag
            for ic in range(c0, NCH):
                EXPREL[(jt, ic)] = ac
            EXP_END[jt] = ac
            NOPS[jt] = 1
        else:
            ac += 1
            EXPG[(jt, 0)] = ac  # diag
            EXPREL[(jt, c0)] = ac
            gi = 1
            for run in row_groups[jt]:
                ac += 1
                EXPG[(jt, gi)] = ac
                for ic in run:
                    EXPREL[(jt, ic)] = ac
                gi += 1
            EXP_END[jt] = ac
            NOPS[jt] = gi
        if jt >= 1 and (jt - 1) % 2 == 1:
            ac += 1
            VP8A[jt - 1] = ac
        for j in aoc1_due.get(jt, []):
            ac += 1
            AOC1[j] = ac
    if (NT - 1) % 2 == 1:
        ac += 1
        VP8A[NT - 1] = ac
    for pm in AOCP_TRAIL:
        ac += 1
        AOCP[pm] = ac
    for j in AOC1_TRAIL:
        ac += 1
        AOC1[j] = ac

    # ---- DVE plan (sDV): 16 v-copies, then per jt: DRED (diag max from
    #      bank; folded into NMX when nch==1), RED groups, NMX, [DENOM],
    #      RECIP, even-jt V'-scale, due paired outcopies ----
    VCP, DRED, REDG, NMX, DENOM, RECIP, VP8D, DOCP, DOC1 = {}, {}, {}, {}, {}, {}, {}, {}, {}
    dv = 0
    for jt in range(NT):
        dv += 1
        VCP[jt] = dv

    def _dve_tail(jt):
        # denominator chain of row jt, emitted one block later
        nonlocal_dv = []
        return nonlocal_dv

    for jt in range(NT + 1):
        if jt < NT:
            nch = NCH - _c0(jt)
            if jt in FULL:
                dv += 1
                DRED[jt] = dv  # merged 2-bank reduce
            elif nch > 1:
                dv += 1
                DRED[jt] = dv
                for ric in range(_c0(jt) + 1, NCH):
                    dv += 1
                    REDG[(jt, ric)] = dv
            dv += 1
            NMX[jt] = dv
        if jt < NT:
            for j in doc1_due.get(jt, []):
                dv += 1
                DOC1[j] = dv
        pj = jt - 1  # previous row's denominator chain
        if 0 <= pj < NT:
            if NOPS[pj] > 1:
                dv += 1
                DENOM[pj] = dv
            dv += 1
            RECIP[pj] = dv
            if pj % 2 == 0:
                dv += 1
                VP8D[pj] = dv
        if jt < NT:
            for pm in docp_due.get(jt, []):
                dv += 1
                DOCP[pm] = dv
    for pm in DOCP_TRAIL:
        dv += 1
        DOCP[pm] = dv
    for j in DOC1_TRAIL:
        dv += 1
        DOC1[j] = dv

    # ---- Pool plan (sPO): slot-1 memsets only ----
    NMEMSET = 2 * NP

    def st_thr(pm):
        # paired store pm is the (pm+1)-th store
        return 16 * (pm + 1)

    def st1_thr(j):
        # single store for row j is the (NPAIRED + j - NSING0 + 1)-th store
        return 16 * (NPAIRED + j - NSING0 + 1)

    def oc1_st_wait(j):
        # outst slot (s2=(j//2)%2, s1=j%2) previously used by single j-4 or
        # by pair (j-4)//2
        if j - 4 >= NSING0:
            return st1_thr(j - 4)
        return st_thr((j - 4) // 2)

    def bank_ap(coord, w=512):
        t, slot = coord
        if t == "E":
            return pse[:, slot, 0:w]
        return psRD[:, slot, 0:w]

    def bank_run_ap(coord, ln):
        t, slot = coord
        if t == "E":
            return pse[:, slot : slot + ln, :]
        return psRD[:, slot : slot + ln, :]

    def vp8_wait(te_or_none, jt):
        # (sem, thr) releasing vp8(jt)
        if jt % 2 == 0:
            return ("DV", VP8D[jt])
        return ("AC", VP8A[jt])

    with (
        nc.semaphore("sLv") as sLv,
        nc.semaphore("sLk") as sLk,
        nc.semaphore("sLk2") as sLk2,
        nc.semaphore("sLq") as sLq,
        nc.semaphore("sLc") as sLc,
        nc.semaphore("sLm") as sLm,
        nc.semaphore("sLx0") as sLx0,
        nc.semaphore("sLxb") as sLxb,
        nc.semaphore("sLx1") as sLx1,
        nc.semaphore("sLx2") as sLx2,
        nc.semaphore("sLx3") as sLx3,
        nc.semaphore("sL80") as sL80,
        nc.semaphore("sL81") as sL81,
        nc.semaphore("sL82") as sL82,
        nc.semaphore("sL83") as sL83,
        nc.semaphore("sPE") as sPE,
        nc.semaphore("sDV") as sDV,
        nc.semaphore("sAC") as sAC,
        nc.semaphore("sPO") as sPO,
        nc.semaphore("sST") as sST,
        nc.Block() as block,
    ):
        sLxs = [sLx0, sLx1, sLx2, sLx3]
        sL8s = [sL80, sL81, sL82, sL83]

        @block.sync
        def _(sp):
            def ldx16(ic):
                sp.dma_start(
                    out=xTr[:, :, 512 * ic : 512 * (ic + 1)],
                    in_=x16_d.ap()[:, 512 * ic : 512 * (ic + 1)].rearrange(
                        "(t p) i -> p t i", p=128
                    ),
                ).then_inc(sLxs[ic], 16)

            def ldx8(ic):
                sp.dma_start(
                    out=x8s[:, :, :, 512 * ic : 512 * (ic + 1)],
                    in_=x8_d.ap()[:, :, :, 512 * ic : 512 * (ic + 1)],
                ).then_inc(sL8s[ic], 16)

            wk_re = wk_d.ap().rearrange("(t p) k -> p t k", p=128)
            sp.dma_start(out=wkr[:, :, 0:256], in_=wk_re[:, :, 0:256]).then_inc(
                sLk, 16
            )
            x16_re0 = x16_d.ap()[:, 0:512].rearrange("(t p) i -> p t i", p=128)
            sp.dma_start(out=xTr[:, :, 0:256], in_=x16_re0[:, :, 0:256]).then_inc(
                sLx0, 16
            )
            sp.dma_start(out=xTr[:, :, 256:512], in_=x16_re0[:, :, 256:512]).then_inc(
                sLxb, 16
            )
            sp.dma_start(out=wv8[:, :, :, :], in_=wv_d.ap()).then_inc(sLv, 16)
            ldx8(0)
            sp.dma_start(out=wkr[:, :, 256:512], in_=wk_re[:, :, 256:512]).then_inc(
                sLk2, 16
            )
            bv_ap = bv_d.ap()
            bv_bcast = bass.AP(
                tensor=bv_ap.tensor, offset=bv_ap.offset, ap=[[0, 128]] + list(bv_ap.ap)
            )
            sp.dma_start(out=bvb[:, :], in_=bv_bcast).then_inc(sLc, 16)
            with nc.allow_non_contiguous_dma(reason="16B/partition bias loads"):
                sp.dma_start(
                    out=bkc[:, :], in_=bk_d.ap().rearrange("(t p) -> p t", p=128)
                ).then_inc(sLc, 16)
                sp.dma_start(
                    out=bqc[:, :], in_=bq_d.ap().rearrange("(t p) -> p t", p=128)
                ).then_inc(sLc, 16)
            sp.dma_start(
                out=wqr[:, :, :],
                in_=wq_d.ap().rearrange("(t p) k -> p t k", p=128),
            ).then_inc(sLq, 16)
            sp.dma_start(out=mask16[:, :], in_=msk_d.ap()).then_inc(sLm, 16)
            sp.dma_start(out=ident16[:, :], in_=idn_d.ap()).then_inc(sLm, 16)
            ldx16(1)
            ldx8(1)
            ldx16(2)
            ldx8(2)
            ldx16(3)
            ldx8(3)
            # paired stores: 256 output rows each
            out_ap = out_d.ap()
            for pm in range(NPAIRED):
                if pm % 2 == 0:
                    sp.wait_ge(sAC, AOCP[pm])
                else:
                    sp.wait_ge(sDV, DOCP[pm])
                sp.dma_start(
                    out=out_ap[256 * pm : 256 * (pm + 1), :].rearrange(
                        "(s p) c -> p s c", p=128
                    ),
                    in_=outst[:, pm % 2, :, :],
                ).then_inc(sST, 16)
            for j in range(NSING0, NT):
                if j % 2 == 0:
                    sp.wait_ge(sAC, AOC1[j])
                else:
                    sp.wait_ge(sDV, DOC1[j])
                sp.dma_start(
                    out=out_ap[128 * j : 128 * (j + 1), :],
                    in_=outst[:, (j // 2) % 2, j % 2, :],
                ).then_inc(sST, 16)
            sp.wait_ge(sST, 16 * (NPAIRED + NT - NSING0))

        @block.tensor
        def _(te):
            waited = set()

            def ldwait(sem, thr=16):
                if sem not in waited:
                    te.wait_ge(sem, thr)
                    waited.add(sem)

            for item in p1_order:
                if item[0] == "v":
                    jt = item[1]
                    ldwait(sLv)
                    ldwait(sL8s[jt // 4])
                    if jt >= 2:
                        te.wait_ge(sDV, VCP[jt - 2])
                    for dm in range(2):
                        mm = te.matmul(
                            pse[:, jt % 2, :],
                            lhsT=x8s[:, dm, :, 128 * jt : 128 * (jt + 1)],
                            rhs=wv8[:, dm, :, :],
                            start=(dm == 0),
                            stop=(dm == 1),
                            perf_mode=DR,
                        )
                        if dm == 1:
                            mm.then_inc(sPE, 1)
                else:
                    kind, kt, ic = item
                    wsb = wkr if kind == "k" else wqr
                    if kind == "k":
                        ldwait(sLk if kt < 2 else sLk2)
                    else:
                        ldwait(sLq)
                    ldwait(sLxs[ic])
                    if ic == 0 and not (kind == "k" and kt == 0):
                        ldwait(sLxb)
                    seq = KQSEQ[(kind, kt, ic)]
                    if seq > 4:
                        te.wait_ge(sAC, seq - 4)
                    if kind == "k" and ic == 0:
                        # two half-width groups so the first k's start on the
                        # first half-load of x16 strip 0
                        for h in range(2):
                            if h == 1:
                                ldwait(sLxb)
                            for dt_ in range(ND):
                                mm = te.matmul(
                                    pse[:, 2 + ((seq - 1) % 4), 256 * h : 256 * (h + 1)],
                                    lhsT=wsb[:, dt_, 128 * kt : 128 * (kt + 1)],
                                    rhs=xTr[:, dt_, 256 * h : 256 * (h + 1)],
                                    start=(dt_ == 0),
                                    stop=(dt_ == ND - 1),
                                )
                                if h == 1 and dt_ == ND - 1:
                                    mm.then_inc(sPE, 1)
                    else:
                        for dt_ in range(ND):
                            mm = te.matmul(
                                pse[:, 2 + ((seq - 1) % 4), :],
                                lhsT=wsb[:, dt_, 128 * kt : 128 * (kt + 1)],
                                rhs=xTr[:, dt_, 512 * ic : 512 * (ic + 1)],
                                start=(dt_ == 0),
                                stop=(dt_ == ND - 1),
                            )
                            if dt_ == ND - 1:
                                mm.then_inc(sPE, 1)
            # fused: logits chunks (mask matmul appended to diag groups) +
            # DoubleRow read groups
            for kind, jt in fused_order:
                c0 = _c0(jt)
                if kind == "L":
                    for ic in range(c0, NCH):
                        gg = g_of_chunk[(jt, ic)]
                        diag = ic == c0
                        need_ac = KQSEQ[("q", ND - 1, ic)]
                        need_dv = None
                        pg = prev_user_g(gg)
                        if pg is not None:
                            pj, pic = chunk_of_g[pg]
                            need_ac = max(need_ac, EXPREL[(pj, pic)])
                        elif gg < 2:
                            need_dv = VCP[NT - 2 + gg]
                        elif gg < 6:
                            need_ac = max(need_ac, 2 * KQ)
                        te.wait_ge(sAC, need_ac)
                        if need_dv is not None:
                            te.wait_ge(sDV, need_dv)
                        if diag:
                            ldwait(sLm, 32)
                        dlo = 512 * c0 if jt in FULL else 128 * jt
                        w = 512 * (ic + 1) - (dlo if diag else 512 * ic)
                        bank = bank_ap(gbank[(jt, ic)], w)
                        ilo = dlo if diag else 512 * ic
                        for kt in range(ND):
                            mm = te.matmul(
                                bank,
                                lhsT=kT[:, kt, 128 * jt : 128 * (jt + 1)],
                                rhs=qT[:, kt, ilo : 512 * (ic + 1)],
                                start=(kt == 0),
                                stop=(kt == ND - 1) and not diag,
                            )
                            if kt == ND - 1 and not diag:
                                mm.then_inc(sPE, 1)
                        if diag:
                            # causal mask accumulated by the PE: identity
                            # stationary x (0/-60000) f16 pattern, class
                            # slice of the class-3 mask
                            cls = (jt % 4) if jt in FULL else 0
                            te.matmul(
                                bank_ap(gbank[(jt, ic)], 128 * (cls + 1)),
                                lhsT=ident16[:, :],
                                rhs=mask16[:, 512 - 128 * (cls + 1) : 512],
                                start=False,
                                stop=True,
                                skip_group_check=True,
                            ).then_inc(sPE, 1)
                else:
                    npair = (jt + 2) // 2
                    if jt >= 1:
                        sem, thr = vp8_wait(None, jt - 1)
                        te.wait_ge(sPO if sem == "PO" else (sDV if sem == "DV" else sAC), thr)
                    if jt < 2:
                        te.wait_ge(sAC, EXPREL[(1, 2)])
                    elif jt >= NSING0 + 2:
                        if jt % 2 == 0:
                            te.wait_ge(sAC, AOC1[jt - 2])
                        else:
                            te.wait_ge(sDV, DOC1[jt - 2])
                    else:
                        pm = (jt - 2) // 2
                        if pm % 2 == 0:
                            te.wait_ge(sAC, AOCP[pm])
                        else:
                            te.wait_ge(sDV, DOCP[pm])
                    if jt == 0:
                        te.wait_ge(sPO, NMEMSET)
                    for m in range(npair):
                        if m == npair - 1:
                            sem, thr = vp8_wait(None, jt)
                            te.wait_ge(sDV if sem == "DV" else sAC, thr)
                        mm = te.matmul(
                            psRD[:, jt % 2, :],
                            lhsT=e8s[m][
                                :, :, 128 * jt - BASE[m] : 128 * jt - BASE[m] + 128
                            ],
                            rhs=vp8[:, m, :, :],
                            start=(m == 0),
                            stop=(m == npair - 1),
                            perf_mode=DR,
                        )
                        if m == npair - 1:
                            mm.then_inc(sPE, 1)

        @block.vector
        def _(ve):
            ndv = 0

            def inc(x):
                nonlocal ndv
                ndv += 1
                x.then_inc(sDV, 1)

            ve.wait_ge(sLc, 16)
            for jt in range(NT):
                ve.wait_ge(sPE, VG[jt])
                inc(
                    ve.tensor_tensor(
                        out=v_sb[:, jt, :], in0=pse[:, jt % 2, :],
                        in1=bvb[:, :], op=AOP.add,
                    )
                )
                assert ndv == VCP[jt]

            def den_chain(rj):
                # denominator chain of row rj (one block late)
                rp = rj
                ve.wait_ge(sAC, EXP_END[rj])
                if NOPS[rj] > 1:
                    inc(
                        ve.reduce_sum(
                            denom[:, rp : rp + 1],
                            dparts[:, rp, 0 : NOPS[rj]],
                            mybir.AxisListType.X,
                        )
                    )
                    assert ndv == DENOM[rj]
                    ve.wait_ge(sDV, DENOM[rj])  # same-engine RAW fence
                    src = denom[:, rp : rp + 1]
                else:
                    src = dparts[:, rp, 0:1]
                inc(ve.reciprocal(rec[:, rp : rp + 1], src))
                assert ndv == RECIP[rj]
                if rj % 2 == 0:
                    ve.wait_ge(sDV, RECIP[rj])  # same-engine RAW fence
                    inc(
                        ve.tensor_scalar(
                            out=vp8[:, rj // 2, (rj % 2), :],
                            in0=v_sb[:, rj, :],
                            scalar1=rec[:, rp : rp + 1],
                            scalar2=None,
                            op0=AOP.mult,
                        )
                    )
                    assert ndv == VP8D[rj]

            for jt in range(NT):
                c0 = _c0(jt)
                nch = NCH - c0
                w0 = 512 * (c0 + 1) - 128 * jt
                pj = jt  # dedicated per-row softmax state, no reuse
                ve.wait_ge(sPE, LG[(jt, c0)])
                if jt in FULL:
                    ve.wait_ge(sPE, LG[(jt, c0 + 1)])
                    inc(
                        ve.reduce_max(
                            macc[:, pj, c0 : c0 + 2],
                            bank_run_ap(gbank[(jt, c0)], 2),
                            mybir.AxisListType.X,
                        )
                    )
                    assert ndv == DRED[jt]
                    ve.wait_ge(sDV, DRED[jt])
                    inc(
                        ve.reduce_max(
                            negmax[:, pj : pj + 1],
                            macc[:, pj, c0:NCH],
                            mybir.AxisListType.X, negate=True,
                        )
                    )
                elif nch > 1:
                    inc(
                        ve.reduce_max(
                            macc[:, pj, c0 : c0 + 1],
                            bank_ap(gbank[(jt, c0)], w0),
                            mybir.AxisListType.X,
                        )
                    )
                    assert ndv == DRED[jt]
                    for ric in range(c0 + 1, NCH):
                        ve.wait_ge(sPE, LG[(jt, ric)])
                        inc(
                            ve.reduce_max(
                                macc[:, pj, ric : ric + 1],
                                bank_ap(gbank[(jt, ric)], 512),
                                mybir.AxisListType.X,
                            )
                        )
                        assert ndv == REDG[(jt, ric)]
                    ve.wait_ge(sDV, REDG[(jt, NCH - 1)])
                    inc(
                        ve.reduce_max(
                            negmax[:, pj : pj + 1],
                            macc[:, pj, c0:NCH],
                            mybir.AxisListType.X, negate=True,
                        )
                    )
                else:
                    inc(
                        ve.reduce_max(
                            negmax[:, pj : pj + 1],
                            bank_ap(gbank[(jt, c0)], w0),
                            mybir.AxisListType.X, negate=True,
                        )
                    )
                assert ndv == NMX[jt]
                for j in doc1_due.get(jt, []):
                    ve.wait_ge(sPE, RG[j])
                    if j >= 4:
                        ve.wait_ge(sST, oc1_st_wait(j))
                    inc(
                        ve.tensor_scalar_add(
                            out=outst[:, (j // 2) % 2, j % 2, :],
                            in0=psRD[:, j % 2, :],
                            scalar1=0.0,
                        )
                    )
                    assert ndv == DOC1[j]
                if jt >= 1:
                    den_chain(jt - 1)
                for pm in docp_due.get(jt, []):
                    ve.wait_ge(sPE, RG[2 * pm + 1])
                    if pm >= 2:
                        ve.wait_ge(sST, st_thr(pm - 2))
                    inc(
                        ve.tensor_scalar_add(
                            out=outst[:, pm % 2, :, :], in0=psRD[:, :, :],
                            scalar1=0.0,
                        )
                    )
                    assert ndv == DOCP[pm]
            den_chain(NT - 1)
            for pm in DOCP_TRAIL:
                ve.wait_ge(sPE, RG[2 * pm + 1])
                if pm >= 2:
                    ve.wait_ge(sST, st_thr(pm - 2))
                inc(
                    ve.tensor_scalar_add(
                        out=outst[:, pm % 2, :, :], in0=psRD[:, :, :],
                        scalar1=0.0,
                    )
                )
                assert ndv == DOCP[pm]
            for j in DOC1_TRAIL:
                ve.wait_ge(sPE, RG[j])
                if j >= 4:
                    ve.wait_ge(sST, oc1_st_wait(j))
                inc(
                    ve.tensor_scalar_add(
                        out=outst[:, (j // 2) % 2, j % 2, :],
                        in0=psRD[:, j % 2, :],
                        scalar1=0.0,
                    )
                )
                assert ndv == DOC1[j]

        @block.scalar
        def _(ac_):
            ac_.wait_ge(sLc, 48)
            nac = 0
            for ic in range(NCH):
                for wsel, g_tab, bias in ((0, KG, bkc), (1, QG, bqc)):
                    dst = kT if wsel == 0 else qT
                    for kt in range(ND):
                        seq = KQSEQ[("k" if wsel == 0 else "q", kt, ic)]
                        ac_.wait_ge(sPE, g_tab[(kt, ic)])
                        ac_.activation(
                            out=dst[:, kt, 512 * ic : 512 * (ic + 1)],
                            in_=pse[:, 2 + ((seq - 1) % 4), :],
                            func=AFT.Identity,
                            bias=bias[:, kt : kt + 1],
                            scale=1.0,
                        ).then_inc(sAC, 1)
                        nac += 1
            assert nac == 2 * KQ

            def outcopy(pm):
                nonlocal nac
                ac_.wait_ge(sPE, RG[2 * pm + 1])
                if pm >= 2:
                    ac_.wait_ge(sST, st_thr(pm - 2))
                nac += 1
                ac_.activation(
                    out=outst[:, pm % 2, :, :], in_=psRD[:, :, :], func=AFT.Copy
                ).then_inc(sAC, 1)
                assert nac == AOCP[pm]

            def outcopy1(j):
                nonlocal nac
                ac_.wait_ge(sPE, RG[j])
                if j >= 4:
                    ac_.wait_ge(sST, oc1_st_wait(j))
                nac += 1
                ac_.activation(
                    out=outst[:, (j // 2) % 2, j % 2, :],
                    in_=psRD[:, j % 2, :],
                    func=AFT.Copy,
                ).then_inc(sAC, 1)
                assert nac == AOC1[j]

            def act_vp8(rj):
                nonlocal nac
                rp = rj
                ac_.wait_ge(sDV, RECIP[rj])
                nac += 1
                ac_.activation(
                    out=vp8[:, rj // 2, rj % 2, :],
                    in_=v_sb[:, rj, :],
                    func=AFT.Copy,
                    bias=0.0,
                    scale=rec[:, rp : rp + 1],
                ).then_inc(sAC, 1)
                assert nac == VP8A[rj]

            for jt in range(NT):
                c0 = _c0(jt)
                pj = jt  # dedicated per-row softmax state, no reuse
                m = jt // 2
                base = BASE[m]
                for pm in aocp_due.get(jt, []):
                    outcopy(pm)
                ac_.wait_ge(sDV, NMX[jt])
                if jt in FULL:
                    # one exp over the adjacent diag+nondiag bank pair
                    nac += 1
                    ac_.activation(
                        out=e8s[m][
                            :, jt % 2, 512 * c0 - base : 512 * (c0 + 2) - base
                        ],
                        in_=bank_run_ap(gbank[(jt, c0)], 2),
                        func=AFT.Exp,
                        bias=negmax[:, pj : pj + 1],
                        scale=1.0,
                        accum_out=dparts[:, pj, 0:1],
                    ).then_inc(sAC, 1)
                    assert nac == EXPG[(jt, 0)]
                else:
                    # exps: diag first (frees the rotation bank soonest)
                    w0 = 512 * (c0 + 1) - 128 * jt
                    nac += 1
                    ac_.activation(
                        out=e8s[m][
                            :, jt % 2, 128 * jt - base : 512 * (c0 + 1) - base
                        ],
                        in_=bank_ap(gbank[(jt, c0)], w0),
                        func=AFT.Exp,
                        bias=negmax[:, pj : pj + 1],
                        scale=1.0,
                        accum_out=dparts[:, pj, 0:1],
                    ).then_inc(sAC, 1)
                    assert nac == EXPG[(jt, 0)]
                    sidx = 1
                    for run in row_groups[jt]:
                        a = 512 * run[0]
                        bcol = 512 * (run[-1] + 1)
                        nac += 1
                        ac_.activation(
                            out=e8s[m][:, jt % 2, a - base : bcol - base],
                            in_=bank_run_ap(gbank[(jt, run[0])], len(run)),
                            func=AFT.Exp,
                            bias=negmax[:, pj : pj + 1],
                            scale=1.0,
                            accum_out=dparts[:, pj, sidx : sidx + 1],
                        ).then_inc(sAC, 1)
                        assert nac == EXPG[(jt, sidx)]
                        sidx += 1
                    assert sidx == NOPS[jt]
                if jt >= 1 and (jt - 1) % 2 == 1:
                    act_vp8(jt - 1)
                for j in aoc1_due.get(jt, []):
                    outcopy1(j)
            if (NT - 1) % 2 == 1:
                act_vp8(NT - 1)
            for pm in AOCP_TRAIL:
                outcopy(pm)
            for j in AOC1_TRAIL:
                outcopy1(j)

        @block.gpsimd
        def _(po):
            npo = 0
            for m in range(NP):
                zw = 384 if m == 5 else 128
                po.memset(e8s[m][:, 1, 0:zw], 0.0).then_inc(sPO, 1)
                npo += 1
                po.memset(vp8[:, m, 1, :], 0.0).then_inc(sPO, 1)
                npo += 1
            assert npo == NMEMSET

    nc.finalize()
    return nc


def _host_inputs(xb, wq16, wk16, wv8h, bq, bk, bv, T):
    # mask class 3: cols x < 384 + p get MASKVAL; class c uses the slice
    # [512-128*(c+1) : 512]
    p = np.arange(128, dtype=np.float32)[:, None]
    xx = np.arange(512, dtype=np.float32)[None, :]
    msk = np.where(xx >= 384.0 + p, 0.0, MASKVAL).astype(np.float16)
    idn = np.eye(128, dtype=np.float16)

    xT = np.ascontiguousarray(xb.T)  # [D, T] f32
    x16 = xT.astype(np.float16)
    x8 = np.ascontiguousarray(
        xT.reshape(2, 2, 128, T).transpose(2, 0, 1, 3)
    ).astype(ml_dtypes.float8_e4m3fn)
    return dict(
        x16=x16,
        x8=np.ascontiguousarray(x8),
        wq16=wq16,
        wk16=wk16,
        wv8=wv8h,
        bq=bq,
        bk=bk,
        bv=bv,
        msk16=np.ascontiguousarray(msk),
        idn16=np.ascontiguousarray(idn),
    )


def kernel(x, Wk, bk, Wq, bq, Wv, bv):
    global LAST_RESULTS
    T = 2048
    x = np.ascontiguousarray(np.asarray(x, dtype=np.float32))
    Wk = np.asarray(Wk, dtype=np.float32)
    Wq = np.asarray(Wq, dtype=np.float32)
    Wv = np.asarray(Wv, dtype=np.float32)
    bk = np.ascontiguousarray(np.asarray(bk, dtype=np.float32))
    bq = np.ascontiguousarray(np.asarray(bq, dtype=np.float32))
    bv = np.ascontiguousarray(np.asarray(bv, dtype=np.float32))

    wq16 = np.ascontiguousarray(Wq.T).astype(np.float16)
    wk16 = np.ascontiguousarray(Wk.T).astype(np.float16)
    wvT = Wv.T * np.float32(INV_SQRT_K)  # [D, KS]
    wv8h = np.ascontiguousarray(
        wvT.reshape(2, 2, 128, KS).transpose(2, 0, 1, 3)
    ).astype(ml_dtypes.float8_e4m3fn)
    bvs = np.ascontiguousarray(bv * np.float32(INV_SQRT_K))

    nc = build_nc(T)
    in_maps = [_host_inputs(x[b], wq16, wk16, wv8h, bq, bk, bvs, T) for b in range(B)]
    res = None
    read = None
    last_exc = None
    for attempt in range(4):
        try:
            res = run_bass_kernel_spmd(nc, in_maps, list(range(B)), trace=TRACE)
            read = np.stack(
                [
                    np.asarray(res.results[b]["out"]).astype(np.float32)
                    for b in range(B)
                ],
                axis=0,
            )
            if np.isfinite(read).all():
                break
            read = None  # flaky device run produced NaN/inf; retry
        except Exception as e:  # transient NRT device errors; retry fresh
            last_exc = e
        import time as _time
        _time.sleep(5)
        nc = build_nc(T)
    if read is None:
        if last_exc is not None:
            raise last_exc
        raise RuntimeError("device produced non-finite output on all attempts")
    LAST_RESULTS = res
    return (x + read).astype(np.float32)
